# revision 12
# baseline (speedup 1.0000x reference)
"""Trainium2 Bass kernel for AdaNSABlock (7x7 neighborhood attention block).

Sharding: 8 cores = batch(4) x row-halves(2). Each core computes 16 image
rows (512 tokens) of one sample, reading 19 rows (3-row halo) of input
padded to 640 tokens (5 clean 128-token tiles). Bottom halves are
row-flipped on host so all cores run one SPMD graph.

v2 rewrite vs v1:
  - tokens padded to 640: all tiles 128 wide, key chunks = absolute
    128-aligned token tiles (no dedup, V computed once per tile)
  - tile-major S: per (quad, key-tile) one wide-N matmul per head slot
    (split only at PSUM bank boundaries) -> ~52 S matmuls vs 88, far
    fewer LDWEIGHTS
  - one merged exp ACTIVATE per (quad, tile) over [128, 4*qspan]
  - one E-multiply per (quad, tile); small tiles on GpSimd, rest Vector
  - AV per unit as 4 sequential accumulation chains (M=33 a/b col-tiles)
  - softmax denominators: reciprocal_approx_fast straight off PSUM, then
    DMA partition-broadcast (stride-0 src) instead of PE broadcast matmuls
  - input DMAs split across sync/scalar/gpsimd queues by need-time
"""

import numpy as np
import ml_dtypes

KS = 7
HEADS = 8
DIM = 256
HID = 1024
HD = 32
H = 32
W = 32
NT = 19 * 32          # valid local tokens incl halo
NTP = 640             # padded tokens (5 tiles of 128)
NQ = 512              # query tokens per core
EPS = 1e-5
BF16 = ml_dtypes.bfloat16

# tile-major schedule: per key-tile t -> (qbase, qspan)
TILE_Q = [(0, 256), (0, 384), (128, 384), (256, 256), (384, 128)]
# per query group g -> key tiles used
G_TILES = [[0, 1], [0, 1, 2], [1, 2, 3], [2, 3, 4]]
# units finishing at tile t
LAST_UNITS = {1: [0], 2: [1], 3: [2], 4: [3]}
E_NCOL = sum(4 * qs for (_, qs) in TILE_Q) * 2   # 11264

MAGIC = 0x5F3759DF

_CACHE = {}


def _bf(x):
    return np.ascontiguousarray(np.asarray(x, np.float32).astype(BF16))


def _f32(x):
    return np.ascontiguousarray(np.asarray(x, np.float32))


# --------------------------------------------------------------------------
# Host-side folding + E construction
# --------------------------------------------------------------------------

def _fold_weights(inp):
    quality = inp['quality']
    s = int(quality) - 1
    l = float(quality % 1)
    g1 = np.abs(np.asarray(inp['gamma_1'], np.float64))
    g2 = np.abs(np.asarray(inp['gamma_2'], np.float64))
    if s == g1.shape[0] - 1:
        G1, G2 = g1[s], g2[s]
    else:
        G1 = g1[s] ** (1 - l) * g1[s + 1] ** l
        G2 = g2[s] ** (1 - l) * g2[s + 1] ** l

    qkv_w = np.asarray(inp['qkv_w'], np.float64)
    qkv_b = np.asarray(inp['qkv_b'], np.float64)
    n1w = np.asarray(inp['norm1_w'], np.float64)
    n1b = np.asarray(inp['norm1_b'], np.float64)
    Wq = qkv_w * n1w[None, :]
    bq = qkv_b + qkv_w @ n1b
    sc = HD ** -0.5
    Wq[:DIM] *= sc
    bq[:DIM] *= sc

    pw = G1[:, None] * np.asarray(inp['proj_w'], np.float64)
    pb = G1 * np.asarray(inp['proj_b'], np.float64)

    n2w = np.asarray(inp['norm2_w'], np.float64)
    n2b = np.asarray(inp['norm2_b'], np.float64)
    f1w = np.asarray(inp['fc1_w'], np.float64) * n2w[None, :]
    f1b = np.asarray(inp['fc1_b'], np.float64) + np.asarray(inp['fc1_w'], np.float64) @ n2b
    f2w = G2[:, None] * np.asarray(inp['fc2_w'], np.float64)
    f2b = G2 * np.asarray(inp['fc2_b'], np.float64)

    Wv = Wq[2 * DIM:]
    bv = bq[2 * DIM:]
    # V'' pair p cols [66p..66p+66): [32 a-dims][one_a][32 b-dims][one_b]
    Wv_aug = np.zeros((DIM, 264))
    vb_aug = np.zeros(264)
    for p in range(4):
        ha, hb = 2 * p, 2 * p + 1
        base = 66 * p
        Wv_aug[:, base:base + 32] = Wv[32 * ha:32 * ha + 32].T
        vb_aug[base:base + 32] = bv[32 * ha:32 * ha + 32]
        vb_aug[base + 32] = 1.0
        Wv_aug[:, base + 33:base + 65] = Wv[32 * hb:32 * hb + 32].T
        vb_aug[base + 33:base + 65] = bv[32 * hb:32 * hb + 32]
        vb_aug[base + 65] = 1.0

    def kblocked(wT, kb):
        n = wT.shape[1]
        return np.ascontiguousarray(wT.reshape(kb, 128, n).transpose(1, 0, 2))

    # smallw bf16 [1, 776]: vb(264) | pb(256) | f2b(256)
    smallw = np.zeros((1, 776))
    smallw[0, 0:264] = vb_aug
    smallw[0, 264:520] = pb
    smallw[0, 520:776] = f2b

    # psel f32 [1, 194]: rows-0..63 selector | rows-64..96 selector
    psel = np.zeros((1, 194))
    psel[0, 0:64] = 1.0
    psel[0, 97 + 64:97 + 97] = 1.0

    # cbias f32 [128, 12]: qkb (q0,q1,k0,k1) | f1b (8 cols)
    cbias = np.zeros((128, 12))
    for mt in range(4):
        cbias[:, mt] = bq[128 * mt:128 * mt + 128]
    for mh in range(8):
        cbias[:, 4 + mh] = f1b[128 * mh:128 * mh + 128]

    # proj lhsT from attnT pair tiles [97, 128]: rows 0-31 head 2p (den row
    # 32 zero-weighted), rows 64-95 head 2p+1 (den row 96)
    pwT_aug = np.zeros((4, 97, DIM))
    for p in range(4):
        ha, hb = 2 * p, 2 * p + 1
        pwT_aug[p, 0:32] = pw[:, 32 * ha:32 * ha + 32].T
        pwT_aug[p, 64:96] = pw[:, 32 * hb:32 * hb + 32].T

    return dict(
        wqk=_bf(kblocked(Wq[:512].T, 2)),       # [128, 2, 512]
        wv=_bf(kblocked(Wv_aug, 2)),            # [128, 2, 264]
        f1w=_bf(kblocked(f1w.T, 2)),            # [128, 2, 1024]
        f2w=_bf(kblocked(f2w.T, 8)),            # [128, 8, 256]
        pwT=_bf(np.ascontiguousarray(pwT_aug.transpose(1, 0, 2))),  # [97, 4, 256]
        smallw=_bf(smallw),                     # [1, 776]
        psel=_f32(psel),                        # [1, 194]
        cbias=_f32(cbias),                      # [128, 12]
        rpb=np.asarray(inp['rpb'], np.float64),
    )


def _build_E(rpb, flip):
    """Tile-major E: [128 keys, (quad, t, slot, qspan)] -> [128, 11264] f32."""
    def img_row(r):
        return (31 - r) if flip else r
    blocks = []
    for quad in range(2):
        for t in range(5):
            qbase, qspan = TILE_Q[t]
            kk = np.arange(128)[:, None, None]
            ss = np.arange(4)[None, :, None]
            qq = qbase + np.arange(qspan)[None, None, :]
            tk = 128 * t + kk
            ki_loc, kj = tk // 32, tk % 32
            qi_loc, qj = qq // 32, qq % 32
            qi = img_row(qi_loc)
            ki = img_row(np.minimum(ki_loc, 19))
            sh = np.clip(qi - 3, 0, H - KS)
            sw = np.clip(qj - 3, 0, W - KS)
            valid = (tk < NT) & (ki >= sh) & (ki < sh + KS) \
                & (kj >= sw) & (kj < sw + KS)
            bh = np.clip(ki - qi + KS - 1, 0, 2 * KS - 2)
            bw = np.clip(kj - qj + KS - 1, 0, 2 * KS - 2)
            h = 4 * quad + ss
            bias = rpb[h, bh, bw]
            E = np.where(valid, np.exp(bias), 0.0)
            blocks.append(np.ascontiguousarray(E.reshape(128, 4 * qspan)))
    return np.concatenate(blocks, axis=1)


def _prepare_inputs(inp):
    F = _fold_weights(inp)
    E_top = _bf(_build_E(F['rpb'], flip=False))
    E_bot = _bf(_build_E(F['rpb'], flip=True))
    x = np.asarray(inp['x'], np.float32)
    Bsz = x.shape[0]
    shared = {k: v for k, v in F.items() if k != 'rpb'}
    in_maps = []
    for b in range(Bsz):
        for half in range(2):
            if half == 0:
                x_loc = x[b, 0:19].reshape(NT, DIM)
                E = E_top
            else:
                x_loc = x[b, 31:12:-1].reshape(NT, DIM)
                E = E_bot
            x_pad = np.zeros((NTP, DIM), np.float32)
            x_pad[:NT] = x_loc
            m = dict(shared)
            m['x'] = x_pad
            m['Eall'] = E
            in_maps.append(m)
    return in_maps


# --------------------------------------------------------------------------
# Bass kernel graph
# --------------------------------------------------------------------------

def build_graph():
    import concourse.bass as bass
    import concourse.tile as tile
    import concourse.mybir as mybir
    from concourse import bacc
    from concourse.masks import make_identity

    dt = mybir.dt
    Alu = mybir.AluOpType
    Act = mybir.ActivationFunctionType

    nc = bacc.Bacc()

    def param(name, shape, dtype, out=False):
        return nc.declare_dram_parameter(name, list(shape), dtype, isOutput=out)

    x_d = param("x", (NTP, DIM), dt.float32)
    wqk_d = param("wqk", (128, 2, 512), dt.bfloat16)
    wv_d = param("wv", (128, 2, 264), dt.bfloat16)
    f1w_d = param("f1w", (128, 2, HID), dt.bfloat16)
    f2w_d = param("f2w", (128, 8, DIM), dt.bfloat16)
    pwT_d = param("pwT", (97, 4, DIM), dt.bfloat16)
    smallw_d = param("smallw", (1, 776), dt.bfloat16)
    psel_d = param("psel", (1, 194), dt.float32)
    cbias_d = param("cbias", (128, 12), dt.float32)
    Eall_d = param("Eall", (128, E_NCOL), dt.bfloat16)
    out_d = param("out", (NQ, DIM), dt.float32, out=True)

    def bcast_ap(src, nparts):
        """Partition-broadcast AP: repeat a [1, n] AP across nparts."""
        return bass.AP(tensor=src.tensor, offset=src.offset,
                       ap=[[0, nparts]] + list(src.ap[1:]))

    with tile.TileContext(nc) as tc:
        with (
            tc.tile_pool(name="consts", bufs=1) as consts,
            tc.tile_pool(name="persist", bufs=1) as persist,
            tc.tile_pool(name="work", bufs=3) as work,
            tc.tile_pool(name="aqp", bufs=2) as aqp,
            tc.tile_pool(name="psS", bufs=2, space="PSUM") as psS,
            tc.tile_pool(name="psav", bufs=2, space="PSUM") as psav,
        ):
            # ---------------- input DMAs ---------------------------------
            cbias_sb = consts.tile([128, 12], dt.float32, tag="cbias")
            nc.sync.dma_start(out=cbias_sb, in_=cbias_d[:])
            xs = persist.tile([128, 5, DIM], dt.float32, tag="xs")
            nc.sync.dma_start(
                out=xs[:, 0:2, :],
                in_=x_d[0:256, :].rearrange("(t p) c -> p t c", p=128))
            nc.sync.dma_start(
                out=xs[:, 2:5, :],
                in_=x_d[256:640, :].rearrange("(t p) c -> p t c", p=128))
            wqk_sb = consts.tile([128, 2, 512], dt.bfloat16, tag="wqk")
            nc.sync.dma_start(out=wqk_sb, in_=wqk_d[:])
            Eall_sb = consts.tile([128, E_NCOL], dt.bfloat16, tag="Eall")
            nc.sync.dma_start(out=Eall_sb[:, :E_NCOL // 2],
                              in_=Eall_d[:, :E_NCOL // 2])
            nc.sync.dma_start(out=Eall_sb[:, E_NCOL // 2:],
                              in_=Eall_d[:, E_NCOL // 2:])

            smallw_sb = consts.tile([1, 776], dt.bfloat16, tag="smallw")
            nc.sync.dma_start(out=smallw_sb, in_=smallw_d[:])
            psel_sb = consts.tile([1, 194], dt.float32, tag="psel")
            nc.sync.dma_start(out=psel_sb, in_=psel_d[:])
            wv_sb = consts.tile([128, 2, 264], dt.bfloat16, tag="wv")
            nc.sync.dma_start(out=wv_sb, in_=wv_d[:])
            pwT_sb = consts.tile([97, 4, DIM], dt.bfloat16, tag="pwT")
            nc.sync.dma_start(out=pwT_sb, in_=pwT_d[:])

            f1w_sb = consts.tile([128, 2, HID], dt.bfloat16, tag="f1w")
            nc.gpsimd.dma_start(out=f1w_sb, in_=f1w_d[:])
            f2w_sb = consts.tile([128, 8, DIM], dt.bfloat16, tag="f2w")
            nc.gpsimd.dma_start(out=f2w_sb, in_=f2w_d[:])

            vb_sb = smallw_sb[0:1, 0:264]
            pb_sb = smallw_sb[0:1, 264:520]
            f2b_sb = smallw_sb[0:1, 520:776]
            qkb_sb = cbias_sb[:, 0:4]
            f1b_sb = cbias_sb[:, 4:12]

            # E block views [128, 4, qspan]
            E_view = {}
            off = 0
            for quad in range(2):
                for t in range(5):
                    qspan = TILE_Q[t][1]
                    E_view[(quad, t)] = Eall_sb[:, off:off + 4 * qspan] \
                        .rearrange("p (s q) -> p s q", s=4)
                    off += 4 * qspan

            ident = consts.tile([128, 128], dt.bfloat16, tag="ident")
            make_identity(nc, ident)
            # per-slot zero-masked K tiles: full-128-K S matmuls avoid
            # concurrent row-group writes to one PSUM bank (HW collision)
            kTm = [[persist.tile([128, NTP], dt.bfloat16, tag=f"kTm{q}_{s}",
                                 name=f"kTm{q}_{s}") for s in range(4)]
                   for q in range(2)]
            for q in range(2):
                for s in range(4):
                    nc.gpsimd.memset(kTm[q][s], 0.0)
            ones_sb = consts.tile([1, 512], dt.bfloat16, tag="ones")
            nc.vector.memset(ones_sb, 1.0)
            # ACT table preload (Exp) while DMAs land
            idummy = work.tile([1, 2], dt.float32, tag="idummy")
            nc.scalar.activation(out=idummy, in_=ones_sb[0:1, 0:2], func=Act.Exp)

            # ---------------- PE keepalive (HAM warm-up) ------------------
            def keepalive(n, nk=128):
                ka = psS.tile([128, 4, 384], dt.float32, tag="psS", name="ka")
                kaf = ka.rearrange("p a b -> p (a b)")
                for _ in range(n):
                    nc.tensor.matmul(kaf[:, :nk], ident, ident[:, :nk],
                                     start=True, stop=True)

            keepalive(24)

            # ---------------- helpers ----------------
            def dve_rsqrt(dst, src, n):
                ve = work.tile([128, n], dt.float32, tag="rsq_ve", bufs=2)
                nc.vector.tensor_scalar(out=ve, in0=src, scalar1=float(EPS),
                                        scalar2=None, op0=Alu.add)
                yi = work.tile([128, n], dt.int32, tag="rsq_yi", bufs=2)
                nc.vector.tensor_scalar(out=yi, in0=ve[:].bitcast(dt.int32),
                                        scalar1=1, scalar2=None,
                                        op0=Alu.logical_shift_right)
                nc.vector.tensor_scalar(out=yi, in0=yi, scalar1=-1,
                                        scalar2=MAGIC, op0=Alu.mult, op1=Alu.add)
                y = yi[:].bitcast(dt.float32)
                t = work.tile([128, n], dt.float32, tag="rsq_t", bufs=2)
                for _ in range(2):
                    nc.vector.tensor_tensor(out=t, in0=y, in1=y, op=Alu.mult)
                    nc.vector.tensor_tensor(out=t, in0=t, in1=ve, op=Alu.mult)
                    nc.vector.tensor_scalar(out=t, in0=t, scalar1=-0.5,
                                            scalar2=1.5, op0=Alu.mult, op1=Alu.add)
                    nc.vector.tensor_tensor(out=y, in0=y, in1=t, op=Alu.mult)
                nc.vector.tensor_copy(out=dst, in_=y)

            def ln_stats(x_list, tagp):
                """Returns (rstd, negmr) [128, ntile] f32 tiles."""
                ntile = len(x_list)
                mv = work.tile([128, 2 * ntile], dt.float32, tag=tagp + "_mv", bufs=2)
                for t, xt in enumerate(x_list):
                    stats = work.tile([128, 6], dt.float32, tag=tagp + "_st", bufs=2)
                    nc.vector.bn_stats(out=stats, in_=xt)
                    nc.vector.bn_aggr(out=mv[:, 2 * t:2 * t + 2], in_=stats)
                rstd = work.tile([128, ntile], dt.float32, tag=tagp + "_rs", bufs=2)
                dve_rsqrt(rstd, mv[:, 1::2], ntile)
                negmr = work.tile([128, ntile], dt.float32, tag=tagp + "_nm", bufs=2)
                nc.vector.scalar_tensor_tensor(out=negmr, in0=mv[:, 0::2],
                                               scalar=-1.0, in1=rstd,
                                               op0=Alu.mult, op1=Alu.mult)
                return rstd, negmr

            def transpose_tile(dst_cblocks, xh, col):
                """PE-transpose xh [128, 256] bf16 into dst cblock tiles at
                column offset col (128 wide)."""
                for cb in range(2):
                    tp = psav.tile([128, 512], dt.float32, tag="pav", name=f"tp{col}_{cb}")
                    ptb = tp[:, 0:64].bitcast(dt.bfloat16)
                    nc.tensor.transpose(ptb, xh[:, 128 * cb:128 * (cb + 1)], ident)
                    nc.scalar.activation(out=dst_cblocks[cb][:, col:col + 128],
                                         in_=ptb, func=Act.Copy)

            # ---------------- LN1 + transpose + QKV + V ------------------
            xhatT = [persist.tile([128, NTP], dt.bfloat16, tag=f"xhatT{cb}",
                                  name=f"xhatT{cb}") for cb in range(2)]

            x_t = [xs[:, t, :] for t in range(5)]
            rs1a, nm1a = ln_stats(x_t[:2], "ln1a")
            xh_t = []
            for t in range(2):
                xh = work.tile([128, DIM], dt.bfloat16, tag=f"xh{t}", bufs=1)
                nc.scalar.activation(out=xh, in_=x_t[t], func=Act.Identity,
                                     scale=rs1a[:, t:t + 1], bias=nm1a[:, t:t + 1])
                xh_t.append(xh)
                transpose_tile(xhatT, xh, 128 * t)

            rs1b, nm1b = ln_stats(x_t[2:], "ln1b")
            for t in range(2, 5):
                xh = work.tile([128, DIM], dt.bfloat16, tag=f"xh{t}", bufs=1)
                nc.scalar.activation(out=xh, in_=x_t[t], func=Act.Identity,
                                     scale=rs1b[:, t - 2:t - 1], bias=nm1b[:, t - 2:t - 1])
                xh_t.append(xh)
                transpose_tile(xhatT, xh, 128 * t)

            # QKV: mt0 -> qT[0], mt1 -> qT[1], mt2 -> kT[0], mt3 -> kT[1]
            qT = [persist.tile([128, NQ], dt.bfloat16, tag=f"qT{i}", name=f"qT{i}")
                  for i in range(2)]
            kT = [persist.tile([128, NTP], dt.bfloat16, tag=f"kT{i}", name=f"kT{i}")
                  for i in range(2)]

            qkps = [psS.tile([128, 4, 384], dt.float32, tag="psS", name=f"qkps{i}")
                    .rearrange("p a b -> p (a b)") for i in range(2)]

            def qkv_chunk(mt, n0, nn):
                # qkps[0] holds mt0 (flat 0:512) + mt2 (512:1152);
                # qkps[1] holds mt1 + mt3. All chunk regions stay in-bank.
                dst = (qT if mt < 2 else kT)[mt % 2]
                base = 0 if mt < 2 else 512
                pt = qkps[mt % 2][:, base + n0:base + n0 + nn]
                for kb in range(2):
                    nc.tensor.matmul(pt, wqk_sb[:, kb, 128 * mt:128 * (mt + 1)],
                                     xhatT[kb][:, n0:n0 + nn],
                                     start=(kb == 0), stop=(kb == 1))
                nc.scalar.activation(out=dst[:, n0:n0 + nn], in_=pt,
                                     func=Act.Identity, bias=qkb_sb[:, mt:mt + 1])

            # early chunks for quad0 (needs T1 tiles 0,1 only)
            def kmask(quad, n0, nn):
                for s in range(4):
                    nc.gpsimd.tensor_copy(
                        out=kTm[quad][s][32 * s:32 * s + 32, n0:n0 + nn],
                        in_=kT[quad][32 * s:32 * s + 32, n0:n0 + nn])

            qkv_chunk(0, 0, 256)
            qkv_chunk(2, 0, 256)
            kmask(0, 0, 256)
            # rest
            qkv_chunk(0, 256, 256)
            qkv_chunk(2, 256, 256)
            kmask(0, 256, 256)
            qkv_chunk(2, 512, 128)
            kmask(0, 512, 128)
            qkv_chunk(1, 0, 512)
            qkv_chunk(3, 0, 512)
            kmask(1, 0, 512)
            qkv_chunk(3, 512, 128)
            kmask(1, 512, 128)

            # V per token tile: [128 tok, 264]
            vt = []
            for t in range(5):
                pv = psav.tile([128, 512], dt.float32, tag="pav", name=f"pv{t}")
                pvs = pv[:, 0:264]
                for kb in range(2):
                    nc.tensor.matmul(pvs, xhatT[kb][:, 128 * t:128 * (t + 1)],
                                     wv_sb[:, kb, :], start=(kb == 0), stop=False)
                nc.tensor.matmul(pvs, ones_sb[:, :128], vb_sb,
                                 start=False, stop=True)
                v = persist.tile([128, 264], dt.bfloat16, tag=f"vt{t}", name=f"vt{t}")
                nc.vector.tensor_copy(out=v, in_=pvs)
                vt.append(v)

            # ---------------- attention ----------------
            attnP2 = [persist.tile([97, 2, NQ], dt.bfloat16, tag=f"attnP{q}",
                                   name=f"attnP{q}") for q in range(2)]

            def split_banks(start, n):
                """Split f32 col range [start, start+n) at 512-boundaries."""
                pieces = []
                cur = start
                end = start + n
                while cur < end:
                    nxt = min(end, (cur // 512 + 1) * 512)
                    pieces.append((cur - start, nxt - cur))
                    cur = nxt
                return pieces

            def unit_av(quad, g, aq_tiles):
                pav = psav.tile([128, 512], dt.float32, tag="pav",
                                name=f"pav{quad}_{g}")
                tiles = G_TILES[g]
                for pl in range(2):
                    p = 2 * quad + pl
                    for sub in range(2):   # 0 = a-heads, 1 = b-heads
                        vlo = 66 * p + 33 * sub
                        po = 64 * sub
                        for i, t in enumerate(tiles):
                            qbase, qspan = TILE_Q[t]
                            qoff = 128 * g - qbase
                            nc.tensor.matmul(
                                pav[po:po + 33, 128 * pl:128 * (pl + 1)],
                                vt[t][:, vlo:vlo + 33],
                                aq_tiles[t][:, 2 * pl + sub, qoff:qoff + 128],
                                start=(i == 0), stop=(i == len(tiles) - 1))
                # denominators at rows 32 (a) / 96 (b), cols 0:256
                den = work.tile([1, 512], dt.float32, tag="den", bufs=3)
                if quad == 0:
                    nc.vector.tensor_copy(out=den[:, 0:256], in_=pav[32:33, 0:256])
                    nc.vector.tensor_copy(out=den[:, 256:512], in_=pav[96:97, 0:256])
                else:
                    nc.scalar.activation(out=den[:, 0:256], in_=pav[32:33, 0:256],
                                         func=Act.Copy)
                    nc.scalar.activation(out=den[:, 256:512], in_=pav[96:97, 0:256],
                                         func=Act.Copy)
                rcd = work.tile([1, 512], dt.float32, tag="rcd", bufs=3)
                nc.vector.reciprocal_approx_fast(out=rcd, in_=den)
                # PE broadcast of reciprocals across partitions (f32 matmuls)
                for pl in range(2):
                    pB = pav[:97, 256 + 128 * pl:384 + 128 * pl]
                    nc.tensor.matmul(pB, psel_sb[0:1, 0:97],
                                     rcd[:, 128 * pl:128 * (pl + 1)],
                                     start=True, stop=False)
                    nc.tensor.matmul(pB, psel_sb[0:1, 97:194],
                                     rcd[:, 256 + 128 * pl:384 + 128 * pl],
                                     start=False, stop=True)
                rcb = work.tile([97, 256], dt.bfloat16, tag="rcb", bufs=3)
                rcbeng = nc.vector if quad == 0 else nc.scalar
                if quad == 0:
                    nc.vector.tensor_copy(out=rcb, in_=pav[:97, 256:512])
                else:
                    nc.scalar.activation(out=rcb, in_=pav[:97, 256:512],
                                         func=Act.Copy)
                nc.vector.tensor_tensor(
                    out=attnP2[quad][:, :, 128 * g:128 * (g + 1)],
                    in0=pav[:97, 0:256].rearrange("p (two q) -> p two q", two=2),
                    in1=rcb.rearrange("p (two q) -> p two q", two=2),
                    op=Alu.mult)

            def proj_tail(mt):
                """proj + residual for query tile mt (= group mt)."""
                pt = psav.tile([128, 512], dt.float32, tag="pav", name=f"pproj{mt}")
                pp = pt[:, :DIM]
                for p in range(4):
                    nc.tensor.matmul(pp,
                                     attnP2[p // 2][:, p % 2, 128 * mt:128 * (mt + 1)],
                                     pwT_sb[:, p, :], start=(p == 0), stop=False)
                nc.tensor.matmul(pp, ones_sb[:, :128], pb_sb, start=False, stop=True)
                yt = persist.tile([128, DIM], dt.float32, tag=f"y{mt}")
                nc.vector.tensor_tensor(out=yt, in0=pp, in1=x_t[mt], op=Alu.add)
                return yt

            y_tiles = [None] * 4
            aq_cur = {}
            for quad in range(2):
                for t in range(5):
                    qbase, qspan = TILE_Q[t]
                    pS = psS.tile([128, 4, 384], dt.float32, tag="psS",
                                  name=f"pS{quad}_{t}")
                    pSf = pS.rearrange("p a b -> p (a b)")
                    for s in range(4):
                        for (off, n) in split_banks(384 * s, qspan):
                            nc.tensor.matmul(
                                pSf[:, 384 * s + off:384 * s + off + n],
                                kTm[quad][s][:, 128 * t:128 * (t + 1)],
                                qT[quad][:, qbase + off:qbase + off + n],
                                start=True, stop=True)
                    aq = aqp.tile([128, 4, qspan], dt.bfloat16, tag=f"aq{t}",
                                  name=f"aq{quad}_{t}")
                    nc.scalar.activation(out=aq, in_=pS[:, :, :qspan], func=Act.Exp)
                    emeng = nc.gpsimd if t in (0, 4) else nc.vector
                    emeng.tensor_tensor(
                        out=aq.rearrange("p s q -> p (s q)"),
                        in0=aq.rearrange("p s q -> p (s q)"),
                        in1=E_view[(quad, t)].rearrange("p s q -> p (s q)"),
                        op=Alu.mult)
                    aq_cur[t] = aq
                    for g in LAST_UNITS.get(t, []):
                        unit_av(quad, g, aq_cur)
                        if quad == 1:
                            y_tiles[g] = proj_tail(g)

            # gelu table preload right after last exp
            gdummy = work.tile([1, 2], dt.float32, tag="gdummy")
            nc.scalar.activation(out=gdummy, in_=ones_sb[0:1, 0:2], func=Act.Gelu)

            # ---------------- LN2 + MLP ----------------
            x2T = [persist.tile([128, NQ], dt.bfloat16, tag=f"x2T{cb}",
                                name=f"x2T{cb}") for cb in range(2)]
            for half in range(2):
                ys = y_tiles[2 * half:2 * half + 2]
                rs2, nm2 = ln_stats(ys, f"ln2{half}")
                for i, yt in enumerate(ys):
                    xh2 = work.tile([128, DIM], dt.bfloat16,
                                    tag=f"xh2_{half}{i}", bufs=1)
                    nc.vector.tensor_scalar(out=xh2, in0=yt,
                                            scalar1=rs2[:, i:i + 1],
                                            scalar2=nm2[:, i:i + 1],
                                            op0=Alu.mult, op1=Alu.add)
                    transpose_tile(x2T, xh2, 128 * (2 * half + i))

            m1 = []
            for mh in range(8):
                if mh % 3 == 0:
                    f1ps = psS.tile([128, 4, 384], dt.float32, tag="psS",
                                    name=f"f1ps{mh}").rearrange("p a b -> p (a b)")
                pt = f1ps[:, 512 * (mh % 3):512 * (mh % 3) + 512]
                for kb in range(2):
                    nc.tensor.matmul(pt, f1w_sb[:, kb, 128 * mh:128 * (mh + 1)],
                                     x2T[kb], start=(kb == 0), stop=(kb == 1))
                mg = persist.tile([128, NQ], dt.bfloat16, tag=f"m1_{mh}")
                nc.scalar.activation(out=mg, in_=pt, func=Act.Gelu,
                                     bias=f1b_sb[:, mh:mh + 1], scale=1.0)
                m1.append(mg)

            for mt in range(4):
                pt = psav.tile([128, 512], dt.float32, tag="pav", name=f"pfc2{mt}")
                pp = pt[:, :DIM]
                for kb in range(8):
                    nc.tensor.matmul(pp, m1[kb][:, 128 * mt:128 * (mt + 1)],
                                     f2w_sb[:, kb, :], start=(kb == 0), stop=False)
                nc.tensor.matmul(pp, ones_sb[:, :128], f2b_sb, start=False, stop=True)
                ot = work.tile([128, DIM], dt.float32, tag="outt", bufs=2)
                nc.vector.tensor_tensor(out=ot, in0=pp, in1=y_tiles[mt][:],
                                        op=Alu.add)
                eng = nc.sync if mt % 2 == 0 else nc.gpsimd
                eng.dma_start(out=out_d[128 * mt:128 * (mt + 1), :], in_=ot)

    nc.finalize()
    return nc


# --------------------------------------------------------------------------
# Entry point
# --------------------------------------------------------------------------

def kernel(**inputs):
    from concourse.bass_utils import run_bass_kernel_spmd

    if 'nc' not in _CACHE:
        _CACHE['nc'] = build_graph()
    nc = _CACHE['nc']

    in_maps = _prepare_inputs(inputs)
    res = run_bass_kernel_spmd(nc, in_maps, core_ids=list(range(8)))
    x = np.asarray(inputs['x'])
    Bsz, Hh, Ww, C = x.shape
    out = np.zeros((Bsz, Hh, Ww, C), np.float32)
    for i in range(2 * Bsz):
        b, half = divmod(i, 2)
        o = np.asarray(res.results[i]['out']).reshape(16, Ww, C)
        if half == 0:
            out[b, 0:16] = o
        else:
            out[b, 16:32] = o[::-1]
    return out.astype(x.dtype)


# revision 13
# speedup vs baseline: 1.1198x; 1.1198x over previous
"""Trainium2 Bass kernel for AdaNSABlock (7x7 neighborhood attention block).

Sharding: 8 cores = batch(4) x row-halves(2). Each core computes 16 image
rows (512 tokens) of one sample, reading 19 rows (3-row halo) of input
padded to 640 tokens (5 clean 128-token tiles). Bottom halves are
row-flipped on host so all cores run one SPMD graph.

v2 rewrite vs v1:
  - tokens padded to 640: all tiles 128 wide, key chunks = absolute
    128-aligned token tiles (no dedup, V computed once per tile)
  - tile-major S: per (quad, key-tile) one wide-N matmul per head slot
    (split only at PSUM bank boundaries) -> ~52 S matmuls vs 88, far
    fewer LDWEIGHTS
  - one merged exp ACTIVATE per (quad, tile) over [128, 4*qspan]
  - one E-multiply per (quad, tile); small tiles on GpSimd, rest Vector
  - AV per unit as 4 sequential accumulation chains (M=33 a/b col-tiles)
  - softmax denominators: reciprocal_approx_fast straight off PSUM, then
    DMA partition-broadcast (stride-0 src) instead of PE broadcast matmuls
  - input DMAs split across sync/scalar/gpsimd queues by need-time
"""

import numpy as np
import ml_dtypes

KS = 7
HEADS = 8
DIM = 256
HID = 1024
HD = 32
H = 32
W = 32
NT = 19 * 32          # valid local tokens incl halo
NTP = 640             # padded tokens (5 tiles of 128)
NQ = 512              # query tokens per core
EPS = 1e-5
BF16 = ml_dtypes.bfloat16

# tile-major schedule: per key-tile t -> (qbase, qspan)
TILE_Q = [(0, 256), (0, 384), (128, 384), (256, 256), (384, 128)]
# per query group g -> key tiles used
G_TILES = [[0, 1], [0, 1, 2], [1, 2, 3], [2, 3, 4]]
# units finishing at tile t
LAST_UNITS = {1: [0], 2: [1], 3: [2], 4: [3]}
E_NCOL = sum(4 * qs for (_, qs) in TILE_Q) * 2   # 11264

MAGIC = 0x5F3759DF

_CACHE = {}


def _bf(x):
    return np.ascontiguousarray(np.asarray(x, np.float32).astype(BF16))


def _f32(x):
    return np.ascontiguousarray(np.asarray(x, np.float32))


# --------------------------------------------------------------------------
# Host-side folding + E construction
# --------------------------------------------------------------------------

def _fold_weights(inp):
    quality = inp['quality']
    s = int(quality) - 1
    l = float(quality % 1)
    g1 = np.abs(np.asarray(inp['gamma_1'], np.float64))
    g2 = np.abs(np.asarray(inp['gamma_2'], np.float64))
    if s == g1.shape[0] - 1:
        G1, G2 = g1[s], g2[s]
    else:
        G1 = g1[s] ** (1 - l) * g1[s + 1] ** l
        G2 = g2[s] ** (1 - l) * g2[s + 1] ** l

    qkv_w = np.asarray(inp['qkv_w'], np.float64)
    qkv_b = np.asarray(inp['qkv_b'], np.float64)
    n1w = np.asarray(inp['norm1_w'], np.float64)
    n1b = np.asarray(inp['norm1_b'], np.float64)
    Wq = qkv_w * n1w[None, :]
    bq = qkv_b + qkv_w @ n1b
    sc = HD ** -0.5
    Wq[:DIM] *= sc
    bq[:DIM] *= sc

    pw = G1[:, None] * np.asarray(inp['proj_w'], np.float64)
    pb = G1 * np.asarray(inp['proj_b'], np.float64)

    n2w = np.asarray(inp['norm2_w'], np.float64)
    n2b = np.asarray(inp['norm2_b'], np.float64)
    f1w = np.asarray(inp['fc1_w'], np.float64) * n2w[None, :]
    f1b = np.asarray(inp['fc1_b'], np.float64) + np.asarray(inp['fc1_w'], np.float64) @ n2b
    f2w = G2[:, None] * np.asarray(inp['fc2_w'], np.float64)
    f2b = G2 * np.asarray(inp['fc2_b'], np.float64)

    Wv = Wq[2 * DIM:]
    bv = bq[2 * DIM:]
    # V'' pair p cols [66p..66p+66): [32 a-dims][one_a][32 b-dims][one_b]
    Wv_aug = np.zeros((DIM, 264))
    vb_aug = np.zeros(264)
    for p in range(4):
        ha, hb = 2 * p, 2 * p + 1
        base = 66 * p
        Wv_aug[:, base:base + 32] = Wv[32 * ha:32 * ha + 32].T
        vb_aug[base:base + 32] = bv[32 * ha:32 * ha + 32]
        vb_aug[base + 32] = 1.0
        Wv_aug[:, base + 33:base + 65] = Wv[32 * hb:32 * hb + 32].T
        vb_aug[base + 33:base + 65] = bv[32 * hb:32 * hb + 32]
        vb_aug[base + 65] = 1.0

    def kblocked(wT, kb):
        n = wT.shape[1]
        return np.ascontiguousarray(wT.reshape(kb, 128, n).transpose(1, 0, 2))

    # smallw bf16 [1, 776]: vb(264) | pb(256) | f2b(256)
    smallw = np.zeros((1, 776))
    smallw[0, 0:264] = vb_aug
    smallw[0, 264:520] = pb
    smallw[0, 520:776] = f2b

    # psel f32 [1, 194]: rows-0..63 selector | rows-64..96 selector
    psel = np.zeros((1, 194))
    psel[0, 0:64] = 1.0
    psel[0, 97 + 64:97 + 97] = 1.0

    # cbias f32 [128, 12]: qkb (q0,q1,k0,k1) | f1b (8 cols)
    cbias = np.zeros((128, 12))
    for mt in range(4):
        cbias[:, mt] = bq[128 * mt:128 * mt + 128]
    for mh in range(8):
        cbias[:, 4 + mh] = f1b[128 * mh:128 * mh + 128]

    # proj lhsT from attnT pair tiles [97, 128]: rows 0-31 head 2p (den row
    # 32 zero-weighted), rows 64-95 head 2p+1 (den row 96)
    pwT_aug = np.zeros((4, 97, DIM))
    for p in range(4):
        ha, hb = 2 * p, 2 * p + 1
        pwT_aug[p, 0:32] = pw[:, 32 * ha:32 * ha + 32].T
        pwT_aug[p, 64:96] = pw[:, 32 * hb:32 * hb + 32].T

    return dict(
        wqk=_bf(kblocked(Wq[:512].T, 2)),       # [128, 2, 512]
        wv=_bf(kblocked(Wv_aug, 2)),            # [128, 2, 264]
        f1w=_bf(kblocked(f1w.T, 2)),            # [128, 2, 1024]
        f2w=_bf(kblocked(f2w.T, 8)),            # [128, 8, 256]
        pwT=_bf(np.ascontiguousarray(pwT_aug.transpose(1, 0, 2))),  # [97, 4, 256]
        smallw=_bf(smallw),                     # [1, 776]
        psel=_f32(psel),                        # [1, 194]
        cbias=_f32(cbias),                      # [128, 12]
        rpb=np.asarray(inp['rpb'], np.float64),
    )


def _build_E(rpb, flip):
    """Tile-major E: [128 keys, (quad, t, slot, qspan)] -> [128, 11264] f32."""
    def img_row(r):
        return (31 - r) if flip else r
    blocks = []
    for quad in range(2):
        for t in range(5):
            qbase, qspan = TILE_Q[t]
            kk = np.arange(128)[:, None, None]
            ss = np.arange(4)[None, :, None]
            qq = qbase + np.arange(qspan)[None, None, :]
            tk = 128 * t + kk
            ki_loc, kj = tk // 32, tk % 32
            qi_loc, qj = qq // 32, qq % 32
            qi = img_row(qi_loc)
            ki = img_row(np.minimum(ki_loc, 19))
            sh = np.clip(qi - 3, 0, H - KS)
            sw = np.clip(qj - 3, 0, W - KS)
            valid = (tk < NT) & (ki >= sh) & (ki < sh + KS) \
                & (kj >= sw) & (kj < sw + KS)
            bh = np.clip(ki - qi + KS - 1, 0, 2 * KS - 2)
            bw = np.clip(kj - qj + KS - 1, 0, 2 * KS - 2)
            h = 4 * quad + ss
            bias = rpb[h, bh, bw]
            E = np.where(valid, np.exp(bias), 0.0)
            blocks.append(np.ascontiguousarray(E.reshape(128, 4 * qspan)))
    return np.concatenate(blocks, axis=1)


def _prepare_inputs(inp):
    F = _fold_weights(inp)
    E_top = _bf(_build_E(F['rpb'], flip=False))
    E_bot = _bf(_build_E(F['rpb'], flip=True))
    x = np.asarray(inp['x'], np.float32)
    Bsz = x.shape[0]
    shared = {k: v for k, v in F.items() if k != 'rpb'}
    in_maps = []
    for b in range(Bsz):
        for half in range(2):
            if half == 0:
                x_loc = x[b, 0:19].reshape(NT, DIM)
                E = E_top
            else:
                x_loc = x[b, 31:12:-1].reshape(NT, DIM)
                E = E_bot
            x_pad = np.zeros((NTP, DIM), np.float32)
            x_pad[:NT] = x_loc
            m = dict(shared)
            m['x'] = x_pad
            m['Eall'] = E
            in_maps.append(m)
    return in_maps


# --------------------------------------------------------------------------
# Bass kernel graph
# --------------------------------------------------------------------------

def build_graph():
    import concourse.bass as bass
    import concourse.tile as tile
    import concourse.mybir as mybir
    from concourse import bacc
    from concourse.masks import make_identity

    dt = mybir.dt
    Alu = mybir.AluOpType
    Act = mybir.ActivationFunctionType

    nc = bacc.Bacc()

    def param(name, shape, dtype, out=False):
        return nc.declare_dram_parameter(name, list(shape), dtype, isOutput=out)

    x_d = param("x", (NTP, DIM), dt.float32)
    wqk_d = param("wqk", (128, 2, 512), dt.bfloat16)
    wv_d = param("wv", (128, 2, 264), dt.bfloat16)
    f1w_d = param("f1w", (128, 2, HID), dt.bfloat16)
    f2w_d = param("f2w", (128, 8, DIM), dt.bfloat16)
    pwT_d = param("pwT", (97, 4, DIM), dt.bfloat16)
    smallw_d = param("smallw", (1, 776), dt.bfloat16)
    psel_d = param("psel", (1, 194), dt.float32)
    cbias_d = param("cbias", (128, 12), dt.float32)
    Eall_d = param("Eall", (128, E_NCOL), dt.bfloat16)
    out_d = param("out", (NQ, DIM), dt.float32, out=True)

    def bcast_ap(src, nparts):
        """Partition-broadcast AP: repeat a [1, n] AP across nparts."""
        return bass.AP(tensor=src.tensor, offset=src.offset,
                       ap=[[0, nparts]] + list(src.ap[1:]))

    with tile.TileContext(nc) as tc:
        with (
            tc.tile_pool(name="consts", bufs=1) as consts,
            tc.tile_pool(name="persist", bufs=1) as persist,
            tc.tile_pool(name="work", bufs=3) as work,
            tc.tile_pool(name="aqp", bufs=2) as aqp,
            tc.tile_pool(name="psS", bufs=2, space="PSUM") as psS,
            tc.tile_pool(name="psav", bufs=2, space="PSUM") as psav,
        ):
            # ---------------- input DMAs ---------------------------------
            cbias_sb = consts.tile([128, 12], dt.float32, tag="cbias")
            nc.sync.dma_start(out=cbias_sb, in_=cbias_d[:])
            xs = persist.tile([128, 5, DIM], dt.float32, tag="xs")
            nc.sync.dma_start(
                out=xs[:, 0:2, :],
                in_=x_d[0:256, :].rearrange("(t p) c -> p t c", p=128))
            nc.sync.dma_start(
                out=xs[:, 2:5, :],
                in_=x_d[256:640, :].rearrange("(t p) c -> p t c", p=128))
            wqk_sb = consts.tile([128, 2, 512], dt.bfloat16, tag="wqk")
            nc.sync.dma_start(out=wqk_sb, in_=wqk_d[:])
            Eall_sb = consts.tile([128, E_NCOL], dt.bfloat16, tag="Eall")
            nc.sync.dma_start(out=Eall_sb[:, :E_NCOL // 2],
                              in_=Eall_d[:, :E_NCOL // 2])
            nc.sync.dma_start(out=Eall_sb[:, E_NCOL // 2:],
                              in_=Eall_d[:, E_NCOL // 2:])

            smallw_sb = consts.tile([1, 776], dt.bfloat16, tag="smallw")
            nc.sync.dma_start(out=smallw_sb, in_=smallw_d[:])
            psel_sb = consts.tile([1, 194], dt.float32, tag="psel")
            nc.sync.dma_start(out=psel_sb, in_=psel_d[:])
            wv_sb = consts.tile([128, 2, 264], dt.bfloat16, tag="wv")
            nc.sync.dma_start(out=wv_sb, in_=wv_d[:])
            pwT_sb = consts.tile([97, 4, DIM], dt.bfloat16, tag="pwT")
            nc.sync.dma_start(out=pwT_sb, in_=pwT_d[:])

            f1w_sb = consts.tile([128, 2, HID], dt.bfloat16, tag="f1w")
            nc.gpsimd.dma_start(out=f1w_sb, in_=f1w_d[:])
            f2w_sb = consts.tile([128, 8, DIM], dt.bfloat16, tag="f2w")
            nc.gpsimd.dma_start(out=f2w_sb, in_=f2w_d[:])

            vb_sb = smallw_sb[0:1, 0:264]
            pb_sb = smallw_sb[0:1, 264:520]
            f2b_sb = smallw_sb[0:1, 520:776]
            qkb_sb = cbias_sb[:, 0:4]
            f1b_sb = cbias_sb[:, 4:12]

            # E block views [128, 4, qspan]
            E_view = {}
            off = 0
            for quad in range(2):
                for t in range(5):
                    qspan = TILE_Q[t][1]
                    E_view[(quad, t)] = Eall_sb[:, off:off + 4 * qspan] \
                        .rearrange("p (s q) -> p s q", s=4)
                    off += 4 * qspan

            ident = consts.tile([128, 128], dt.bfloat16, tag="ident")
            make_identity(nc, ident)
            # per-slot zero-masked K tiles: full-128-K S matmuls avoid
            # concurrent row-group writes to one PSUM bank (HW collision)
            kTm = [[persist.tile([128, NTP], dt.bfloat16, tag=f"kTm{q}_{s}",
                                 name=f"kTm{q}_{s}") for s in range(4)]
                   for q in range(2)]
            for q in range(2):
                for s in range(4):
                    nc.gpsimd.memset(kTm[q][s], 0.0)
            ones_sb = consts.tile([1, 512], dt.bfloat16, tag="ones")
            nc.vector.memset(ones_sb, 1.0)
            # ACT table preload (Exp) while DMAs land
            idummy = work.tile([1, 2], dt.float32, tag="idummy")
            nc.scalar.activation(out=idummy, in_=ones_sb[0:1, 0:2], func=Act.Exp)

            # ---------------- PE keepalive (HAM warm-up) ------------------
            def keepalive(n, nk=128):
                ka = psS.tile([128, 4, 384], dt.float32, tag="psS", name="ka")
                kaf = ka.rearrange("p a b -> p (a b)")
                for _ in range(n):
                    nc.tensor.matmul(kaf[:, :nk], ident, ident[:, :nk],
                                     start=True, stop=True)

            keepalive(24)

            # ---------------- helpers ----------------
            def dve_rsqrt(dst, src, n):
                ve = work.tile([128, n], dt.float32, tag="rsq_ve", bufs=2)
                nc.vector.tensor_scalar(out=ve, in0=src, scalar1=float(EPS),
                                        scalar2=None, op0=Alu.add)
                yi = work.tile([128, n], dt.int32, tag="rsq_yi", bufs=2)
                nc.vector.tensor_scalar(out=yi, in0=ve[:].bitcast(dt.int32),
                                        scalar1=1, scalar2=None,
                                        op0=Alu.logical_shift_right)
                nc.vector.tensor_scalar(out=yi, in0=yi, scalar1=-1,
                                        scalar2=MAGIC, op0=Alu.mult, op1=Alu.add)
                y = yi[:].bitcast(dt.float32)
                t = work.tile([128, n], dt.float32, tag="rsq_t", bufs=2)
                for _ in range(2):
                    nc.vector.tensor_tensor(out=t, in0=y, in1=y, op=Alu.mult)
                    nc.vector.tensor_tensor(out=t, in0=t, in1=ve, op=Alu.mult)
                    nc.vector.tensor_scalar(out=t, in0=t, scalar1=-0.5,
                                            scalar2=1.5, op0=Alu.mult, op1=Alu.add)
                    nc.vector.tensor_tensor(out=y, in0=y, in1=t, op=Alu.mult)
                nc.vector.tensor_copy(out=dst, in_=y)

            def ln_stats(x_list, tagp):
                """Returns (rstd, negmr) [128, ntile] f32 tiles."""
                ntile = len(x_list)
                mv = work.tile([128, 2 * ntile], dt.float32, tag=tagp + "_mv", bufs=2)
                for t, xt in enumerate(x_list):
                    stats = work.tile([128, 6], dt.float32, tag=tagp + "_st", bufs=2)
                    nc.vector.bn_stats(out=stats, in_=xt)
                    nc.vector.bn_aggr(out=mv[:, 2 * t:2 * t + 2], in_=stats)
                rstd = work.tile([128, ntile], dt.float32, tag=tagp + "_rs", bufs=2)
                dve_rsqrt(rstd, mv[:, 1::2], ntile)
                negmr = work.tile([128, ntile], dt.float32, tag=tagp + "_nm", bufs=2)
                nc.vector.scalar_tensor_tensor(out=negmr, in0=mv[:, 0::2],
                                               scalar=-1.0, in1=rstd,
                                               op0=Alu.mult, op1=Alu.mult)
                return rstd, negmr

            def transpose_tile(dst_cblocks, xh, col, scalar_evac=False):
                """PE-transpose xh [128, 256] bf16 into dst cblock tiles at
                column offset col (128 wide)."""
                for cb in range(2):
                    tp = psav.tile([128, 512], dt.float32, tag="pav", name=f"tp{col}_{cb}")
                    ptb = tp[:, 0:64].bitcast(dt.bfloat16)
                    nc.tensor.transpose(ptb, xh[:, 128 * cb:128 * (cb + 1)], ident)
                    if scalar_evac:
                        nc.scalar.activation(out=dst_cblocks[cb][:, col:col + 128],
                                             in_=ptb, func=Act.Copy)
                    else:
                        nc.vector.tensor_copy(
                            out=dst_cblocks[cb][:, col:col + 128], in_=ptb)

            # ---------------- LN1 + transpose + QKV + V ------------------
            xhatT = [persist.tile([128, NTP], dt.bfloat16, tag=f"xhatT{cb}",
                                  name=f"xhatT{cb}") for cb in range(2)]

            x_t = [xs[:, t, :] for t in range(5)]
            rs1a, nm1a = ln_stats(x_t[:2], "ln1a")
            xh_t = []
            for t in range(2):
                xh = work.tile([128, DIM], dt.bfloat16, tag=f"xh{t}", bufs=1)
                nc.scalar.activation(out=xh, in_=x_t[t], func=Act.Identity,
                                     scale=rs1a[:, t:t + 1], bias=nm1a[:, t:t + 1])
                xh_t.append(xh)
                transpose_tile(xhatT, xh, 128 * t)

            rs1b, nm1b = ln_stats(x_t[2:], "ln1b")
            for t in range(2, 5):
                xh = work.tile([128, DIM], dt.bfloat16, tag=f"xh{t}", bufs=1)
                nc.scalar.activation(out=xh, in_=x_t[t], func=Act.Identity,
                                     scale=rs1b[:, t - 2:t - 1], bias=nm1b[:, t - 2:t - 1])
                xh_t.append(xh)
                transpose_tile(xhatT, xh, 128 * t)

            # QKV: mt0 -> qT[0], mt1 -> qT[1], mt2 -> kT[0], mt3 -> kT[1]
            qT = [persist.tile([128, NQ], dt.bfloat16, tag=f"qT{i}", name=f"qT{i}")
                  for i in range(2)]
            kT = [persist.tile([128, NTP], dt.bfloat16, tag=f"kT{i}", name=f"kT{i}")
                  for i in range(2)]

            qkps = [psS.tile([128, 4, 384], dt.float32, tag="psS", name=f"qkps{i}")
                    .rearrange("p a b -> p (a b)") for i in range(2)]

            def qkv_chunk(mt, n0, nn):
                # qkps[0] holds mt0 (flat 0:512) + mt2 (512:1152);
                # qkps[1] holds mt1 + mt3. All chunk regions stay in-bank.
                dst = (qT if mt < 2 else kT)[mt % 2]
                base = 0 if mt < 2 else 512
                pt = qkps[mt % 2][:, base + n0:base + n0 + nn]
                for kb in range(2):
                    nc.tensor.matmul(pt, wqk_sb[:, kb, 128 * mt:128 * (mt + 1)],
                                     xhatT[kb][:, n0:n0 + nn],
                                     start=(kb == 0), stop=(kb == 1))
                nc.scalar.activation(out=dst[:, n0:n0 + nn], in_=pt,
                                     func=Act.Identity, bias=qkb_sb[:, mt:mt + 1])

            # early chunks for quad0 (needs T1 tiles 0,1 only)
            def kmask(quad, n0, nn):
                for s in range(4):
                    nc.vector.tensor_copy(
                        out=kTm[quad][s][32 * s:32 * s + 32, n0:n0 + nn],
                        in_=kT[quad][32 * s:32 * s + 32, n0:n0 + nn])

            qkv_chunk(0, 0, 256)
            qkv_chunk(2, 0, 256)
            kmask(0, 0, 256)
            # rest
            qkv_chunk(0, 256, 256)
            qkv_chunk(2, 256, 256)
            kmask(0, 256, 256)
            qkv_chunk(2, 512, 128)
            kmask(0, 512, 128)
            qkv_chunk(1, 0, 512)
            qkv_chunk(3, 0, 512)
            kmask(1, 0, 512)
            qkv_chunk(3, 512, 128)
            kmask(1, 512, 128)

            # V per token tile: [128 tok, 264]
            vt = []
            for t in range(5):
                pv = psav.tile([128, 512], dt.float32, tag="pav", name=f"pv{t}")
                pvs = pv[:, 0:264]
                for kb in range(2):
                    nc.tensor.matmul(pvs, xhatT[kb][:, 128 * t:128 * (t + 1)],
                                     wv_sb[:, kb, :], start=(kb == 0), stop=False)
                nc.tensor.matmul(pvs, ones_sb[:, :128], vb_sb,
                                 start=False, stop=True)
                v = persist.tile([128, 264], dt.bfloat16, tag=f"vt{t}", name=f"vt{t}")
                nc.vector.tensor_copy(out=v, in_=pvs)
                vt.append(v)

            # ---------------- attention ----------------
            attnP2 = [persist.tile([97, 2, NQ], dt.bfloat16, tag=f"attnP{q}",
                                   name=f"attnP{q}") for q in range(2)]

            def split_banks(start, n):
                """Split f32 col range [start, start+n) at 512-boundaries."""
                pieces = []
                cur = start
                end = start + n
                while cur < end:
                    nxt = min(end, (cur // 512 + 1) * 512)
                    pieces.append((cur - start, nxt - cur))
                    cur = nxt
                return pieces

            def unit_av(quad, g, aq_tiles):
                pav = psav.tile([128, 512], dt.float32, tag="pav",
                                name=f"pav{quad}_{g}")
                tiles = G_TILES[g]
                for pl in range(2):
                    p = 2 * quad + pl
                    for sub in range(2):   # 0 = a-heads, 1 = b-heads
                        vlo = 66 * p + 33 * sub
                        po = 64 * sub
                        for i, t in enumerate(tiles):
                            qbase, qspan = TILE_Q[t]
                            qoff = 128 * g - qbase
                            nc.tensor.matmul(
                                pav[po:po + 33, 128 * pl:128 * (pl + 1)],
                                vt[t][:, vlo:vlo + 33],
                                aq_tiles[t][:, 2 * pl + sub, qoff:qoff + 128],
                                start=(i == 0), stop=(i == len(tiles) - 1))
                # denominators at rows 32 (a) / 96 (b), cols 0:256
                den = work.tile([1, 512], dt.float32, tag="den", bufs=3)
                if quad == 0:
                    nc.vector.tensor_copy(out=den[:, 0:256], in_=pav[32:33, 0:256])
                    nc.vector.tensor_copy(out=den[:, 256:512], in_=pav[96:97, 0:256])
                else:
                    nc.scalar.activation(out=den[:, 0:256], in_=pav[32:33, 0:256],
                                         func=Act.Copy)
                    nc.scalar.activation(out=den[:, 256:512], in_=pav[96:97, 0:256],
                                         func=Act.Copy)
                rcd = work.tile([1, 512], dt.float32, tag="rcd", bufs=3)
                nc.vector.reciprocal_approx_fast(out=rcd, in_=den)
                # PE broadcast of reciprocals across partitions (f32 matmuls)
                for pl in range(2):
                    pB = pav[:97, 256 + 128 * pl:384 + 128 * pl]
                    nc.tensor.matmul(pB, psel_sb[0:1, 0:97],
                                     rcd[:, 128 * pl:128 * (pl + 1)],
                                     start=True, stop=False)
                    nc.tensor.matmul(pB, psel_sb[0:1, 97:194],
                                     rcd[:, 256 + 128 * pl:384 + 128 * pl],
                                     start=False, stop=True)
                rcb = work.tile([97, 256], dt.bfloat16, tag="rcb", bufs=3)
                rcbeng = nc.vector if quad == 0 else nc.scalar
                if quad == 0:
                    nc.vector.tensor_copy(out=rcb, in_=pav[:97, 256:512])
                else:
                    nc.scalar.activation(out=rcb, in_=pav[:97, 256:512],
                                         func=Act.Copy)
                nc.vector.tensor_tensor(
                    out=attnP2[quad][:, :, 128 * g:128 * (g + 1)],
                    in0=pav[:97, 0:256].rearrange("p (two q) -> p two q", two=2),
                    in1=rcb.rearrange("p (two q) -> p two q", two=2),
                    op=Alu.mult)

            def proj_tail(mt):
                """proj + residual for query tile mt (= group mt)."""
                pt = psav.tile([128, 512], dt.float32, tag="pav", name=f"pproj{mt}")
                pp = pt[:, :DIM]
                for p in range(4):
                    nc.tensor.matmul(pp,
                                     attnP2[p // 2][:, p % 2, 128 * mt:128 * (mt + 1)],
                                     pwT_sb[:, p, :], start=(p == 0), stop=False)
                nc.tensor.matmul(pp, ones_sb[:, :128], pb_sb, start=False, stop=True)
                yt = persist.tile([128, DIM], dt.float32, tag=f"y{mt}")
                nc.vector.tensor_tensor(out=yt, in0=pp, in1=x_t[mt], op=Alu.add)
                return yt

            y_tiles = [None] * 4
            aq_cur = {}
            for quad in range(2):
                for t in range(5):
                    qbase, qspan = TILE_Q[t]
                    pS = psS.tile([128, 4, 384], dt.float32, tag="psS",
                                  name=f"pS{quad}_{t}")
                    pSf = pS.rearrange("p a b -> p (a b)")
                    for s in range(4):
                        for (off, n) in split_banks(384 * s, qspan):
                            nc.tensor.matmul(
                                pSf[:, 384 * s + off:384 * s + off + n],
                                kTm[quad][s][:, 128 * t:128 * (t + 1)],
                                qT[quad][:, qbase + off:qbase + off + n],
                                start=True, stop=True)
                    aq = aqp.tile([128, 4, qspan], dt.bfloat16, tag=f"aq{t}",
                                  name=f"aq{quad}_{t}")
                    nc.scalar.activation(out=aq, in_=pS[:, :, :qspan], func=Act.Exp)
                    emeng = nc.gpsimd if t in (0, 4) else nc.vector
                    emeng.tensor_tensor(
                        out=aq.rearrange("p s q -> p (s q)"),
                        in0=aq.rearrange("p s q -> p (s q)"),
                        in1=E_view[(quad, t)].rearrange("p s q -> p (s q)"),
                        op=Alu.mult)
                    aq_cur[t] = aq
                    for g in LAST_UNITS.get(t, []):
                        unit_av(quad, g, aq_cur)
                        if quad == 1:
                            y_tiles[g] = proj_tail(g)

            # gelu table preload right after last exp
            gdummy = work.tile([1, 2], dt.float32, tag="gdummy")
            nc.scalar.activation(out=gdummy, in_=ones_sb[0:1, 0:2], func=Act.Gelu)

            # ---------------- LN2 + MLP ----------------
            x2T = [persist.tile([128, NQ], dt.bfloat16, tag=f"x2T{cb}",
                                name=f"x2T{cb}") for cb in range(2)]
            for half in range(2):
                ys = y_tiles[2 * half:2 * half + 2]
                rs2, nm2 = ln_stats(ys, f"ln2{half}")
                for i, yt in enumerate(ys):
                    xh2 = work.tile([128, DIM], dt.bfloat16,
                                    tag=f"xh2_{half}{i}", bufs=1)
                    nc.vector.tensor_scalar(out=xh2, in0=yt,
                                            scalar1=rs2[:, i:i + 1],
                                            scalar2=nm2[:, i:i + 1],
                                            op0=Alu.mult, op1=Alu.add)
                    transpose_tile(x2T, xh2, 128 * (2 * half + i),
                                   scalar_evac=True)

            m1 = []
            for mh in range(8):
                if mh % 3 == 0:
                    f1ps = psS.tile([128, 4, 384], dt.float32, tag="psS",
                                    name=f"f1ps{mh}").rearrange("p a b -> p (a b)")
                pt = f1ps[:, 512 * (mh % 3):512 * (mh % 3) + 512]
                for kb in range(2):
                    nc.tensor.matmul(pt, f1w_sb[:, kb, 128 * mh:128 * (mh + 1)],
                                     x2T[kb], start=(kb == 0), stop=(kb == 1))
                mg = persist.tile([128, NQ], dt.bfloat16, tag=f"m1_{mh}")
                nc.scalar.activation(out=mg, in_=pt, func=Act.Gelu,
                                     bias=f1b_sb[:, mh:mh + 1], scale=1.0)
                m1.append(mg)

            for mt in range(4):
                pt = psav.tile([128, 512], dt.float32, tag="pav", name=f"pfc2{mt}")
                pp = pt[:, :DIM]
                for kb in range(8):
                    nc.tensor.matmul(pp, m1[kb][:, 128 * mt:128 * (mt + 1)],
                                     f2w_sb[:, kb, :], start=(kb == 0), stop=False)
                nc.tensor.matmul(pp, ones_sb[:, :128], f2b_sb, start=False, stop=True)
                ot = work.tile([128, DIM], dt.float32, tag="outt", bufs=2)
                nc.vector.tensor_tensor(out=ot, in0=pp, in1=y_tiles[mt][:],
                                        op=Alu.add)
                eng = nc.sync if mt % 2 == 0 else nc.gpsimd
                eng.dma_start(out=out_d[128 * mt:128 * (mt + 1), :], in_=ot)

    nc.finalize()
    return nc


# --------------------------------------------------------------------------
# Entry point
# --------------------------------------------------------------------------

def kernel(**inputs):
    from concourse.bass_utils import run_bass_kernel_spmd

    if 'nc' not in _CACHE:
        _CACHE['nc'] = build_graph()
    nc = _CACHE['nc']

    in_maps = _prepare_inputs(inputs)
    res = run_bass_kernel_spmd(nc, in_maps, core_ids=list(range(8)))
    x = np.asarray(inputs['x'])
    Bsz, Hh, Ww, C = x.shape
    out = np.zeros((Bsz, Hh, Ww, C), np.float32)
    for i in range(2 * Bsz):
        b, half = divmod(i, 2)
        o = np.asarray(res.results[i]['out']).reshape(16, Ww, C)
        if half == 0:
            out[b, 0:16] = o
        else:
            out[b, 16:32] = o[::-1]
    return out.astype(x.dtype)


# revision 15
# speedup vs baseline: 1.1819x; 1.0554x over previous
"""Trainium2 Bass kernel for AdaNSABlock (7x7 neighborhood attention block).

Sharding: 8 cores = batch(4) x row-halves(2). Each core computes 16 image
rows (512 tokens) of one sample, reading 19 rows (3-row halo) of input
padded to 640 tokens (5 clean 128-token tiles). Bottom halves are
row-flipped on host so all cores run one SPMD graph.

v2 rewrite vs v1:
  - tokens padded to 640: all tiles 128 wide, key chunks = absolute
    128-aligned token tiles (no dedup, V computed once per tile)
  - tile-major S: per (quad, key-tile) one wide-N matmul per head slot
    (split only at PSUM bank boundaries) -> ~52 S matmuls vs 88, far
    fewer LDWEIGHTS
  - one merged exp ACTIVATE per (quad, tile) over [128, 4*qspan]
  - one E-multiply per (quad, tile); small tiles on GpSimd, rest Vector
  - AV per unit as 4 sequential accumulation chains (M=33 a/b col-tiles)
  - softmax denominators: reciprocal_approx_fast straight off PSUM, then
    DMA partition-broadcast (stride-0 src) instead of PE broadcast matmuls
  - input DMAs split across sync/scalar/gpsimd queues by need-time
"""

import numpy as np
import ml_dtypes

KS = 7
HEADS = 8
DIM = 256
HID = 1024
HD = 32
H = 32
W = 32
NT = 19 * 32          # valid local tokens incl halo
NTP = 640             # padded tokens (5 tiles of 128)
NQ = 512              # query tokens per core
EPS = 1e-5
BF16 = ml_dtypes.bfloat16

# tile-major schedule: per key-tile t -> (qbase, qspan)
TILE_Q = [(0, 256), (0, 384), (128, 384), (256, 256), (384, 128)]
# per query group g -> key tiles used
G_TILES = [[0, 1], [0, 1, 2], [1, 2, 3], [2, 3, 4]]
# units finishing at tile t
LAST_UNITS = {1: [0], 2: [1], 3: [2], 4: [3]}
E_NCOL = sum(4 * qs for (_, qs) in TILE_Q) * 2   # 11264

MAGIC = 0x5F3759DF

_CACHE = {}


def _bf(x):
    return np.ascontiguousarray(np.asarray(x, np.float32).astype(BF16))


def _f32(x):
    return np.ascontiguousarray(np.asarray(x, np.float32))


# --------------------------------------------------------------------------
# Host-side folding + E construction
# --------------------------------------------------------------------------

def _fold_weights(inp):
    quality = inp['quality']
    s = int(quality) - 1
    l = float(quality % 1)
    g1 = np.abs(np.asarray(inp['gamma_1'], np.float64))
    g2 = np.abs(np.asarray(inp['gamma_2'], np.float64))
    if s == g1.shape[0] - 1:
        G1, G2 = g1[s], g2[s]
    else:
        G1 = g1[s] ** (1 - l) * g1[s + 1] ** l
        G2 = g2[s] ** (1 - l) * g2[s + 1] ** l

    qkv_w = np.asarray(inp['qkv_w'], np.float64)
    qkv_b = np.asarray(inp['qkv_b'], np.float64)
    n1w = np.asarray(inp['norm1_w'], np.float64)
    n1b = np.asarray(inp['norm1_b'], np.float64)
    Wq = qkv_w * n1w[None, :]
    bq = qkv_b + qkv_w @ n1b
    sc = HD ** -0.5
    Wq[:DIM] *= sc
    bq[:DIM] *= sc

    pw = G1[:, None] * np.asarray(inp['proj_w'], np.float64)
    pb = G1 * np.asarray(inp['proj_b'], np.float64)

    n2w = np.asarray(inp['norm2_w'], np.float64)
    n2b = np.asarray(inp['norm2_b'], np.float64)
    f1w = np.asarray(inp['fc1_w'], np.float64) * n2w[None, :]
    f1b = np.asarray(inp['fc1_b'], np.float64) + np.asarray(inp['fc1_w'], np.float64) @ n2b
    f2w = G2[:, None] * np.asarray(inp['fc2_w'], np.float64)
    f2b = G2 * np.asarray(inp['fc2_b'], np.float64)

    Wv = Wq[2 * DIM:]
    bv = bq[2 * DIM:]
    # V'' pair p cols [66p..66p+66): [32 a-dims][one_a][32 b-dims][one_b]
    Wv_aug = np.zeros((DIM, 264))
    vb_aug = np.zeros(264)
    for p in range(4):
        ha, hb = 2 * p, 2 * p + 1
        base = 66 * p
        Wv_aug[:, base:base + 32] = Wv[32 * ha:32 * ha + 32].T
        vb_aug[base:base + 32] = bv[32 * ha:32 * ha + 32]
        vb_aug[base + 32] = 1.0
        Wv_aug[:, base + 33:base + 65] = Wv[32 * hb:32 * hb + 32].T
        vb_aug[base + 33:base + 65] = bv[32 * hb:32 * hb + 32]
        vb_aug[base + 65] = 1.0

    def kblocked(wT, kb):
        n = wT.shape[1]
        return np.ascontiguousarray(wT.reshape(kb, 128, n).transpose(1, 0, 2))

    # smallw bf16 [1, 776]: vb(264) | pb(256) | f2b(256)
    smallw = np.zeros((1, 776))
    smallw[0, 0:264] = vb_aug
    smallw[0, 264:520] = pb
    smallw[0, 520:776] = f2b

    # psel f32 [1, 194]: rows-0..63 selector | rows-64..96 selector
    psel = np.zeros((1, 194))
    psel[0, 0:64] = 1.0
    psel[0, 97 + 64:97 + 97] = 1.0

    # cbias f32 [128, 12]: qkb (q0,q1,k0,k1) | f1b (8 cols)
    cbias = np.zeros((128, 12))
    for mt in range(4):
        cbias[:, mt] = bq[128 * mt:128 * mt + 128]
    for mh in range(8):
        cbias[:, 4 + mh] = f1b[128 * mh:128 * mh + 128]

    # proj lhsT from attnT pair tiles [97, 128]: rows 0-31 head 2p (den row
    # 32 zero-weighted), rows 64-95 head 2p+1 (den row 96)
    pwT_aug = np.zeros((4, 97, DIM))
    for p in range(4):
        ha, hb = 2 * p, 2 * p + 1
        pwT_aug[p, 0:32] = pw[:, 32 * ha:32 * ha + 32].T
        pwT_aug[p, 64:96] = pw[:, 32 * hb:32 * hb + 32].T

    return dict(
        wqk=_bf(kblocked(Wq[:512].T, 2)),       # [128, 2, 512]
        wv=_bf(kblocked(Wv_aug, 2)),            # [128, 2, 264]
        f1w=_bf(kblocked(f1w.T, 2)),            # [128, 2, 1024]
        f2w=_bf(kblocked(f2w.T, 8)),            # [128, 8, 256]
        pwT=_bf(np.ascontiguousarray(pwT_aug.transpose(1, 0, 2))),  # [97, 4, 256]
        smallw=_bf(smallw),                     # [1, 776]
        psel=_f32(psel),                        # [1, 194]
        cbias=_f32(cbias),                      # [128, 12]
        rpb=np.asarray(inp['rpb'], np.float64),
    )


def _build_E(rpb, flip):
    """Tile-major E: [128 keys, (quad, t, slot, qspan)] -> [128, 11264] f32."""
    def img_row(r):
        return (31 - r) if flip else r
    blocks = []
    for quad in range(2):
        for t in range(5):
            qbase, qspan = TILE_Q[t]
            kk = np.arange(128)[:, None, None]
            ss = np.arange(4)[None, :, None]
            qq = qbase + np.arange(qspan)[None, None, :]
            tk = 128 * t + kk
            ki_loc, kj = tk // 32, tk % 32
            qi_loc, qj = qq // 32, qq % 32
            qi = img_row(qi_loc)
            ki = img_row(np.minimum(ki_loc, 19))
            sh = np.clip(qi - 3, 0, H - KS)
            sw = np.clip(qj - 3, 0, W - KS)
            valid = (tk < NT) & (ki >= sh) & (ki < sh + KS) \
                & (kj >= sw) & (kj < sw + KS)
            bh = np.clip(ki - qi + KS - 1, 0, 2 * KS - 2)
            bw = np.clip(kj - qj + KS - 1, 0, 2 * KS - 2)
            h = 4 * quad + ss
            bias = rpb[h, bh, bw]
            E = np.where(valid, np.exp(bias), 0.0)
            blocks.append(np.ascontiguousarray(E.reshape(128, 4 * qspan)))
    return np.concatenate(blocks, axis=1)


def _prepare_inputs(inp):
    F = _fold_weights(inp)
    E_top = _bf(_build_E(F['rpb'], flip=False))
    E_bot = _bf(_build_E(F['rpb'], flip=True))
    x = np.asarray(inp['x'], np.float32)
    Bsz = x.shape[0]
    shared = {k: v for k, v in F.items() if k != 'rpb'}
    in_maps = []
    for b in range(Bsz):
        for half in range(2):
            if half == 0:
                x_loc = x[b, 0:19].reshape(NT, DIM)
                E = E_top
            else:
                x_loc = x[b, 31:12:-1].reshape(NT, DIM)
                E = E_bot
            x_pad = np.zeros((NTP, DIM), np.float32)
            x_pad[:NT] = x_loc
            m = dict(shared)
            m['x'] = x_pad
            m['Eall'] = E
            in_maps.append(m)
    return in_maps


# --------------------------------------------------------------------------
# Bass kernel graph
# --------------------------------------------------------------------------

def build_graph():
    import concourse.bass as bass
    import concourse.tile as tile
    import concourse.mybir as mybir
    from concourse import bacc
    from concourse.masks import make_identity

    dt = mybir.dt
    Alu = mybir.AluOpType
    Act = mybir.ActivationFunctionType

    nc = bacc.Bacc()

    def param(name, shape, dtype, out=False):
        return nc.declare_dram_parameter(name, list(shape), dtype, isOutput=out)

    x_d = param("x", (NTP, DIM), dt.float32)
    wqk_d = param("wqk", (128, 2, 512), dt.bfloat16)
    wv_d = param("wv", (128, 2, 264), dt.bfloat16)
    f1w_d = param("f1w", (128, 2, HID), dt.bfloat16)
    f2w_d = param("f2w", (128, 8, DIM), dt.bfloat16)
    pwT_d = param("pwT", (97, 4, DIM), dt.bfloat16)
    smallw_d = param("smallw", (1, 776), dt.bfloat16)
    psel_d = param("psel", (1, 194), dt.float32)
    cbias_d = param("cbias", (128, 12), dt.float32)
    Eall_d = param("Eall", (128, E_NCOL), dt.bfloat16)
    out_d = param("out", (NQ, DIM), dt.float32, out=True)

    def bcast_ap(src, nparts):
        """Partition-broadcast AP: repeat a [1, n] AP across nparts."""
        return bass.AP(tensor=src.tensor, offset=src.offset,
                       ap=[[0, nparts]] + list(src.ap[1:]))

    with tile.TileContext(nc) as tc:
        with (
            tc.tile_pool(name="consts", bufs=1) as consts,
            tc.tile_pool(name="persist", bufs=1) as persist,
            tc.tile_pool(name="work", bufs=3) as work,
            tc.tile_pool(name="aqp", bufs=2) as aqp,
            tc.tile_pool(name="psS", bufs=3, space="PSUM") as psS,
            tc.tile_pool(name="psav", bufs=2, space="PSUM") as psav,
        ):
            # ---------------- input DMAs ---------------------------------
            cbias_sb = consts.tile([128, 12], dt.float32, tag="cbias")
            nc.sync.dma_start(out=cbias_sb, in_=cbias_d[:])
            xs = persist.tile([128, 5, DIM], dt.float32, tag="xs")
            nc.sync.dma_start(
                out=xs[:, 0:2, :],
                in_=x_d[0:256, :].rearrange("(t p) c -> p t c", p=128))
            nc.sync.dma_start(
                out=xs[:, 2:5, :],
                in_=x_d[256:640, :].rearrange("(t p) c -> p t c", p=128))
            wqk_sb = consts.tile([128, 2, 512], dt.bfloat16, tag="wqk")
            nc.sync.dma_start(out=wqk_sb, in_=wqk_d[:])
            Eall_sb = consts.tile([128, E_NCOL], dt.bfloat16, tag="Eall")
            nc.sync.dma_start(out=Eall_sb[:, :E_NCOL // 2],
                              in_=Eall_d[:, :E_NCOL // 2])
            nc.sync.dma_start(out=Eall_sb[:, E_NCOL // 2:],
                              in_=Eall_d[:, E_NCOL // 2:])

            smallw_sb = consts.tile([1, 776], dt.bfloat16, tag="smallw")
            nc.sync.dma_start(out=smallw_sb, in_=smallw_d[:])
            psel_sb = consts.tile([1, 194], dt.float32, tag="psel")
            nc.sync.dma_start(out=psel_sb, in_=psel_d[:])
            wv_sb = consts.tile([128, 2, 264], dt.bfloat16, tag="wv")
            nc.sync.dma_start(out=wv_sb, in_=wv_d[:])
            pwT_sb = consts.tile([97, 4, DIM], dt.bfloat16, tag="pwT")
            nc.sync.dma_start(out=pwT_sb, in_=pwT_d[:])

            f1w_sb = consts.tile([128, 2, HID], dt.bfloat16, tag="f1w")
            nc.gpsimd.dma_start(out=f1w_sb, in_=f1w_d[:])
            f2w_sb = consts.tile([128, 8, DIM], dt.bfloat16, tag="f2w")
            nc.gpsimd.dma_start(out=f2w_sb, in_=f2w_d[:])

            vb_sb = smallw_sb[0:1, 0:264]
            pb_sb = smallw_sb[0:1, 264:520]
            f2b_sb = smallw_sb[0:1, 520:776]
            qkb_sb = cbias_sb[:, 0:4]
            f1b_sb = cbias_sb[:, 4:12]

            # E block views [128, 4, qspan]
            E_view = {}
            off = 0
            for quad in range(2):
                for t in range(5):
                    qspan = TILE_Q[t][1]
                    E_view[(quad, t)] = Eall_sb[:, off:off + 4 * qspan] \
                        .rearrange("p (s q) -> p s q", s=4)
                    off += 4 * qspan

            ident = consts.tile([128, 128], dt.bfloat16, tag="ident")
            make_identity(nc, ident)
            ones_sb = consts.tile([1, 512], dt.bfloat16, tag="ones")
            nc.vector.memset(ones_sb, 1.0)
            # ACT table preload (Exp) while DMAs land
            idummy = work.tile([1, 2], dt.float32, tag="idummy")
            nc.scalar.activation(out=idummy, in_=ones_sb[0:1, 0:2], func=Act.Exp)

            # ---------------- PE keepalive (HAM warm-up) ------------------
            def keepalive(n, nk=128):
                ka = psS.tile([128, 2, 512], dt.float32, tag="psS", name="ka")
                kaf = ka.rearrange("p a b -> p (a b)")
                for _ in range(n):
                    nc.tensor.matmul(kaf[:, :nk], ident, ident[:, :nk],
                                     start=True, stop=True)

            keepalive(52)

            # ---------------- helpers ----------------
            def dve_rsqrt(dst, src, n):
                ve = work.tile([128, n], dt.float32, tag="rsq_ve", bufs=2)
                nc.vector.tensor_scalar(out=ve, in0=src, scalar1=float(EPS),
                                        scalar2=None, op0=Alu.add)
                yi = work.tile([128, n], dt.int32, tag="rsq_yi", bufs=2)
                nc.vector.tensor_scalar(out=yi, in0=ve[:].bitcast(dt.int32),
                                        scalar1=1, scalar2=None,
                                        op0=Alu.logical_shift_right)
                nc.vector.tensor_scalar(out=yi, in0=yi, scalar1=-1,
                                        scalar2=MAGIC, op0=Alu.mult, op1=Alu.add)
                y = yi[:].bitcast(dt.float32)
                t = work.tile([128, n], dt.float32, tag="rsq_t", bufs=2)
                for _ in range(1):
                    nc.vector.tensor_tensor(out=t, in0=y, in1=y, op=Alu.mult)
                    nc.vector.tensor_tensor(out=t, in0=t, in1=ve, op=Alu.mult)
                    nc.vector.tensor_scalar(out=t, in0=t, scalar1=-0.5,
                                            scalar2=1.5, op0=Alu.mult, op1=Alu.add)
                    nc.vector.tensor_tensor(out=y, in0=y, in1=t, op=Alu.mult)
                nc.vector.tensor_copy(out=dst, in_=y)

            def ln_stats(x_list, tagp):
                """Returns (rstd, negmr) [128, ntile] f32 tiles."""
                ntile = len(x_list)
                mv = work.tile([128, 2 * ntile], dt.float32, tag=tagp + "_mv", bufs=2)
                for t, xt in enumerate(x_list):
                    stats = work.tile([128, 6], dt.float32, tag=tagp + "_st", bufs=2)
                    nc.vector.bn_stats(out=stats, in_=xt)
                    nc.vector.bn_aggr(out=mv[:, 2 * t:2 * t + 2], in_=stats)
                rstd = work.tile([128, ntile], dt.float32, tag=tagp + "_rs", bufs=2)
                dve_rsqrt(rstd, mv[:, 1::2], ntile)
                negmr = work.tile([128, ntile], dt.float32, tag=tagp + "_nm", bufs=2)
                nc.vector.scalar_tensor_tensor(out=negmr, in0=mv[:, 0::2],
                                               scalar=-1.0, in1=rstd,
                                               op0=Alu.mult, op1=Alu.mult)
                return rstd, negmr

            def transpose_tile(dst_cblocks, xh, col, scalar_evac=False):
                """PE-transpose xh [128, 256] bf16 into dst cblock tiles at
                column offset col (128 wide)."""
                for cb in range(2):
                    tp = psav.tile([128, 512], dt.float32, tag="pav", name=f"tp{col}_{cb}")
                    ptb = tp[:, 0:64].bitcast(dt.bfloat16)
                    nc.tensor.transpose(ptb, xh[:, 128 * cb:128 * (cb + 1)], ident)
                    if scalar_evac:
                        nc.scalar.activation(out=dst_cblocks[cb][:, col:col + 128],
                                             in_=ptb, func=Act.Copy)
                    else:
                        nc.vector.tensor_copy(
                            out=dst_cblocks[cb][:, col:col + 128], in_=ptb)

            # ---------------- LN1 + transpose + QKV + V ------------------
            xhatT = [persist.tile([128, NTP], dt.bfloat16, tag=f"xhatT{cb}",
                                  name=f"xhatT{cb}") for cb in range(2)]

            x_t = [xs[:, t, :] for t in range(5)]
            rs1a, nm1a = ln_stats(x_t[:2], "ln1a")
            xh_t = []
            for t in range(2):
                xh = work.tile([128, DIM], dt.bfloat16, tag=f"xh{t}", bufs=1)
                nc.scalar.activation(out=xh, in_=x_t[t], func=Act.Identity,
                                     scale=rs1a[:, t:t + 1], bias=nm1a[:, t:t + 1])
                xh_t.append(xh)
                transpose_tile(xhatT, xh, 128 * t)

            rs1b, nm1b = ln_stats(x_t[2:], "ln1b")
            for t in range(2, 5):
                xh = work.tile([128, DIM], dt.bfloat16, tag=f"xh{t}", bufs=1)
                nc.scalar.activation(out=xh, in_=x_t[t], func=Act.Identity,
                                     scale=rs1b[:, t - 2:t - 1], bias=nm1b[:, t - 2:t - 1])
                xh_t.append(xh)
                transpose_tile(xhatT, xh, 128 * t)

            # QKV: mt0 -> qT[0], mt1 -> qT[1], mt2 -> kT[0], mt3 -> kT[1]
            qT = [persist.tile([128, NQ], dt.bfloat16, tag=f"qT{i}", name=f"qT{i}")
                  for i in range(2)]
            kT = [persist.tile([128, NTP], dt.bfloat16, tag=f"kT{i}", name=f"kT{i}")
                  for i in range(2)]

            qkps = [psS.tile([128, 2, 512], dt.float32, tag="psS", name=f"qkps{i}")
                    .rearrange("p a b -> p (a b)") for i in range(3)]

            def qkv_chunk(mt, n0, nn):
                # qkps[0] holds mt0+mt1 (Q), qkps[1] mt2, qkps[2] mt3;
                # K chunk [512:640] goes in the other bank half.
                dst = (qT if mt < 2 else kT)[mt % 2]
                if mt < 2:
                    pt = qkps[0][:, 512 * mt + n0:512 * mt + n0 + nn]
                else:
                    base = 512 if n0 >= 512 else 0
                    pt = qkps[1 + mt % 2][:, base + (n0 % 512):base + (n0 % 512) + nn]
                for kb in range(2):
                    nc.tensor.matmul(pt, wqk_sb[:, kb, 128 * mt:128 * (mt + 1)],
                                     xhatT[kb][:, n0:n0 + nn],
                                     start=(kb == 0), stop=(kb == 1))
                nc.scalar.activation(out=dst[:, n0:n0 + nn], in_=pt,
                                     func=Act.Identity, bias=qkb_sb[:, mt:mt + 1])

            # early chunks for quad0 (needs T1 tiles 0,1 only)
            qkv_chunk(0, 0, 256)
            qkv_chunk(2, 0, 256)
            # rest
            qkv_chunk(0, 256, 256)
            qkv_chunk(2, 256, 256)
            qkv_chunk(2, 512, 128)
            qkv_chunk(1, 0, 512)
            qkv_chunk(3, 0, 512)
            qkv_chunk(3, 512, 128)

            # V per token tile: [128 tok, 264]
            vt = []
            for t in range(5):
                pv = psav.tile([128, 512], dt.float32, tag="pav", name=f"pv{t}")
                pvs = pv[:, 0:264]
                for kb in range(2):
                    nc.tensor.matmul(pvs, xhatT[kb][:, 128 * t:128 * (t + 1)],
                                     wv_sb[:, kb, :], start=(kb == 0), stop=False)
                nc.tensor.matmul(pvs, ones_sb[:, :128], vb_sb,
                                 start=False, stop=True)
                v = persist.tile([128, 264], dt.bfloat16, tag=f"vt{t}", name=f"vt{t}")
                nc.vector.tensor_copy(out=v, in_=pvs)
                vt.append(v)

            # ---------------- attention ----------------
            attnP2 = [persist.tile([97, 2, NQ], dt.bfloat16, tag=f"attnP{q}",
                                   name=f"attnP{q}") for q in range(2)]

            def split_banks(start, n):
                """Split f32 col range [start, start+n) at 512-boundaries."""
                pieces = []
                cur = start
                end = start + n
                while cur < end:
                    nxt = min(end, (cur // 512 + 1) * 512)
                    pieces.append((cur - start, nxt - cur))
                    cur = nxt
                return pieces

            def unit_av(quad, g, aq_tiles):
                pav = psav.tile([128, 512], dt.float32, tag="pav",
                                name=f"pav{quad}_{g}")
                tiles = G_TILES[g]
                for pl in range(2):
                    p = 2 * quad + pl
                    for sub in range(2):   # 0 = a-heads, 1 = b-heads
                        vlo = 66 * p + 33 * sub
                        po = 64 * sub
                        for i, t in enumerate(tiles):
                            qbase, qspan = TILE_Q[t]
                            qoff = 128 * g - qbase
                            nc.tensor.matmul(
                                pav[po:po + 33, 128 * pl:128 * (pl + 1)],
                                vt[t][:, vlo:vlo + 33],
                                aq_tiles[(t, pl)][:, sub, qoff:qoff + 128],
                                start=(i == 0), stop=(i == len(tiles) - 1))
                # denominators at rows 32 (a) / 96 (b), cols 0:256
                den = work.tile([1, 512], dt.float32, tag="den", bufs=3)
                if quad == 0:
                    nc.vector.tensor_copy(out=den[:, 0:256], in_=pav[32:33, 0:256])
                    nc.vector.tensor_copy(out=den[:, 256:512], in_=pav[96:97, 0:256])
                else:
                    nc.scalar.activation(out=den[:, 0:256], in_=pav[32:33, 0:256],
                                         func=Act.Copy)
                    nc.scalar.activation(out=den[:, 256:512], in_=pav[96:97, 0:256],
                                         func=Act.Copy)
                rcd = work.tile([1, 512], dt.float32, tag="rcd", bufs=3)
                nc.vector.reciprocal_approx_fast(out=rcd, in_=den)
                # PE broadcast of reciprocals (f32 matmuls)
                for pl in range(2):
                    pB = pav[:97, 256 + 128 * pl:384 + 128 * pl]
                    nc.tensor.matmul(pB, psel_sb[0:1, 0:97],
                                     rcd[:, 128 * pl:128 * (pl + 1)],
                                     start=True, stop=False)
                    nc.tensor.matmul(pB, psel_sb[0:1, 97:194],
                                     rcd[:, 256 + 128 * pl:384 + 128 * pl],
                                     start=False, stop=True)
                rcb = work.tile([97, 256], dt.bfloat16, tag="rcb", bufs=3)
                rcbeng = nc.vector if quad == 0 else nc.scalar
                if quad == 0:
                    nc.vector.tensor_copy(out=rcb, in_=pav[:97, 256:512])
                else:
                    nc.scalar.activation(out=rcb, in_=pav[:97, 256:512],
                                         func=Act.Copy)
                nc.vector.tensor_tensor(
                    out=attnP2[quad][:, :, 128 * g:128 * (g + 1)],
                    in0=pav[:97, 0:256].rearrange("p (two q) -> p two q", two=2),
                    in1=rcb.rearrange("p (two q) -> p two q", two=2),
                    op=Alu.mult)

            def proj_tail(mt):
                """proj + residual for query tile mt (= group mt)."""
                pt = psav.tile([128, 512], dt.float32, tag="pav", name=f"pproj{mt}")
                pp = pt[:, :DIM]
                for p in range(4):
                    nc.tensor.matmul(pp,
                                     attnP2[p // 2][:, p % 2, 128 * mt:128 * (mt + 1)],
                                     pwT_sb[:, p, :], start=(p == 0), stop=False)
                nc.tensor.matmul(pp, ones_sb[:, :128], pb_sb, start=False, stop=True)
                yt = persist.tile([128, DIM], dt.float32, tag=f"y{mt}")
                nc.vector.tensor_tensor(out=yt, in0=pp, in1=x_t[mt], op=Alu.add)
                return yt

            y_tiles = [None] * 4
            aq_cur = {}
            for quad in range(2):
                for t in range(5):
                    qbase, qspan = TILE_Q[t]
                    for pl in range(2):
                        pS = psS.tile([128, 2, 512], dt.float32, tag="psS",
                                      name=f"pS{quad}_{t}_{pl}")
                        for sl in range(2):
                            s = 2 * pl + sl
                            nc.tensor.matmul(
                                pS[:, sl, :qspan],
                                kT[quad][32 * s:32 * s + 32, 128 * t:128 * (t + 1)],
                                qT[quad][32 * s:32 * s + 32,
                                         qbase:qbase + qspan],
                                start=True, stop=True,
                                tile_position=(32 * s, 0))
                        aq = aqp.tile([128, 2, qspan], dt.bfloat16,
                                      tag=f"aq{t}_{pl}", name=f"aq{quad}_{t}_{pl}")
                        nc.scalar.activation(out=aq, in_=pS[:, :, :qspan],
                                             func=Act.Exp)
                        emeng = nc.gpsimd if t in (0, 4) else nc.vector
                        emeng.tensor_tensor(
                            out=aq.rearrange("p s q -> p (s q)"),
                            in0=aq.rearrange("p s q -> p (s q)"),
                            in1=E_view[(quad, t)][:, 2 * pl:2 * pl + 2, :]
                            .rearrange("p s q -> p (s q)"),
                            op=Alu.mult)
                        aq_cur[(t, pl)] = aq
                    for g in LAST_UNITS.get(t, []):
                        unit_av(quad, g, aq_cur)
                        if quad == 1:
                            y_tiles[g] = proj_tail(g)

            # gelu table preload right after last exp
            gdummy = work.tile([1, 2], dt.float32, tag="gdummy")
            nc.scalar.activation(out=gdummy, in_=ones_sb[0:1, 0:2], func=Act.Gelu)

            # ---------------- LN2 + MLP ----------------
            x2T = [persist.tile([128, NQ], dt.bfloat16, tag=f"x2T{cb}",
                                name=f"x2T{cb}") for cb in range(2)]
            for half in range(2):
                ys = y_tiles[2 * half:2 * half + 2]
                rs2, nm2 = ln_stats(ys, f"ln2{half}")
                for i, yt in enumerate(ys):
                    xh2 = work.tile([128, DIM], dt.bfloat16,
                                    tag=f"xh2_{half}{i}", bufs=1)
                    nc.vector.tensor_scalar(out=xh2, in0=yt,
                                            scalar1=rs2[:, i:i + 1],
                                            scalar2=nm2[:, i:i + 1],
                                            op0=Alu.mult, op1=Alu.add)
                    transpose_tile(x2T, xh2, 128 * (2 * half + i),
                                   scalar_evac=True)

            m1 = []
            for mh in range(8):
                if mh % 2 == 0:
                    f1ps = psS.tile([128, 2, 512], dt.float32, tag="psS",
                                    name=f"f1ps{mh}").rearrange("p a b -> p (a b)")
                pt = f1ps[:, 512 * (mh % 2):512 * (mh % 2) + 512]
                for kb in range(2):
                    nc.tensor.matmul(pt, f1w_sb[:, kb, 128 * mh:128 * (mh + 1)],
                                     x2T[kb], start=(kb == 0), stop=(kb == 1))
                mg = persist.tile([128, NQ], dt.bfloat16, tag=f"m1_{mh}")
                nc.scalar.activation(out=mg, in_=pt, func=Act.Gelu,
                                     bias=f1b_sb[:, mh:mh + 1], scale=1.0)
                m1.append(mg)

            for mt in range(4):
                pt = psav.tile([128, 512], dt.float32, tag="pav", name=f"pfc2{mt}")
                pp = pt[:, :DIM]
                for kb in range(8):
                    nc.tensor.matmul(pp, m1[kb][:, 128 * mt:128 * (mt + 1)],
                                     f2w_sb[:, kb, :], start=(kb == 0), stop=False)
                nc.tensor.matmul(pp, ones_sb[:, :128], f2b_sb, start=False, stop=True)
                ot = work.tile([128, DIM], dt.float32, tag="outt", bufs=2)
                nc.vector.tensor_tensor(out=ot, in0=pp, in1=y_tiles[mt][:],
                                        op=Alu.add)
                eng = nc.sync if mt % 2 == 0 else nc.gpsimd
                eng.dma_start(out=out_d[128 * mt:128 * (mt + 1), :], in_=ot)

    nc.finalize()
    return nc


# --------------------------------------------------------------------------
# Entry point
# --------------------------------------------------------------------------

def kernel(**inputs):
    from concourse.bass_utils import run_bass_kernel_spmd

    if 'nc' not in _CACHE:
        _CACHE['nc'] = build_graph()
    nc = _CACHE['nc']

    in_maps = _prepare_inputs(inputs)
    res = run_bass_kernel_spmd(nc, in_maps, core_ids=list(range(8)))
    x = np.asarray(inputs['x'])
    Bsz, Hh, Ww, C = x.shape
    out = np.zeros((Bsz, Hh, Ww, C), np.float32)
    for i in range(2 * Bsz):
        b, half = divmod(i, 2)
        o = np.asarray(res.results[i]['out']).reshape(16, Ww, C)
        if half == 0:
            out[b, 0:16] = o
        else:
            out[b, 16:32] = o[::-1]
    return out.astype(x.dtype)


# revision 16
# speedup vs baseline: 1.2753x; 1.0790x over previous
"""Trainium2 Bass kernel for AdaNSABlock (7x7 neighborhood attention block).

Sharding: 8 cores = batch(4) x row-halves(2). Each core computes 16 image
rows (512 tokens) of one sample, reading 19 rows (3-row halo) of input
padded to 640 tokens (5 clean 128-token tiles). Bottom halves are
row-flipped on host so all cores run one SPMD graph.

v2 rewrite vs v1:
  - tokens padded to 640: all tiles 128 wide, key chunks = absolute
    128-aligned token tiles (no dedup, V computed once per tile)
  - tile-major S: per (quad, key-tile) one wide-N matmul per head slot
    (split only at PSUM bank boundaries) -> ~52 S matmuls vs 88, far
    fewer LDWEIGHTS
  - one merged exp ACTIVATE per (quad, tile) over [128, 4*qspan]
  - one E-multiply per (quad, tile); small tiles on GpSimd, rest Vector
  - AV per unit as 4 sequential accumulation chains (M=33 a/b col-tiles)
  - softmax denominators: reciprocal_approx_fast straight off PSUM, then
    DMA partition-broadcast (stride-0 src) instead of PE broadcast matmuls
  - input DMAs split across sync/scalar/gpsimd queues by need-time
"""

import numpy as np
import ml_dtypes

KS = 7
HEADS = 8
DIM = 256
HID = 1024
HD = 32
H = 32
W = 32
NT = 19 * 32          # valid local tokens incl halo
NTP = 640             # padded tokens (5 tiles of 128)
NQ = 512              # query tokens per core
EPS = 1e-5
BF16 = ml_dtypes.bfloat16

# tile-major schedule: per key-tile t -> (qbase, qspan)
TILE_Q = [(0, 256), (0, 384), (128, 384), (256, 256), (384, 128)]
# per query group g -> key tiles used
G_TILES = [[0, 1], [0, 1, 2], [1, 2, 3], [2, 3, 4]]
# units finishing at tile t
LAST_UNITS = {1: [0], 2: [1], 3: [2], 4: [3]}
E_NCOL = sum(4 * qs for (_, qs) in TILE_Q) * 2   # 11264

MAGIC = 0x5F3759DF

_CACHE = {}


def _bf(x):
    return np.ascontiguousarray(np.asarray(x, np.float32).astype(BF16))


def _f32(x):
    return np.ascontiguousarray(np.asarray(x, np.float32))


# --------------------------------------------------------------------------
# Host-side folding + E construction
# --------------------------------------------------------------------------

def _fold_weights(inp):
    quality = inp['quality']
    s = int(quality) - 1
    l = float(quality % 1)
    g1 = np.abs(np.asarray(inp['gamma_1'], np.float64))
    g2 = np.abs(np.asarray(inp['gamma_2'], np.float64))
    if s == g1.shape[0] - 1:
        G1, G2 = g1[s], g2[s]
    else:
        G1 = g1[s] ** (1 - l) * g1[s + 1] ** l
        G2 = g2[s] ** (1 - l) * g2[s + 1] ** l

    qkv_w = np.asarray(inp['qkv_w'], np.float64)
    qkv_b = np.asarray(inp['qkv_b'], np.float64)
    n1w = np.asarray(inp['norm1_w'], np.float64)
    n1b = np.asarray(inp['norm1_b'], np.float64)
    Wq = qkv_w * n1w[None, :]
    bq = qkv_b + qkv_w @ n1b
    sc = HD ** -0.5
    Wq[:DIM] *= sc
    bq[:DIM] *= sc

    pw = G1[:, None] * np.asarray(inp['proj_w'], np.float64)
    pb = G1 * np.asarray(inp['proj_b'], np.float64)

    n2w = np.asarray(inp['norm2_w'], np.float64)
    n2b = np.asarray(inp['norm2_b'], np.float64)
    f1w = np.asarray(inp['fc1_w'], np.float64) * n2w[None, :]
    f1b = np.asarray(inp['fc1_b'], np.float64) + np.asarray(inp['fc1_w'], np.float64) @ n2b
    f2w = G2[:, None] * np.asarray(inp['fc2_w'], np.float64)
    f2b = G2 * np.asarray(inp['fc2_b'], np.float64)

    Wv = Wq[2 * DIM:]
    bv = bq[2 * DIM:]
    # V'' pair p cols [66p..66p+66): [32 a-dims][one_a][32 b-dims][one_b]
    Wv_aug = np.zeros((DIM, 264))
    vb_aug = np.zeros(264)
    for p in range(4):
        ha, hb = 2 * p, 2 * p + 1
        base = 66 * p
        Wv_aug[:, base:base + 32] = Wv[32 * ha:32 * ha + 32].T
        vb_aug[base:base + 32] = bv[32 * ha:32 * ha + 32]
        vb_aug[base + 32] = 1.0
        Wv_aug[:, base + 33:base + 65] = Wv[32 * hb:32 * hb + 32].T
        vb_aug[base + 33:base + 65] = bv[32 * hb:32 * hb + 32]
        vb_aug[base + 65] = 1.0

    def kblocked(wT, kb):
        n = wT.shape[1]
        return np.ascontiguousarray(wT.reshape(kb, 128, n).transpose(1, 0, 2))

    # smallw bf16 [1, 776]: vb(264) | pb(256) | f2b(256)
    smallw = np.zeros((1, 776))
    smallw[0, 0:264] = vb_aug
    smallw[0, 264:520] = pb
    smallw[0, 520:776] = f2b

    # psel f32 [1, 194]: rows-0..63 selector | rows-64..96 selector
    psel = np.zeros((1, 194))
    psel[0, 0:64] = 1.0
    psel[0, 97 + 64:97 + 97] = 1.0

    # cbias f32 [128, 12]: qkb (q0,q1,k0,k1) | f1b (8 cols)
    cbias = np.zeros((128, 12))
    for mt in range(4):
        cbias[:, mt] = bq[128 * mt:128 * mt + 128]
    for mh in range(8):
        cbias[:, 4 + mh] = f1b[128 * mh:128 * mh + 128]

    # proj lhsT from attnT pair tiles [97, 128]: rows 0-31 head 2p (den row
    # 32 zero-weighted), rows 64-95 head 2p+1 (den row 96)
    pwT_aug = np.zeros((4, 97, DIM))
    for p in range(4):
        ha, hb = 2 * p, 2 * p + 1
        pwT_aug[p, 0:32] = pw[:, 32 * ha:32 * ha + 32].T
        pwT_aug[p, 64:96] = pw[:, 32 * hb:32 * hb + 32].T

    return dict(
        wqk=_bf(kblocked(Wq[:512].T, 2)),       # [128, 2, 512]
        wv=_bf(kblocked(Wv_aug, 2)),            # [128, 2, 264]
        f1w=_bf(kblocked(f1w.T, 2)),            # [128, 2, 1024]
        f2w=_bf(kblocked(f2w.T, 8)),            # [128, 8, 256]
        pwT=_bf(np.ascontiguousarray(pwT_aug.transpose(1, 0, 2))),  # [97, 4, 256]
        smallw=_bf(smallw),                     # [1, 776]
        psel=_f32(psel),                        # [1, 194]
        cbias=_f32(cbias),                      # [128, 12]
        rpb=np.asarray(inp['rpb'], np.float64),
    )


def _build_E(rpb, flip):
    """Tile-major E: [128 keys, (quad, t, slot, qspan)] -> [128, 11264] f32."""
    def img_row(r):
        return (31 - r) if flip else r
    blocks = []
    for quad in range(2):
        for t in range(5):
            qbase, qspan = TILE_Q[t]
            kk = np.arange(128)[:, None, None]
            ss = np.arange(4)[None, :, None]
            qq = qbase + np.arange(qspan)[None, None, :]
            tk = 128 * t + kk
            ki_loc, kj = tk // 32, tk % 32
            qi_loc, qj = qq // 32, qq % 32
            qi = img_row(qi_loc)
            ki = img_row(np.minimum(ki_loc, 19))
            sh = np.clip(qi - 3, 0, H - KS)
            sw = np.clip(qj - 3, 0, W - KS)
            valid = (tk < NT) & (ki >= sh) & (ki < sh + KS) \
                & (kj >= sw) & (kj < sw + KS)
            bh = np.clip(ki - qi + KS - 1, 0, 2 * KS - 2)
            bw = np.clip(kj - qj + KS - 1, 0, 2 * KS - 2)
            h = 4 * quad + ss
            bias = rpb[h, bh, bw]
            E = np.where(valid, np.exp(bias), 0.0)
            blocks.append(np.ascontiguousarray(E.reshape(128, 4 * qspan)))
    return np.concatenate(blocks, axis=1)


def _prepare_inputs(inp):
    F = _fold_weights(inp)
    E_top = _bf(_build_E(F['rpb'], flip=False))
    E_bot = _bf(_build_E(F['rpb'], flip=True))
    x = np.asarray(inp['x'], np.float32)
    Bsz = x.shape[0]
    shared = {k: v for k, v in F.items() if k != 'rpb'}
    in_maps = []
    for b in range(Bsz):
        for half in range(2):
            if half == 0:
                x_loc = x[b, 0:19].reshape(NT, DIM)
                E = E_top
            else:
                x_loc = x[b, 31:12:-1].reshape(NT, DIM)
                E = E_bot
            x_pad = np.zeros((NTP, DIM), np.float32)
            x_pad[:NT] = x_loc
            m = dict(shared)
            m['x'] = x_pad
            m['Eall'] = E
            in_maps.append(m)
    return in_maps


# --------------------------------------------------------------------------
# Bass kernel graph
# --------------------------------------------------------------------------

def build_graph():
    import concourse.bass as bass
    import concourse.tile as tile
    import concourse.mybir as mybir
    from concourse import bacc
    from concourse.masks import make_identity

    dt = mybir.dt
    Alu = mybir.AluOpType
    Act = mybir.ActivationFunctionType

    nc = bacc.Bacc()

    def param(name, shape, dtype, out=False):
        return nc.declare_dram_parameter(name, list(shape), dtype, isOutput=out)

    x_d = param("x", (NTP, DIM), dt.float32)
    wqk_d = param("wqk", (128, 2, 512), dt.bfloat16)
    wv_d = param("wv", (128, 2, 264), dt.bfloat16)
    f1w_d = param("f1w", (128, 2, HID), dt.bfloat16)
    f2w_d = param("f2w", (128, 8, DIM), dt.bfloat16)
    pwT_d = param("pwT", (97, 4, DIM), dt.bfloat16)
    smallw_d = param("smallw", (1, 776), dt.bfloat16)
    psel_d = param("psel", (1, 194), dt.float32)
    cbias_d = param("cbias", (128, 12), dt.float32)
    Eall_d = param("Eall", (128, E_NCOL), dt.bfloat16)
    out_d = param("out", (NQ, DIM), dt.float32, out=True)

    def bcast_ap(src, nparts):
        """Partition-broadcast AP: repeat a [1, n] AP across nparts."""
        return bass.AP(tensor=src.tensor, offset=src.offset,
                       ap=[[0, nparts]] + list(src.ap[1:]))

    with tile.TileContext(nc) as tc:
        with (
            tc.tile_pool(name="consts", bufs=1) as consts,
            tc.tile_pool(name="persist", bufs=1) as persist,
            tc.tile_pool(name="work", bufs=3) as work,
            tc.tile_pool(name="aqp", bufs=2) as aqp,
            tc.tile_pool(name="psS", bufs=2, space="PSUM") as psS,
            tc.tile_pool(name="psav", bufs=4, space="PSUM") as psav,
        ):
            # ---------------- input DMAs ---------------------------------
            cbias_sb = consts.tile([128, 12], dt.float32, tag="cbias")
            nc.sync.dma_start(out=cbias_sb, in_=cbias_d[:])
            xs = persist.tile([128, 5, DIM], dt.float32, tag="xs")
            nc.sync.dma_start(
                out=xs[:, 0:2, :],
                in_=x_d[0:256, :].rearrange("(t p) c -> p t c", p=128))
            nc.sync.dma_start(
                out=xs[:, 2:5, :],
                in_=x_d[256:640, :].rearrange("(t p) c -> p t c", p=128))
            wqk_sb = consts.tile([128, 2, 512], dt.bfloat16, tag="wqk")
            nc.sync.dma_start(out=wqk_sb, in_=wqk_d[:])
            Eall_sb = consts.tile([128, E_NCOL], dt.bfloat16, tag="Eall")
            nc.sync.dma_start(out=Eall_sb[:, :E_NCOL // 2],
                              in_=Eall_d[:, :E_NCOL // 2])
            nc.sync.dma_start(out=Eall_sb[:, E_NCOL // 2:],
                              in_=Eall_d[:, E_NCOL // 2:])

            smallw_sb = consts.tile([1, 776], dt.bfloat16, tag="smallw")
            nc.sync.dma_start(out=smallw_sb, in_=smallw_d[:])
            psel_sb = consts.tile([1, 194], dt.float32, tag="psel")
            nc.sync.dma_start(out=psel_sb, in_=psel_d[:])
            wv_sb = consts.tile([128, 2, 264], dt.bfloat16, tag="wv")
            nc.sync.dma_start(out=wv_sb, in_=wv_d[:])
            pwT_sb = consts.tile([97, 4, DIM], dt.bfloat16, tag="pwT")
            nc.sync.dma_start(out=pwT_sb, in_=pwT_d[:])

            f1w_sb = consts.tile([128, 2, HID], dt.bfloat16, tag="f1w")
            nc.gpsimd.dma_start(out=f1w_sb, in_=f1w_d[:])
            f2w_sb = consts.tile([128, 8, DIM], dt.bfloat16, tag="f2w")
            nc.gpsimd.dma_start(out=f2w_sb, in_=f2w_d[:])

            vb_sb = smallw_sb[0:1, 0:264]
            pb_sb = smallw_sb[0:1, 264:520]
            f2b_sb = smallw_sb[0:1, 520:776]
            qkb_sb = cbias_sb[:, 0:4]
            f1b_sb = cbias_sb[:, 4:12]

            # E block views [128, 4, qspan]
            E_view = {}
            off = 0
            for quad in range(2):
                for t in range(5):
                    qspan = TILE_Q[t][1]
                    E_view[(quad, t)] = Eall_sb[:, off:off + 4 * qspan] \
                        .rearrange("p (s q) -> p s q", s=4)
                    off += 4 * qspan

            ident = consts.tile([128, 128], dt.bfloat16, tag="ident")
            make_identity(nc, ident)
            ones_sb = consts.tile([1, 512], dt.bfloat16, tag="ones")
            nc.vector.memset(ones_sb, 1.0)
            # ACT table preload (Exp) while DMAs land
            idummy = work.tile([1, 2], dt.float32, tag="idummy")
            nc.scalar.activation(out=idummy, in_=ones_sb[0:1, 0:2], func=Act.Exp)

            # ---------------- PE keepalive (HAM warm-up) ------------------
            def keepalive(n, nk=128):
                ka = psS.tile([128, 2, 512], dt.float32, tag="psS", name="ka")
                kaf = ka.rearrange("p a b -> p (a b)")
                for _ in range(n):
                    nc.tensor.matmul(kaf[:, :nk], ident, ident[:, :nk],
                                     start=True, stop=True)

            keepalive(52)

            # ---------------- helpers ----------------
            def dve_rsqrt(dst, src, n):
                ve = work.tile([128, n], dt.float32, tag="rsq_ve", bufs=2)
                nc.vector.tensor_scalar(out=ve, in0=src, scalar1=float(EPS),
                                        scalar2=None, op0=Alu.add)
                yi = work.tile([128, n], dt.int32, tag="rsq_yi", bufs=2)
                nc.vector.tensor_scalar(out=yi, in0=ve[:].bitcast(dt.int32),
                                        scalar1=1, scalar2=None,
                                        op0=Alu.logical_shift_right)
                nc.vector.tensor_scalar(out=yi, in0=yi, scalar1=-1,
                                        scalar2=MAGIC, op0=Alu.mult, op1=Alu.add)
                y = yi[:].bitcast(dt.float32)
                t = work.tile([128, n], dt.float32, tag="rsq_t", bufs=2)
                for _ in range(1):
                    nc.vector.tensor_tensor(out=t, in0=y, in1=y, op=Alu.mult)
                    nc.vector.tensor_tensor(out=t, in0=t, in1=ve, op=Alu.mult)
                    nc.vector.tensor_scalar(out=t, in0=t, scalar1=-0.5,
                                            scalar2=1.5, op0=Alu.mult, op1=Alu.add)
                    nc.vector.tensor_tensor(out=y, in0=y, in1=t, op=Alu.mult)
                nc.vector.tensor_copy(out=dst, in_=y)

            def ln_stats(x_list, tagp):
                """Returns (rstd, negmr) [128, ntile] f32 tiles."""
                ntile = len(x_list)
                mv = work.tile([128, 2 * ntile], dt.float32, tag=tagp + "_mv", bufs=2)
                for t, xt in enumerate(x_list):
                    stats = work.tile([128, 6], dt.float32, tag=tagp + "_st", bufs=2)
                    nc.vector.bn_stats(out=stats, in_=xt)
                    nc.vector.bn_aggr(out=mv[:, 2 * t:2 * t + 2], in_=stats)
                rstd = work.tile([128, ntile], dt.float32, tag=tagp + "_rs", bufs=2)
                dve_rsqrt(rstd, mv[:, 1::2], ntile)
                negmr = work.tile([128, ntile], dt.float32, tag=tagp + "_nm", bufs=2)
                nc.vector.scalar_tensor_tensor(out=negmr, in0=mv[:, 0::2],
                                               scalar=-1.0, in1=rstd,
                                               op0=Alu.mult, op1=Alu.mult)
                return rstd, negmr

            def transpose_tile(dst_cblocks, xh, col, scalar_evac=False):
                """PE-transpose xh [128, 256] bf16 into dst cblock tiles at
                column offset col (128 wide)."""
                for cb in range(2):
                    tp = psav.tile([128, 512], dt.float32, tag="pav", name=f"tp{col}_{cb}")
                    ptb = tp[:, 0:64].bitcast(dt.bfloat16)
                    nc.tensor.transpose(ptb, xh[:, 128 * cb:128 * (cb + 1)], ident)
                    if scalar_evac:
                        nc.scalar.activation(out=dst_cblocks[cb][:, col:col + 128],
                                             in_=ptb, func=Act.Copy)
                    else:
                        nc.vector.tensor_copy(
                            out=dst_cblocks[cb][:, col:col + 128], in_=ptb)

            # ---------------- LN1 + transpose + QKV + V ------------------
            xhatT = [persist.tile([128, NTP], dt.bfloat16, tag=f"xhatT{cb}",
                                  name=f"xhatT{cb}") for cb in range(2)]

            x_t = [xs[:, t, :] for t in range(5)]
            rs1a, nm1a = ln_stats(x_t[:2], "ln1a")
            xh_t = []
            for t in range(2):
                xh = work.tile([128, DIM], dt.bfloat16, tag=f"xh{t}", bufs=1)
                nc.scalar.activation(out=xh, in_=x_t[t], func=Act.Identity,
                                     scale=rs1a[:, t:t + 1], bias=nm1a[:, t:t + 1])
                xh_t.append(xh)
                transpose_tile(xhatT, xh, 128 * t)

            rs1b, nm1b = ln_stats(x_t[2:], "ln1b")
            for t in range(2, 5):
                xh = work.tile([128, DIM], dt.bfloat16, tag=f"xh{t}", bufs=1)
                nc.scalar.activation(out=xh, in_=x_t[t], func=Act.Identity,
                                     scale=rs1b[:, t - 2:t - 1], bias=nm1b[:, t - 2:t - 1])
                xh_t.append(xh)
                transpose_tile(xhatT, xh, 128 * t)

            # QKV: mt0 -> qT[0], mt1 -> qT[1], mt2 -> kT[0], mt3 -> kT[1]
            qT = [persist.tile([128, NQ], dt.bfloat16, tag=f"qT{i}", name=f"qT{i}")
                  for i in range(2)]
            kT = [persist.tile([128, NTP], dt.bfloat16, tag=f"kT{i}", name=f"kT{i}")
                  for i in range(2)]

            qkps = [psS.tile([128, 2, 512], dt.float32, tag="psS", name=f"qkps{i}")
                    .rearrange("p a b -> p (a b)") for i in range(3)]

            def qkv_chunk(mt, n0, nn):
                # qkps[0] holds mt0+mt1 (Q), qkps[1] mt2, qkps[2] mt3;
                # K chunk [512:640] goes in the other bank half.
                dst = (qT if mt < 2 else kT)[mt % 2]
                if mt < 2:
                    pt = qkps[0][:, 512 * mt + n0:512 * mt + n0 + nn]
                else:
                    base = 512 if n0 >= 512 else 0
                    pt = qkps[1 + mt % 2][:, base + (n0 % 512):base + (n0 % 512) + nn]
                for kb in range(2):
                    nc.tensor.matmul(pt, wqk_sb[:, kb, 128 * mt:128 * (mt + 1)],
                                     xhatT[kb][:, n0:n0 + nn],
                                     start=(kb == 0), stop=(kb == 1))
                nc.scalar.activation(out=dst[:, n0:n0 + nn], in_=pt,
                                     func=Act.Identity, bias=qkb_sb[:, mt:mt + 1])

            # early chunks for quad0 (needs T1 tiles 0,1 only)
            qkv_chunk(0, 0, 256)
            qkv_chunk(2, 0, 256)
            # rest
            qkv_chunk(0, 256, 256)
            qkv_chunk(2, 256, 256)
            qkv_chunk(2, 512, 128)
            qkv_chunk(1, 0, 512)
            qkv_chunk(3, 0, 512)
            qkv_chunk(3, 512, 128)

            # V per token tile: [128 tok, 264]
            vt = []
            for t in range(5):
                pv = psav.tile([128, 512], dt.float32, tag="pav", name=f"pv{t}")
                pvs = pv[:, 0:264]
                for kb in range(2):
                    nc.tensor.matmul(pvs, xhatT[kb][:, 128 * t:128 * (t + 1)],
                                     wv_sb[:, kb, :], start=(kb == 0), stop=False)
                nc.tensor.matmul(pvs, ones_sb[:, :128], vb_sb,
                                 start=False, stop=True)
                v = persist.tile([128, 264], dt.bfloat16, tag=f"vt{t}", name=f"vt{t}")
                nc.vector.tensor_copy(out=v, in_=pvs)
                vt.append(v)

            # ---------------- attention ----------------
            attnP2 = [persist.tile([97, 2, NQ], dt.bfloat16, tag=f"attnP{q}",
                                   name=f"attnP{q}") for q in range(2)]

            def split_banks(start, n):
                """Split f32 col range [start, start+n) at 512-boundaries."""
                pieces = []
                cur = start
                end = start + n
                while cur < end:
                    nxt = min(end, (cur // 512 + 1) * 512)
                    pieces.append((cur - start, nxt - cur))
                    cur = nxt
                return pieces

            def unit_av(quad, g, aq_tiles):
                pav = psav.tile([128, 512], dt.float32, tag="pav",
                                name=f"pav{quad}_{g}")
                tiles = G_TILES[g]
                for pl in range(2):
                    p = 2 * quad + pl
                    for sub in range(2):   # 0 = a-heads, 1 = b-heads
                        vlo = 66 * p + 33 * sub
                        po = 64 * sub
                        for i, t in enumerate(tiles):
                            qbase, qspan = TILE_Q[t]
                            qoff = 128 * g - qbase
                            nc.tensor.matmul(
                                pav[po:po + 33, 128 * pl:128 * (pl + 1)],
                                vt[t][:, vlo:vlo + 33],
                                aq_tiles[(t, pl)][:, sub, qoff:qoff + 128],
                                start=(i == 0), stop=(i == len(tiles) - 1))
                # denominators at rows 32 (a) / 96 (b), cols 0:256
                den = work.tile([1, 512], dt.float32, tag="den", bufs=3)
                if quad == 0:
                    nc.vector.tensor_copy(out=den[:, 0:256], in_=pav[32:33, 0:256])
                    nc.vector.tensor_copy(out=den[:, 256:512], in_=pav[96:97, 0:256])
                else:
                    nc.scalar.activation(out=den[:, 0:256], in_=pav[32:33, 0:256],
                                         func=Act.Copy)
                    nc.scalar.activation(out=den[:, 256:512], in_=pav[96:97, 0:256],
                                         func=Act.Copy)
                rcd = work.tile([1, 512], dt.float32, tag="rcd", bufs=3)
                nc.vector.reciprocal_approx_fast(out=rcd, in_=den)
                # PE broadcast of reciprocals (f32 matmuls)
                for pl in range(2):
                    pB = pav[:97, 256 + 128 * pl:384 + 128 * pl]
                    nc.tensor.matmul(pB, psel_sb[0:1, 0:97],
                                     rcd[:, 128 * pl:128 * (pl + 1)],
                                     start=True, stop=False)
                    nc.tensor.matmul(pB, psel_sb[0:1, 97:194],
                                     rcd[:, 256 + 128 * pl:384 + 128 * pl],
                                     start=False, stop=True)
                rcb = work.tile([97, 256], dt.bfloat16, tag="rcb", bufs=3)
                rcbeng = nc.vector if quad == 0 else nc.scalar
                if quad == 0:
                    nc.vector.tensor_copy(out=rcb, in_=pav[:97, 256:512])
                else:
                    nc.scalar.activation(out=rcb, in_=pav[:97, 256:512],
                                         func=Act.Copy)
                nc.vector.tensor_tensor(
                    out=attnP2[quad][:, :, 128 * g:128 * (g + 1)],
                    in0=pav[:97, 0:256].rearrange("p (two q) -> p two q", two=2),
                    in1=rcb.rearrange("p (two q) -> p two q", two=2),
                    op=Alu.mult)

            def proj_tail(mt):
                """proj + residual for query tile mt (= group mt)."""
                pt = psav.tile([128, 512], dt.float32, tag="pav", name=f"pproj{mt}")
                pp = pt[:, :DIM]
                for p in range(4):
                    nc.tensor.matmul(pp,
                                     attnP2[p // 2][:, p % 2, 128 * mt:128 * (mt + 1)],
                                     pwT_sb[:, p, :], start=(p == 0), stop=False)
                nc.tensor.matmul(pp, ones_sb[:, :128], pb_sb, start=False, stop=True)
                yt = persist.tile([128, DIM], dt.float32, tag=f"y{mt}")
                nc.vector.tensor_tensor(out=yt, in0=pp, in1=x_t[mt], op=Alu.add)
                return yt

            y_tiles = [None] * 4
            aq_cur = {}
            for quad in range(2):
                for t in range(5):
                    qbase, qspan = TILE_Q[t]
                    for pl in range(2):
                        pS = psS.tile([128, 2, 512], dt.float32, tag="psS",
                                      name=f"pS{quad}_{t}_{pl}")
                        for sl in range(2):
                            s = 2 * pl + sl
                            nc.tensor.matmul(
                                pS[:, sl, :qspan],
                                kT[quad][32 * s:32 * s + 32, 128 * t:128 * (t + 1)],
                                qT[quad][32 * s:32 * s + 32,
                                         qbase:qbase + qspan],
                                start=True, stop=True,
                                tile_position=(32 * s, 0))
                        aq = aqp.tile([128, 2, qspan], dt.bfloat16,
                                      tag=f"aq{t}_{pl}", name=f"aq{quad}_{t}_{pl}")
                        nc.scalar.activation(out=aq, in_=pS[:, :, :qspan],
                                             func=Act.Exp)
                        emeng = nc.gpsimd if t in (0, 4) else nc.vector
                        emeng.tensor_tensor(
                            out=aq.rearrange("p s q -> p (s q)"),
                            in0=aq.rearrange("p s q -> p (s q)"),
                            in1=E_view[(quad, t)][:, 2 * pl:2 * pl + 2, :]
                            .rearrange("p s q -> p (s q)"),
                            op=Alu.mult)
                        aq_cur[(t, pl)] = aq
                    for g in LAST_UNITS.get(t, []):
                        unit_av(quad, g, aq_cur)
                        if quad == 1:
                            y_tiles[g] = proj_tail(g)

            # gelu table preload right after last exp
            gdummy = work.tile([1, 2], dt.float32, tag="gdummy")
            nc.scalar.activation(out=gdummy, in_=ones_sb[0:1, 0:2], func=Act.Gelu)

            # ---------------- LN2 + MLP ----------------
            x2T = [persist.tile([128, NQ], dt.bfloat16, tag=f"x2T{cb}",
                                name=f"x2T{cb}") for cb in range(2)]
            for half in range(2):
                ys = y_tiles[2 * half:2 * half + 2]
                rs2, nm2 = ln_stats(ys, f"ln2{half}")
                for i, yt in enumerate(ys):
                    xh2 = work.tile([128, DIM], dt.bfloat16,
                                    tag=f"xh2_{half}{i}", bufs=1)
                    nc.vector.tensor_scalar(out=xh2, in0=yt,
                                            scalar1=rs2[:, i:i + 1],
                                            scalar2=nm2[:, i:i + 1],
                                            op0=Alu.mult, op1=Alu.add)
                    transpose_tile(x2T, xh2, 128 * (2 * half + i),
                                   scalar_evac=True)

            m1 = []
            for mh in range(8):
                if mh % 2 == 0:
                    f1ps = psS.tile([128, 2, 512], dt.float32, tag="psS",
                                    name=f"f1ps{mh}").rearrange("p a b -> p (a b)")
                pt = f1ps[:, 512 * (mh % 2):512 * (mh % 2) + 512]
                for kb in range(2):
                    nc.tensor.matmul(pt, f1w_sb[:, kb, 128 * mh:128 * (mh + 1)],
                                     x2T[kb], start=(kb == 0), stop=(kb == 1))
                mg = persist.tile([128, NQ], dt.bfloat16, tag=f"m1_{mh}")
                nc.scalar.activation(out=mg, in_=pt, func=Act.Gelu,
                                     bias=f1b_sb[:, mh:mh + 1], scale=1.0)
                m1.append(mg)

            for mt in range(4):
                pt = psav.tile([128, 512], dt.float32, tag="pav", name=f"pfc2{mt}")
                pp = pt[:, :DIM]
                for kb in range(8):
                    nc.tensor.matmul(pp, m1[kb][:, 128 * mt:128 * (mt + 1)],
                                     f2w_sb[:, kb, :], start=(kb == 0), stop=False)
                nc.tensor.matmul(pp, ones_sb[:, :128], f2b_sb, start=False, stop=True)
                ot = work.tile([128, DIM], dt.float32, tag="outt", bufs=2)
                nc.vector.tensor_tensor(out=ot, in0=pp, in1=y_tiles[mt][:],
                                        op=Alu.add)
                eng = nc.sync if mt % 2 == 0 else nc.gpsimd
                eng.dma_start(out=out_d[128 * mt:128 * (mt + 1), :], in_=ot)

    nc.finalize()
    return nc


# --------------------------------------------------------------------------
# Entry point
# --------------------------------------------------------------------------

def kernel(**inputs):
    from concourse.bass_utils import run_bass_kernel_spmd

    if 'nc' not in _CACHE:
        _CACHE['nc'] = build_graph()
    nc = _CACHE['nc']

    in_maps = _prepare_inputs(inputs)
    res = run_bass_kernel_spmd(nc, in_maps, core_ids=list(range(8)))
    x = np.asarray(inputs['x'])
    Bsz, Hh, Ww, C = x.shape
    out = np.zeros((Bsz, Hh, Ww, C), np.float32)
    for i in range(2 * Bsz):
        b, half = divmod(i, 2)
        o = np.asarray(res.results[i]['out']).reshape(16, Ww, C)
        if half == 0:
            out[b, 0:16] = o
        else:
            out[b, 16:32] = o[::-1]
    return out.astype(x.dtype)


# revision 17
# speedup vs baseline: 1.3584x; 1.0652x over previous
"""Trainium2 Bass kernel for AdaNSABlock (7x7 neighborhood attention block).

Sharding: 8 cores = batch(4) x row-halves(2). Each core computes 16 image
rows (512 tokens) of one sample, reading 19 rows (3-row halo) of input.
Bottom halves are row-flipped on host so all cores run one SPMD graph.

v1 rewrite vs baseline:
  - input DMAs spread across sync/scalar/gpsimd queues, QKV weights first
  - LN normalize via ACT Identity (scale=rstd, bias=-mean*rstd)
  - Q/K bias folded into ACT Identity evacuation (no bias matmuls)
  - V key-windows computed directly from xhatT column slices (no SBUF DMAs)
  - attention software-pipelined at (group, quad) granularity:
      PE stream:  S(u) | pB(u-2) | AV(u-1); PSUM 4 S-banks + 4 AV-banks
      Scalar: exp;  Vector: E-mult + den/recip;  GpSimd: pB evac + normalize
  - proj on 128-row packed attnT (K=128), gelu table preloaded off-path
  - keepalive matmuls bridge PE gaps to hold the p-state ramp
"""

import numpy as np
import ml_dtypes

KS = 7
HEADS = 8
DIM = 256
HID = 1024
HD = 32
H = 32
W = 32
NT = 19 * 32          # local tokens incl halo
NQ = 512              # query tokens per core
EPS = 1e-5
BF16 = ml_dtypes.bfloat16

# token tiles covering NT
TOK_TILES = [(0, 128), (128, 128), (256, 128), (384, 128), (512, 96)]
# groups: (key_base_token, chunk_starts)
GROUPS = [(0, (0, 96)), (32, (0, 128, 192)), (160, (0, 128, 192)), (288, (0, 128, 192))]
# distinct V key-windows (start tokens, each 128 wide), in first-use order
WINDOW_STARTS = [0, 96, 32, 160, 224, 288, 352, 416, 480]
WIN_IDX = {s: i for i, s in enumerate(WINDOW_STARTS)}

MAGIC = 0x5F3759DF

_CACHE = {}


def _bf(x):
    return np.ascontiguousarray(np.asarray(x, np.float32).astype(BF16))


def _f32(x):
    return np.ascontiguousarray(np.asarray(x, np.float32))


# --------------------------------------------------------------------------
# Host-side folding + mask construction
# --------------------------------------------------------------------------

def _fold_weights(inp):
    quality = inp['quality']
    s = int(quality) - 1
    l = float(quality % 1)
    g1 = np.abs(np.asarray(inp['gamma_1'], np.float64))
    g2 = np.abs(np.asarray(inp['gamma_2'], np.float64))
    if s == g1.shape[0] - 1:
        G1, G2 = g1[s], g2[s]
    else:
        G1 = g1[s] ** (1 - l) * g1[s + 1] ** l
        G2 = g2[s] ** (1 - l) * g2[s + 1] ** l

    qkv_w = np.asarray(inp['qkv_w'], np.float64)
    qkv_b = np.asarray(inp['qkv_b'], np.float64)
    n1w = np.asarray(inp['norm1_w'], np.float64)
    n1b = np.asarray(inp['norm1_b'], np.float64)
    Wq = qkv_w * n1w[None, :]
    bq = qkv_b + qkv_w @ n1b
    sc = HD ** -0.5
    Wq[:DIM] *= sc
    bq[:DIM] *= sc

    pw = G1[:, None] * np.asarray(inp['proj_w'], np.float64)
    pb = G1 * np.asarray(inp['proj_b'], np.float64)

    n2w = np.asarray(inp['norm2_w'], np.float64)
    n2b = np.asarray(inp['norm2_b'], np.float64)
    f1w = np.asarray(inp['fc1_w'], np.float64) * n2w[None, :]
    f1b = np.asarray(inp['fc1_b'], np.float64) + np.asarray(inp['fc1_w'], np.float64) @ n2b
    f2w = G2[:, None] * np.asarray(inp['fc2_w'], np.float64)
    f2b = G2 * np.asarray(inp['fc2_b'], np.float64)

    Wv = Wq[2 * DIM:]            # [256 vdims, 256 c]
    bv = bq[2 * DIM:]
    # ones-augmented V: pair p = heads (2p, 2p+1), V'' cols 97p..97p+96:
    #   col 0..31  = dims of head 2p      (psum rows 0..31)
    #   col 32     = one_a                (psum row 32 = denom_a)
    #   col 64..95 = dims of head 2p+1    (psum rows 64..95)
    #   col 96     = one_b                (psum row 96 = denom_b)
    # head-a AV matmul: M=64 cols [0:64); head-b: M=33 cols [64:97)
    # (partition offsets 0/32/64/96 keep every slice quarter-aligned)
    Wv_aug = np.zeros((DIM, 388))
    vb_aug = np.zeros(388)
    for p in range(4):
        ha, hb = 2 * p, 2 * p + 1
        base = 97 * p
        Wv_aug[:, base:base + 32] = Wv[32 * ha:32 * ha + 32].T
        vb_aug[base:base + 32] = bv[32 * ha:32 * ha + 32]
        vb_aug[base + 32] = 1.0
        Wv_aug[:, base + 64:base + 96] = Wv[32 * hb:32 * hb + 32].T
        vb_aug[base + 64:base + 96] = bv[32 * hb:32 * hb + 32]
        vb_aug[base + 96] = 1.0

    def kblocked(wT, kb):
        # [kb*128, N] -> [128, kb, N] (partition-major, contiguous for DMA)
        n = wT.shape[1]
        return np.ascontiguousarray(wT.reshape(kb, 128, n).transpose(1, 0, 2))

    # pairsel for recip broadcast: two K=1 matmuls per pair, lhsT [1, 97]
    # psel_a ones at rows 0..63 (head-a block), psel_b ones at rows 64..96
    # smallw bf16 [1, 1094]: pairsel(194) | vb | pb | f2b
    smallw = np.zeros((1, 1094))
    smallw[0, 0:64] = 1.0
    smallw[0, 97 + 64:97 + 97] = 1.0
    smallw[0, 194:582] = vb_aug
    smallw[0, 582:838] = pb
    smallw[0, 838:1094] = f2b

    # cbias f32 [128, 12]: qkb (q0,q1,k0,k1) | f1b (8 cols)
    cbias = np.zeros((128, 12))
    for mt in range(4):
        cbias[:, mt] = bq[128 * mt:128 * mt + 128]
    for mh in range(8):
        cbias[:, 4 + mh] = f1b[128 * mh:128 * mh + 128]

    # proj lhsT comes from attnT pair tiles [97, 128]: pair p rows 0-31 =
    # head 2p (+den rows 32/96 junk -> zero weight), rows 64-95 = head 2p+1.
    pwT_aug = np.zeros((4, 97, DIM))
    for p in range(4):
        ha, hb = 2 * p, 2 * p + 1
        pwT_aug[p, 0:32] = pw[:, 32 * ha:32 * ha + 32].T
        pwT_aug[p, 64:96] = pw[:, 32 * hb:32 * hb + 32].T

    return dict(
        wqk=_bf(kblocked(Wq[:512].T, 2)),       # [128, 2, 512]
        wv=_bf(kblocked(Wv_aug, 2)),            # [128, 2, 388]
        f1w=_bf(kblocked(f1w.T, 2)),            # [128, 2, 1024]
        f2w=_bf(kblocked(f2w.T, 8)),            # [128, 8, 256]
        pwT=_bf(np.ascontiguousarray(pwT_aug.transpose(1, 0, 2))),  # [97, 4, 256]
        smallw=_bf(smallw),                     # [1, 1094]
        cbias=_f32(cbias),                      # [128, 12]
        rpb=np.asarray(inp['rpb'], np.float64),
    )


def _build_E(rpb, flip):
    """Vectorized E (exp of bias, masked/dedup-zeroed).
    Returns E_edge [8,128,256], E_std [8,128,384] float32."""
    def img_row(r):
        return (31 - r) if flip else r

    def make(group):
        if group == 0:
            keybase, chunk_starts = 0, np.array([0, 96])
        else:
            keybase, chunk_starts = (4 * group - 3) * 32, np.array([0, 128, 192])
        nch = len(chunk_starts)
        a = np.arange(4)[:, None, None, None]         # q row in group
        qj = np.arange(32)[None, :, None, None]
        c = np.arange(nch)[None, None, :, None]
        kk = np.arange(128)[None, None, None, :]
        key = chunk_starts[c] + kk                    # rel key idx
        tloc = (keybase + key) // 32
        kj = (keybase + key) % 32
        rloc_q = 4 * group + a
        qi = img_row(rloc_q)
        ki = img_row(tloc)
        sh = np.clip(qi - 3, 0, H - KS)
        sw = np.clip(qj - 3, 0, H - KS)
        valid = (ki >= sh) & (ki < sh + KS) & (kj >= sw) & (kj < sw + KS)
        if nch > 1:
            dedup = ~((c > 0) & (key < chunk_starts[np.maximum(c - 1, 0)] + 128))
            valid = valid & dedup
        bh = np.clip(ki - qi + KS - 1, 0, 2 * KS - 2)
        bw = np.clip(kj - qj + KS - 1, 0, 2 * KS - 2)
        # [8, 4, 32, nch, 128]
        bias = rpb[:, bh, bw]
        E = np.where(valid[None], np.exp(bias), 0.0)
        return np.ascontiguousarray(
            E.reshape(HEADS, 4 * 32, nch * 128).astype(np.float32))
    return make(0), make(1)


def _prepare_inputs(inp):
    F = _fold_weights(inp)
    E_e_t, E_s_t = _build_E(F['rpb'], flip=False)
    E_e_b, E_s_b = _build_E(F['rpb'], flip=True)
    x = np.asarray(inp['x'], np.float32)
    Bsz = x.shape[0]
    shared = {k: v for k, v in F.items() if k != 'rpb'}
    in_maps = []
    for b in range(Bsz):
        for half in range(2):
            if half == 0:
                x_loc = x[b, 0:19].reshape(NT, DIM)
                Ee, Es = E_e_t, E_s_t
            else:
                x_loc = x[b, 31:12:-1].reshape(NT, DIM)
                Ee, Es = E_e_b, E_s_b
            m = dict(shared)
            m['x'] = _f32(x_loc)
            # device layout: [key-within-chunk(128), head, chunk*128 + q]
            def dev(E):
                nch = E.shape[2] // 128
                return np.ascontiguousarray(
                    E.reshape(HEADS, 128, nch, 128)
                    .transpose(3, 0, 2, 1)
                    .reshape(128, HEADS * nch * 128))
            m['Eall'] = _bf(np.concatenate([dev(Ee), dev(Es)], axis=1))
            in_maps.append(m)
    return in_maps


# --------------------------------------------------------------------------
# Bass kernel graph
# --------------------------------------------------------------------------

def build_graph():
    import concourse.bass as bass
    import concourse.tile as tile
    import concourse.mybir as mybir
    from concourse import bacc
    from concourse.masks import make_identity

    dt = mybir.dt
    Alu = mybir.AluOpType
    Act = mybir.ActivationFunctionType

    nc = bacc.Bacc()

    def param(name, shape, dtype, out=False):
        return nc.declare_dram_parameter(name, list(shape), dtype, isOutput=out)

    x_d = param("x", (NT, DIM), dt.float32)
    wqk_d = param("wqk", (128, 2, 512), dt.bfloat16)
    wv_d = param("wv", (128, 2, 388), dt.bfloat16)
    f1w_d = param("f1w", (128, 2, HID), dt.bfloat16)
    f2w_d = param("f2w", (128, 8, DIM), dt.bfloat16)
    pwT_d = param("pwT", (97, 4, DIM), dt.bfloat16)
    smallw_d = param("smallw", (1, 1094), dt.bfloat16)
    cbias_d = param("cbias", (128, 12), dt.float32)
    Eall_d = param("Eall", (128, HEADS * (256 + 384)), dt.bfloat16)
    out_d = param("out", (NQ, DIM), dt.float32, out=True)

    with tile.TileContext(nc) as tc:
        with (
            tc.tile_pool(name="consts", bufs=1) as consts,
            tc.tile_pool(name="persist", bufs=1) as persist,
            tc.tile_pool(name="work", bufs=3) as work,
            tc.tile_pool(name="aq", bufs=3) as aqpool,
            tc.tile_pool(name="ps", bufs=6, space="PSUM") as ps,
            tc.tile_pool(name="psav", bufs=2, space="PSUM") as psav,
        ):
            # ---------------- input DMAs ---------------------------------
            # All issued from sync in need-order: transfers appear to drain
            # a shared DMA device serially, so order = landing order.
            x_tiles = []
            for t, (off, nt) in enumerate(TOK_TILES):
                xt = persist.tile([128, DIM], dt.float32, tag=f"x{t}", name=f"x{t}")
                nc.sync.dma_start(out=xt[:nt], in_=x_d[off:off + nt, :])
                x_tiles.append(xt)
            cbias_sb = consts.tile([128, 12], dt.float32, tag="cbias")
            nc.sync.dma_start(out=cbias_sb, in_=cbias_d[:])
            wqk_sb = consts.tile([128, 2, 512], dt.bfloat16, tag="wqk")
            nc.sync.dma_start(out=wqk_sb, in_=wqk_d[:])
            smallw_sb = consts.tile([1, 1094], dt.bfloat16, tag="smallw")
            nc.sync.dma_start(out=smallw_sb, in_=smallw_d[:])
            wv_sb = consts.tile([128, 2, 388], dt.bfloat16, tag="wv")
            nc.sync.dma_start(out=wv_sb, in_=wv_d[:])
            Eall_sb = consts.tile([128, HEADS * 640], dt.bfloat16, tag="Eall")
            nc.sync.dma_start(out=Eall_sb, in_=Eall_d[:])
            pwT_sb = consts.tile([97, 4, DIM], dt.bfloat16, tag="pwT")
            nc.sync.dma_start(out=pwT_sb, in_=pwT_d[:])
            f1w_sb = consts.tile([128, 2, HID], dt.bfloat16, tag="f1w")
            nc.sync.dma_start(out=f1w_sb, in_=f1w_d[:])
            f2w_sb = consts.tile([128, 8, DIM], dt.bfloat16, tag="f2w")
            nc.sync.dma_start(out=f2w_sb, in_=f2w_d[:])

            psel_a = smallw_sb[0:1, 0:97]
            psel_b = smallw_sb[0:1, 97:194]
            vb_sb = smallw_sb[0:1, 194:582]
            pb_sb = smallw_sb[0:1, 582:838]
            f2b_sb = smallw_sb[0:1, 838:1094]
            qkb_sb = cbias_sb[:, 0:4]
            f1b_sb = cbias_sb[:, 4:12]

            ident = consts.tile([128, 128], dt.bfloat16, tag="ident")
            make_identity(nc, ident)
            Ee_sb = Eall_sb[:, 0:HEADS * 256].rearrange("p (h c) -> p h c", h=HEADS)
            Es_sb = Eall_sb[:, HEADS * 256:].rearrange("p (h c) -> p h c", h=HEADS)

            ones_sb = consts.tile([1, NT], dt.bfloat16, tag="ones")
            nc.vector.memset(ones_sb, 1.0)
            # preload the ACT table set early (Identity lives in every set,
            # but the very first ACTIVATE pays the table load — hide it here)
            idummy = work.tile([1, 2], dt.float32, tag="idummy")
            nc.scalar.activation(out=idummy, in_=ones_sb[0:1, 0:2], func=Act.Exp)

            # ---------------- PE keepalive (p-state ramp) -----------------
            def keepalive(n, nk=128):
                ka = ps.tile([128, 512], dt.float32, tag="pss", name="ka")
                for _ in range(n):
                    nc.tensor.matmul(ka[:, :nk], ident, ident[:, :nk],
                                     start=True, stop=True)

            keepalive(75)

            # ---------------- helpers ----------------
            def dve_rsqrt(dst, src, n):
                """dst[:,0:n] = 1/sqrt(src[:,0:n] + EPS); small-n f32 tiles."""
                ve = work.tile([128, n], dt.float32, tag="rsq_ve", bufs=2)
                nc.vector.tensor_scalar(out=ve, in0=src, scalar1=float(EPS),
                                        scalar2=None, op0=Alu.add)
                yi = work.tile([128, n], dt.int32, tag="rsq_yi", bufs=2)
                nc.vector.tensor_scalar(out=yi, in0=ve[:].bitcast(dt.int32),
                                        scalar1=1, scalar2=None,
                                        op0=Alu.logical_shift_right)
                nc.vector.tensor_scalar(out=yi, in0=yi, scalar1=-1,
                                        scalar2=MAGIC, op0=Alu.mult, op1=Alu.add)
                y = yi[:].bitcast(dt.float32)
                t = work.tile([128, n], dt.float32, tag="rsq_t", bufs=2)
                for _ in range(2):
                    nc.vector.tensor_tensor(out=t, in0=y, in1=y, op=Alu.mult)
                    nc.vector.tensor_tensor(out=t, in0=t, in1=ve, op=Alu.mult)
                    nc.vector.tensor_scalar(out=t, in0=t, scalar1=-0.5,
                                            scalar2=1.5, op0=Alu.mult, op1=Alu.add)
                    nc.vector.tensor_tensor(out=y, in0=y, in1=t, op=Alu.mult)
                nc.vector.tensor_copy(out=dst, in_=y)

            def ln_batch(x_list, sizes, tagp):
                """LayerNorm a batch of tiles. Vector: stats + 1/(v+eps);
                Scalar: sqrt -> rstd; Vector: -m*rstd;
                Scalar: per-tile Identity ACT (x*r - m*r) -> bf16 xhat.
                Returns list of (xhat_tile, nt)."""
                ntile = len(x_list)
                mv = work.tile([128, 2 * ntile], dt.float32, tag=tagp + "_mv", bufs=2)
                nc.vector.memset(mv, 1.0)
                for t, (xt, nt) in enumerate(zip(x_list, sizes)):
                    stats = work.tile([128, 6], dt.float32, tag=tagp + "_st", bufs=2)
                    nc.vector.bn_stats(out=stats[:nt], in_=xt[:nt])
                    nc.vector.bn_aggr(out=mv[:nt, 2 * t:2 * t + 2], in_=stats[:nt])
                rstd = work.tile([128, ntile], dt.float32, tag=tagp + "_rs", bufs=2)
                dve_rsqrt(rstd, mv[:, 1::2], ntile)
                negmr = work.tile([128, ntile], dt.float32, tag=tagp + "_nm", bufs=2)
                nc.vector.scalar_tensor_tensor(out=negmr, in0=mv[:, 0::2],
                                               scalar=-1.0, in1=rstd,
                                               op0=Alu.mult, op1=Alu.mult)
                outs = []
                for t, (xt, nt) in enumerate(zip(x_list, sizes)):
                    xh = persist.tile([128, DIM], dt.bfloat16,
                                      tag=f"{tagp}_xh{t}", name=f"{tagp}_xh{t}")
                    nc.scalar.activation(out=xh[:nt], in_=xt[:nt],
                                         func=Act.Identity,
                                         scale=rstd[:nt, t:t + 1],
                                         bias=negmr[:nt, t:t + 1])
                    outs.append((xh, nt))
                return outs

            def make_tcat(tagp, total):
                return [persist.tile([128, total], dt.bfloat16,
                                     tag=f"{tagp}_{cb}", name=f"{tagp}_{cb}")
                        for cb in range(2)]

            def transpose_into(res, xh_tiles, off, evac_engines, ei0=0):
                """Transpose [nt, 256] bf16 tiles into res c-block tiles at
                column offset off. Returns next offset."""
                ei = ei0
                for xh, nt in xh_tiles:
                    for cb in range(2):
                        ptb = ps.tile([128, 128], dt.bfloat16, tag="pss", name="ptb")
                        nc.tensor.transpose(ptb[:, :nt], xh[:nt, 128 * cb:128 * (cb + 1)],
                                            ident[:nt, :nt])
                        eng = evac_engines[ei % len(evac_engines)]
                        ei += 1
                        eng.tensor_copy(out=res[cb][:, off:off + nt], in_=ptb[:, :nt])
                    off += nt
                return off

            # ---------------- LN1 (two batches) + transposes --------------
            xhatT = make_tcat("xhatT", NT)
            b1 = ln_batch(x_tiles[:3], [128, 128, 128], "ln1a")
            b2 = ln_batch(x_tiles[3:], [128, 96], "ln1b")
            transpose_into(xhatT, b1 + b2, 0, [nc.vector])

            # ---------------- QKV ----------------
            # Q (mt 0,1), K (mt 2,3): psum accum over 2 k-blocks,
            # bias via Identity-ACT evacuation.
            qT, kT = [], []
            for mt in range(4):
                ncols = NQ if mt < 2 else NT
                dst = persist.tile([128, ncols], dt.bfloat16, tag=f"qk{mt}")
                for n0 in range(0, ncols, 512):
                    nn = min(512, ncols - n0)
                    pt = ps.tile([128, 512], dt.float32, tag="pss", name="pqk")
                    for kb in range(2):
                        nc.tensor.matmul(
                            pt[:, :nn], wqk_sb[:, kb, 128 * mt:128 * (mt + 1)],
                            xhatT[kb][:, n0:n0 + nn],
                            start=(kb == 0), stop=(kb == 1))
                    # Q evacs on Scalar, K evacs on Vector (both Identity+bias
                    # capable? DVE copy can't add bias -> K bias via Scalar too)
                    nc.scalar.activation(out=dst[:, n0:n0 + nn], in_=pt[:, :nn],
                                         func=Act.Identity,
                                         bias=qkb_sb[:, mt:mt + 1])
                (qT if mt < 2 else kT).append(dst)

            # V key-windows computed lazily: scheduled into the attention
            # pipeline (see VW_SCHED) so early units start sooner.
            Vw = [None] * len(WINDOW_STARTS)

            def make_vwindow(wi):
                ws = WINDOW_STARTS[wi]
                vt = persist.tile([128, 388], dt.bfloat16, tag=f"vw{wi}", name=f"vw{wi}")
                pt = ps.tile([128, 512], dt.float32, tag="pss", name=f"pv{wi}")
                pv = pt[:, :388]
                for kb in range(2):
                    nc.tensor.matmul(pv, xhatT[kb][:, ws:ws + 128], wv_sb[:, kb, :],
                                     start=(kb == 0), stop=False)
                nc.tensor.matmul(pv, ones_sb[:, :128], vb_sb, start=False, stop=True)
                if wi % 2 == 0:
                    nc.vector.tensor_copy(out=vt, in_=pv)
                else:
                    nc.scalar.activation(out=vt, in_=pv, func=Act.Copy)
                Vw[wi] = vt

            # ---------------- attention: 8 (group, quad) units ------------
            # attnP2[quad]: [97, pair-local(2), NQ] — normalized attention^T
            attnP2 = [persist.tile([97, 2, NQ], dt.bfloat16, tag=f"attnP{q}",
                                   name=f"attnP{q}") for q in range(2)]

            UNITS = [(g, q) for g in range(4) for q in range(2)]

            def unit_S(u):
                """S^T matmuls + exp for unit u. Returns aq tile."""
                g, quad = UNITS[u]
                kb_tok, css = GROUPS[g]
                nch = len(css)
                aq = aqpool.tile([128, 4, 384], dt.bfloat16, tag="aq",
                                 name=f"aq_{u}")
                for slot in range(4):
                    pS = ps.tile([128, 512], dt.float32, tag="pss",
                                 name=f"pS_{u}_{slot}")
                    for c, cs in enumerate(css):
                        nc.tensor.matmul(
                            pS[:, 128 * c:128 * (c + 1)],
                            kT[quad][32 * slot:32 * slot + 32,
                                     kb_tok + cs:kb_tok + cs + 128],
                            qT[quad][32 * slot:32 * slot + 32,
                                     128 * g:128 * (g + 1)],
                            start=True, stop=True,
                            tile_position=(32 * slot, 0))
                    nc.scalar.activation(
                        out=aq[:, slot, :128 * nch],
                        in_=pS[:, :128 * nch], func=Act.Exp)
                return aq

            def unit_emult(u, aq):
                """aq *= E for unit u (pair 0 on GpSimd, pair 1 on Vector)."""
                g, quad = UNITS[u]
                nch = len(GROUPS[g][1])
                E_sb = Ee_sb if g == 0 else Es_sb
                for pl, eng in ((0, nc.gpsimd), (1, nc.vector)):
                    eng.tensor_tensor(
                        out=aq[:, 2 * pl:2 * pl + 2, :128 * nch],
                        in0=aq[:, 2 * pl:2 * pl + 2, :128 * nch],
                        in1=E_sb[:, 4 * quad + 2 * pl:4 * quad + 2 * pl + 2,
                                 :128 * nch],
                        op=Alu.mult)

            def unit_AV(u, aq):
                """AV matmuls for unit u — whole unit in ONE psum bank:
                pair0 numerators cols 0:128, pair1 cols 128:256,
                broadcasts go to 256:384 / 384:512 later.
                Returns (pav, rcd)."""
                g, quad = UNITS[u]
                kb_tok, css = GROUPS[g]
                nch = len(css)
                pav = psav.tile([128, 512], dt.float32, tag="psav",
                                name=f"pav_{u}")
                for pl in range(2):
                    p = 2 * quad + pl
                    pnum = pav[:97, 128 * pl:128 * (pl + 1)]
                    for hh_loc, po, mm in ((2 * pl, 0, 64), (2 * pl + 1, 64, 33)):
                        voff = 97 * p + (0 if po == 0 else 64)
                        for c, cs in enumerate(css):
                            vt = Vw[WIN_IDX[kb_tok + cs]]
                            nc.tensor.matmul(
                                pnum[po:po + mm, :],
                                vt[:, voff:voff + mm],
                                aq[:, hh_loc, 128 * c:128 * (c + 1)],
                                start=(c == 0), stop=(c == nch - 1),
                                tile_position=(0, po))
                # Vector: den gather (both pairs at once), one reciprocal.
                # den layout: [a0 | a1 | b0 | b1] blocks of 128
                den = work.tile([1, 512], dt.float32, tag="den", bufs=3)
                nc.vector.tensor_copy(out=den[:, 0:256], in_=pav[32:33, 0:256])
                nc.vector.tensor_copy(out=den[:, 256:512], in_=pav[96:97, 0:256])
                rcd = work.tile([1, 512], dt.float32, tag="rcd", bufs=3)
                nc.vector.reciprocal_approx_fast(out=rcd, in_=den)
                rc16 = work.tile([1, 512], dt.bfloat16, tag="rc16", bufs=3)
                nc.vector.tensor_copy(out=rc16, in_=rcd)
                return (pav, rc16)

            def unit_pB(avout, u):
                """PE broadcast of reciprocals (f32 matmuls), one evacuation,
                one Vector normalize into attnP2."""
                g, quad = UNITS[u]
                pav, rc16 = avout
                for pl in range(2):
                    pB = pav[:97, 256 + 128 * pl:384 + 128 * pl]
                    nc.tensor.matmul(pB, psel_a,
                                     rc16[:, 128 * pl:128 * (pl + 1)],
                                     start=True, stop=False)
                    nc.tensor.matmul(pB, psel_b,
                                     rc16[:, 256 + 128 * pl:384 + 128 * pl],
                                     start=False, stop=True)
                rcb = work.tile([97, 256], dt.bfloat16, tag="rcb", bufs=3)
                nc.vector.tensor_copy(out=rcb, in_=pav[:97, 256:512])
                nc.vector.tensor_tensor(
                    out=attnP2[quad][:, :, 128 * g:128 * (g + 1)],
                    in0=pav[:97, 0:256].rearrange("p (two q) -> p two q", two=2),
                    in1=rcb.rearrange("p (two q) -> p two q", two=2),
                    op=Alu.mult)

            # software pipeline: PE order  Vw | S(u) | pB(u-2) | AV(u-1)
            # V windows land just before the units that need them, filling
            # the early-pipeline PE bubbles.
            VW_SCHED = {0: [0, 1], 1: [2, 3], 2: [4, 5], 3: [6, 7], 4: [8]}
            aqs = [None] * 8
            avouts = [None] * 8
            for u in range(8):
                for wi in VW_SCHED.get(u, []):
                    make_vwindow(wi)
                aqs[u] = unit_S(u)
                if u >= 2:
                    unit_pB(avouts[u - 2], u - 2)
                unit_emult(u, aqs[u])
                if u >= 1:
                    avouts[u - 1] = unit_AV(u - 1, aqs[u - 1])
            avouts[7] = unit_AV(7, aqs[7])
            unit_pB(avouts[6], 6)
            unit_pB(avouts[7], 7)

            # ---------------- proj + residual ----------------
            keepalive(8)
            y_tiles = []
            for mt in range(4):
                pt = ps.tile([128, 512], dt.float32, tag="pss", name=f"pproj{mt}")
                pp = pt[:, :DIM]
                for p in range(4):
                    nc.tensor.matmul(pp,
                                     attnP2[p // 2][:, p % 2, 128 * mt:128 * (mt + 1)],
                                     pwT_sb[:, p, :], start=(p == 0), stop=False)
                nc.tensor.matmul(pp, ones_sb[:, :128], pb_sb, start=False, stop=True)
                yt = persist.tile([128, DIM], dt.float32, tag=f"y{mt}")
                nc.vector.tensor_tensor(out=yt, in0=pp, in1=x_tiles[mt][:],
                                        op=Alu.add)
                y_tiles.append(yt)

            # gelu table preload (dummy) while LN2 runs
            gdummy = work.tile([1, 2], dt.float32, tag="gdummy")
            nc.scalar.activation(out=gdummy, in_=ones_sb[0:1, 0:2], func=Act.Gelu)

            # ---------------- LN2 + MLP ----------------
            x2T = make_tcat("x2T", NQ)
            xh2a = ln_batch(y_tiles[:2], [128, 128], "ln2a")
            xh2b = ln_batch(y_tiles[2:], [128, 128], "ln2b")
            keepalive(16)
            transpose_into(x2T, xh2a + xh2b, 0, [nc.vector])

            keepalive(6)
            m1 = []
            for mh in range(8):
                pt = ps.tile([128, 512], dt.float32, tag="pss", name=f"pfc1{mh}")
                for kb in range(2):
                    nc.tensor.matmul(pt, f1w_sb[:, kb, 128 * mh:128 * (mh + 1)],
                                     x2T[kb], start=(kb == 0), stop=(kb == 1))
                mg = persist.tile([128, NQ], dt.bfloat16, tag=f"m1_{mh}")
                nc.scalar.activation(out=mg, in_=pt, func=Act.Gelu,
                                     bias=f1b_sb[:, mh:mh + 1], scale=1.0)
                m1.append(mg)

            # fc2 mt-major: finish each output tile early, DMA out alternating
            for mt in range(4):
                pt = ps.tile([128, 512], dt.float32, tag="pss", name=f"pfc2{mt}")
                pp = pt[:, :DIM]
                for kb in range(8):
                    nc.tensor.matmul(pp, m1[kb][:, 128 * mt:128 * (mt + 1)],
                                     f2w_sb[:, kb, :], start=(kb == 0), stop=False)
                nc.tensor.matmul(pp, ones_sb[:, :128], f2b_sb, start=False, stop=True)
                ot = work.tile([128, DIM], dt.float32, tag="outt", bufs=2)
                nc.vector.tensor_tensor(out=ot, in0=pp, in1=y_tiles[mt][:],
                                        op=Alu.add)
                eng = nc.sync if mt % 2 == 0 else nc.gpsimd
                eng.dma_start(out=out_d[128 * mt:128 * (mt + 1), :], in_=ot)

    nc.finalize()
    return nc


# --------------------------------------------------------------------------
# Entry point
# --------------------------------------------------------------------------

def kernel(**inputs):
    from concourse.bass_utils import run_bass_kernel_spmd

    if 'nc' not in _CACHE:
        _CACHE['nc'] = build_graph()
    nc = _CACHE['nc']

    in_maps = _prepare_inputs(inputs)
    res = run_bass_kernel_spmd(nc, in_maps, core_ids=list(range(8)))
    x = np.asarray(inputs['x'])
    Bsz, Hh, Ww, C = x.shape
    out = np.zeros((Bsz, Hh, Ww, C), np.float32)
    for i in range(2 * Bsz):
        b, half = divmod(i, 2)
        o = np.asarray(res.results[i]['out']).reshape(16, Ww, C)
        if half == 0:
            out[b, 0:16] = o
        else:
            out[b, 16:32] = o[::-1]
    return out.astype(x.dtype)



# revision 18
# speedup vs baseline: 1.3672x; 1.0065x over previous
"""Trainium2 Bass kernel for AdaNSABlock (7x7 neighborhood attention block).

Sharding: 8 cores = batch(4) x row-halves(2). Each core computes 16 image
rows (512 tokens) of one sample, reading 19 rows (3-row halo) of input.
Bottom halves are row-flipped on host so all cores run one SPMD graph.

v1 rewrite vs baseline:
  - input DMAs spread across sync/scalar/gpsimd queues, QKV weights first
  - LN normalize via ACT Identity (scale=rstd, bias=-mean*rstd)
  - Q/K bias folded into ACT Identity evacuation (no bias matmuls)
  - V key-windows computed directly from xhatT column slices (no SBUF DMAs)
  - attention software-pipelined at (group, quad) granularity:
      PE stream:  S(u) | pB(u-2) | AV(u-1); PSUM 4 S-banks + 4 AV-banks
      Scalar: exp;  Vector: E-mult + den/recip;  GpSimd: pB evac + normalize
  - proj on 128-row packed attnT (K=128), gelu table preloaded off-path
  - keepalive matmuls bridge PE gaps to hold the p-state ramp
"""

import numpy as np
import ml_dtypes

KS = 7
HEADS = 8
DIM = 256
HID = 1024
HD = 32
H = 32
W = 32
NT = 19 * 32          # local tokens incl halo
NQ = 512              # query tokens per core
EPS = 1e-5
BF16 = ml_dtypes.bfloat16

# token tiles covering NT
TOK_TILES = [(0, 128), (128, 128), (256, 128), (384, 128), (512, 96)]
# groups: (key_base_token, chunk_starts)
GROUPS = [(0, (0, 96)), (32, (0, 128, 192)), (160, (0, 128, 192)), (288, (0, 128, 192))]
# distinct V key-windows (start tokens, each 128 wide), in first-use order
WINDOW_STARTS = [0, 96, 32, 160, 224, 288, 352, 416, 480]
WIN_IDX = {s: i for i, s in enumerate(WINDOW_STARTS)}

MAGIC = 0x5F3759DF

_CACHE = {}


def _bf(x):
    return np.ascontiguousarray(np.asarray(x, np.float32).astype(BF16))


def _f32(x):
    return np.ascontiguousarray(np.asarray(x, np.float32))


# --------------------------------------------------------------------------
# Host-side folding + mask construction
# --------------------------------------------------------------------------

def _fold_weights(inp):
    quality = inp['quality']
    s = int(quality) - 1
    l = float(quality % 1)
    g1 = np.abs(np.asarray(inp['gamma_1'], np.float64))
    g2 = np.abs(np.asarray(inp['gamma_2'], np.float64))
    if s == g1.shape[0] - 1:
        G1, G2 = g1[s], g2[s]
    else:
        G1 = g1[s] ** (1 - l) * g1[s + 1] ** l
        G2 = g2[s] ** (1 - l) * g2[s + 1] ** l

    qkv_w = np.asarray(inp['qkv_w'], np.float64)
    qkv_b = np.asarray(inp['qkv_b'], np.float64)
    n1w = np.asarray(inp['norm1_w'], np.float64)
    n1b = np.asarray(inp['norm1_b'], np.float64)
    Wq = qkv_w * n1w[None, :]
    bq = qkv_b + qkv_w @ n1b
    sc = HD ** -0.5
    Wq[:DIM] *= sc
    bq[:DIM] *= sc

    pw = G1[:, None] * np.asarray(inp['proj_w'], np.float64)
    pb = G1 * np.asarray(inp['proj_b'], np.float64)

    n2w = np.asarray(inp['norm2_w'], np.float64)
    n2b = np.asarray(inp['norm2_b'], np.float64)
    f1w = np.asarray(inp['fc1_w'], np.float64) * n2w[None, :]
    f1b = np.asarray(inp['fc1_b'], np.float64) + np.asarray(inp['fc1_w'], np.float64) @ n2b
    f2w = G2[:, None] * np.asarray(inp['fc2_w'], np.float64)
    f2b = G2 * np.asarray(inp['fc2_b'], np.float64)

    Wv = Wq[2 * DIM:]            # [256 vdims, 256 c]
    bv = bq[2 * DIM:]
    # ones-augmented V: pair p = heads (2p, 2p+1), V'' cols 97p..97p+96:
    #   col 0..31  = dims of head 2p      (psum rows 0..31)
    #   col 32     = one_a                (psum row 32 = denom_a)
    #   col 64..95 = dims of head 2p+1    (psum rows 64..95)
    #   col 96     = one_b                (psum row 96 = denom_b)
    # head-a AV matmul: M=64 cols [0:64); head-b: M=33 cols [64:97)
    # (partition offsets 0/32/64/96 keep every slice quarter-aligned)
    Wv_aug = np.zeros((DIM, 388))
    vb_aug = np.zeros(388)
    for p in range(4):
        ha, hb = 2 * p, 2 * p + 1
        base = 97 * p
        Wv_aug[:, base:base + 32] = Wv[32 * ha:32 * ha + 32].T
        vb_aug[base:base + 32] = bv[32 * ha:32 * ha + 32]
        vb_aug[base + 32] = 1.0
        Wv_aug[:, base + 64:base + 96] = Wv[32 * hb:32 * hb + 32].T
        vb_aug[base + 64:base + 96] = bv[32 * hb:32 * hb + 32]
        vb_aug[base + 96] = 1.0

    def kblocked(wT, kb):
        # [kb*128, N] -> [128, kb, N] (partition-major, contiguous for DMA)
        n = wT.shape[1]
        return np.ascontiguousarray(wT.reshape(kb, 128, n).transpose(1, 0, 2))

    # pairsel for recip broadcast: two K=1 matmuls per pair, lhsT [1, 97]
    # psel_a ones at rows 0..63 (head-a block), psel_b ones at rows 64..96
    # smallw bf16 [1, 1094]: pairsel(194) | vb | pb | f2b
    smallw = np.zeros((1, 1094))
    smallw[0, 0:64] = 1.0
    smallw[0, 97 + 64:97 + 97] = 1.0
    smallw[0, 194:582] = vb_aug
    smallw[0, 582:838] = pb
    smallw[0, 838:1094] = f2b

    # cbias f32 [128, 12]: qkb (q0,q1,k0,k1) | f1b (8 cols)
    cbias = np.zeros((128, 12))
    for mt in range(4):
        cbias[:, mt] = bq[128 * mt:128 * mt + 128]
    for mh in range(8):
        cbias[:, 4 + mh] = f1b[128 * mh:128 * mh + 128]

    # proj lhsT comes from attnT pair tiles [97, 128]: pair p rows 0-31 =
    # head 2p (+den rows 32/96 junk -> zero weight), rows 64-95 = head 2p+1.
    pwT_aug = np.zeros((4, 97, DIM))
    for p in range(4):
        ha, hb = 2 * p, 2 * p + 1
        pwT_aug[p, 0:32] = pw[:, 32 * ha:32 * ha + 32].T
        pwT_aug[p, 64:96] = pw[:, 32 * hb:32 * hb + 32].T

    return dict(
        wqk=_bf(kblocked(Wq[:512].T, 2)),       # [128, 2, 512]
        wv=_bf(kblocked(Wv_aug, 2)),            # [128, 2, 388]
        f1w=_bf(kblocked(f1w.T, 2)),            # [128, 2, 1024]
        f2w=_bf(kblocked(f2w.T, 8)),            # [128, 8, 256]
        pwT=_bf(np.ascontiguousarray(pwT_aug.transpose(1, 0, 2))),  # [97, 4, 256]
        smallw=_bf(smallw),                     # [1, 1094]
        cbias=_f32(cbias),                      # [128, 12]
        rpb=np.asarray(inp['rpb'], np.float64),
    )


def _build_E(rpb, flip):
    """Vectorized E (exp of bias, masked/dedup-zeroed).
    Returns E_edge [8,128,256], E_std [8,128,384] float32."""
    def img_row(r):
        return (31 - r) if flip else r

    def make(group):
        if group == 0:
            keybase, chunk_starts = 0, np.array([0, 96])
        else:
            keybase, chunk_starts = (4 * group - 3) * 32, np.array([0, 128, 192])
        nch = len(chunk_starts)
        a = np.arange(4)[:, None, None, None]         # q row in group
        qj = np.arange(32)[None, :, None, None]
        c = np.arange(nch)[None, None, :, None]
        kk = np.arange(128)[None, None, None, :]
        key = chunk_starts[c] + kk                    # rel key idx
        tloc = (keybase + key) // 32
        kj = (keybase + key) % 32
        rloc_q = 4 * group + a
        qi = img_row(rloc_q)
        ki = img_row(tloc)
        sh = np.clip(qi - 3, 0, H - KS)
        sw = np.clip(qj - 3, 0, H - KS)
        valid = (ki >= sh) & (ki < sh + KS) & (kj >= sw) & (kj < sw + KS)
        if nch > 1:
            dedup = ~((c > 0) & (key < chunk_starts[np.maximum(c - 1, 0)] + 128))
            valid = valid & dedup
        bh = np.clip(ki - qi + KS - 1, 0, 2 * KS - 2)
        bw = np.clip(kj - qj + KS - 1, 0, 2 * KS - 2)
        # [8, 4, 32, nch, 128]
        bias = rpb[:, bh, bw]
        E = np.where(valid[None], np.exp(bias), 0.0)
        return np.ascontiguousarray(
            E.reshape(HEADS, 4 * 32, nch * 128).astype(np.float32))
    return make(0), make(1)


def _prepare_inputs(inp):
    F = _fold_weights(inp)
    E_e_t, E_s_t = _build_E(F['rpb'], flip=False)
    E_e_b, E_s_b = _build_E(F['rpb'], flip=True)
    x = np.asarray(inp['x'], np.float32)
    Bsz = x.shape[0]
    shared = {k: v for k, v in F.items() if k != 'rpb'}
    in_maps = []
    for b in range(Bsz):
        for half in range(2):
            if half == 0:
                x_loc = x[b, 0:19].reshape(NT, DIM)
                Ee, Es = E_e_t, E_s_t
            else:
                x_loc = x[b, 31:12:-1].reshape(NT, DIM)
                Ee, Es = E_e_b, E_s_b
            m = dict(shared)
            m['x'] = _f32(x_loc)
            # device layout: [key-within-chunk(128), head, chunk*128 + q]
            def dev(E):
                nch = E.shape[2] // 128
                return np.ascontiguousarray(
                    E.reshape(HEADS, 128, nch, 128)
                    .transpose(3, 0, 2, 1)
                    .reshape(128, HEADS * nch * 128))
            m['Eall'] = _bf(np.concatenate([dev(Ee), dev(Es)], axis=1))
            in_maps.append(m)
    return in_maps


# --------------------------------------------------------------------------
# Bass kernel graph
# --------------------------------------------------------------------------

def build_graph():
    import concourse.bass as bass
    import concourse.tile as tile
    import concourse.mybir as mybir
    from concourse import bacc
    from concourse.masks import make_identity

    dt = mybir.dt
    Alu = mybir.AluOpType
    Act = mybir.ActivationFunctionType

    nc = bacc.Bacc()

    def param(name, shape, dtype, out=False):
        return nc.declare_dram_parameter(name, list(shape), dtype, isOutput=out)

    x_d = param("x", (NT, DIM), dt.float32)
    wqk_d = param("wqk", (128, 2, 512), dt.bfloat16)
    wv_d = param("wv", (128, 2, 388), dt.bfloat16)
    f1w_d = param("f1w", (128, 2, HID), dt.bfloat16)
    f2w_d = param("f2w", (128, 8, DIM), dt.bfloat16)
    pwT_d = param("pwT", (97, 4, DIM), dt.bfloat16)
    smallw_d = param("smallw", (1, 1094), dt.bfloat16)
    cbias_d = param("cbias", (128, 12), dt.float32)
    Eall_d = param("Eall", (128, HEADS * (256 + 384)), dt.bfloat16)
    out_d = param("out", (NQ, DIM), dt.float32, out=True)

    with tile.TileContext(nc) as tc:
        with (
            tc.tile_pool(name="consts", bufs=1) as consts,
            tc.tile_pool(name="persist", bufs=1) as persist,
            tc.tile_pool(name="work", bufs=3) as work,
            tc.tile_pool(name="aq", bufs=3) as aqpool,
            tc.tile_pool(name="ps", bufs=6, space="PSUM") as ps,
            tc.tile_pool(name="psav", bufs=2, space="PSUM") as psav,
        ):
            # ---------------- input DMAs ---------------------------------
            # All issued from sync in need-order: transfers appear to drain
            # a shared DMA device serially, so order = landing order.
            x_tiles = []
            for t, (off, nt) in enumerate(TOK_TILES):
                xt = persist.tile([128, DIM], dt.float32, tag=f"x{t}", name=f"x{t}")
                nc.sync.dma_start(out=xt[:nt], in_=x_d[off:off + nt, :])
                x_tiles.append(xt)
            cbias_sb = consts.tile([128, 12], dt.float32, tag="cbias")
            nc.sync.dma_start(out=cbias_sb, in_=cbias_d[:])
            wqk_sb = consts.tile([128, 2, 512], dt.bfloat16, tag="wqk")
            nc.sync.dma_start(out=wqk_sb, in_=wqk_d[:])
            smallw_sb = consts.tile([1, 1094], dt.bfloat16, tag="smallw")
            nc.sync.dma_start(out=smallw_sb, in_=smallw_d[:])
            wv_sb = consts.tile([128, 2, 388], dt.bfloat16, tag="wv")
            nc.sync.dma_start(out=wv_sb, in_=wv_d[:])
            Eall_sb = consts.tile([128, HEADS * 640], dt.bfloat16, tag="Eall")
            nc.sync.dma_start(out=Eall_sb, in_=Eall_d[:])
            pwT_sb = consts.tile([97, 4, DIM], dt.bfloat16, tag="pwT")
            nc.sync.dma_start(out=pwT_sb, in_=pwT_d[:])
            f1w_sb = consts.tile([128, 2, HID], dt.bfloat16, tag="f1w")
            nc.sync.dma_start(out=f1w_sb, in_=f1w_d[:])
            f2w_sb = consts.tile([128, 8, DIM], dt.bfloat16, tag="f2w")
            nc.sync.dma_start(out=f2w_sb, in_=f2w_d[:])

            psel_a = smallw_sb[0:1, 0:97]
            psel_b = smallw_sb[0:1, 97:194]
            vb_sb = smallw_sb[0:1, 194:582]
            pb_sb = smallw_sb[0:1, 582:838]
            f2b_sb = smallw_sb[0:1, 838:1094]
            qkb_sb = cbias_sb[:, 0:4]
            f1b_sb = cbias_sb[:, 4:12]

            ident = consts.tile([128, 128], dt.bfloat16, tag="ident")
            make_identity(nc, ident)
            Ee_sb = Eall_sb[:, 0:HEADS * 256].rearrange("p (h c) -> p h c", h=HEADS)
            Es_sb = Eall_sb[:, HEADS * 256:].rearrange("p (h c) -> p h c", h=HEADS)

            ones_sb = consts.tile([1, NT], dt.bfloat16, tag="ones")
            nc.vector.memset(ones_sb, 1.0)
            # preload the ACT table set early (Identity lives in every set,
            # but the very first ACTIVATE pays the table load — hide it here)
            idummy = work.tile([1, 2], dt.float32, tag="idummy")
            nc.scalar.activation(out=idummy, in_=ones_sb[0:1, 0:2], func=Act.Exp)

            # ---------------- PE keepalive (p-state ramp) -----------------
            def keepalive(n, nk=128):
                ka = ps.tile([128, 512], dt.float32, tag="pss", name="ka")
                for _ in range(n):
                    nc.tensor.matmul(ka[:, :nk], ident, ident[:, :nk],
                                     start=True, stop=True)

            keepalive(34)

            # ---------------- helpers ----------------
            def dve_rsqrt(dst, src, n):
                """dst[:,0:n] = 1/sqrt(src[:,0:n] + EPS); small-n f32 tiles."""
                ve = work.tile([128, n], dt.float32, tag="rsq_ve", bufs=2)
                nc.vector.tensor_scalar(out=ve, in0=src, scalar1=float(EPS),
                                        scalar2=None, op0=Alu.add)
                yi = work.tile([128, n], dt.int32, tag="rsq_yi", bufs=2)
                nc.vector.tensor_scalar(out=yi, in0=ve[:].bitcast(dt.int32),
                                        scalar1=1, scalar2=None,
                                        op0=Alu.logical_shift_right)
                nc.vector.tensor_scalar(out=yi, in0=yi, scalar1=-1,
                                        scalar2=MAGIC, op0=Alu.mult, op1=Alu.add)
                y = yi[:].bitcast(dt.float32)
                t = work.tile([128, n], dt.float32, tag="rsq_t", bufs=2)
                for _ in range(1):
                    nc.vector.tensor_tensor(out=t, in0=y, in1=y, op=Alu.mult)
                    nc.vector.tensor_tensor(out=t, in0=t, in1=ve, op=Alu.mult)
                    nc.vector.tensor_scalar(out=t, in0=t, scalar1=-0.5,
                                            scalar2=1.5, op0=Alu.mult, op1=Alu.add)
                    nc.vector.tensor_tensor(out=y, in0=y, in1=t, op=Alu.mult)
                nc.vector.tensor_copy(out=dst, in_=y)

            def ln_batch(x_list, sizes, tagp):
                """LayerNorm a batch of tiles. Vector: stats + 1/(v+eps);
                Scalar: sqrt -> rstd; Vector: -m*rstd;
                Scalar: per-tile Identity ACT (x*r - m*r) -> bf16 xhat.
                Returns list of (xhat_tile, nt)."""
                ntile = len(x_list)
                mv = work.tile([128, 2 * ntile], dt.float32, tag=tagp + "_mv", bufs=2)
                nc.vector.memset(mv, 1.0)
                for t, (xt, nt) in enumerate(zip(x_list, sizes)):
                    stats = work.tile([128, 6], dt.float32, tag=tagp + "_st", bufs=2)
                    nc.vector.bn_stats(out=stats[:nt], in_=xt[:nt])
                    nc.vector.bn_aggr(out=mv[:nt, 2 * t:2 * t + 2], in_=stats[:nt])
                rstd = work.tile([128, ntile], dt.float32, tag=tagp + "_rs", bufs=2)
                dve_rsqrt(rstd, mv[:, 1::2], ntile)
                negmr = work.tile([128, ntile], dt.float32, tag=tagp + "_nm", bufs=2)
                nc.vector.scalar_tensor_tensor(out=negmr, in0=mv[:, 0::2],
                                               scalar=-1.0, in1=rstd,
                                               op0=Alu.mult, op1=Alu.mult)
                outs = []
                for t, (xt, nt) in enumerate(zip(x_list, sizes)):
                    xh = persist.tile([128, DIM], dt.bfloat16,
                                      tag=f"{tagp}_xh{t}", name=f"{tagp}_xh{t}")
                    nc.scalar.activation(out=xh[:nt], in_=xt[:nt],
                                         func=Act.Identity,
                                         scale=rstd[:nt, t:t + 1],
                                         bias=negmr[:nt, t:t + 1])
                    outs.append((xh, nt))
                return outs

            def make_tcat(tagp, total):
                return [persist.tile([128, total], dt.bfloat16,
                                     tag=f"{tagp}_{cb}", name=f"{tagp}_{cb}")
                        for cb in range(2)]

            def transpose_into(res, xh_tiles, off, evac_engines, ei0=0):
                """Transpose [nt, 256] bf16 tiles into res c-block tiles at
                column offset off. Returns next offset."""
                ei = ei0
                for xh, nt in xh_tiles:
                    for cb in range(2):
                        ptb = ps.tile([128, 128], dt.bfloat16, tag="pss", name="ptb")
                        nc.tensor.transpose(ptb[:, :nt], xh[:nt, 128 * cb:128 * (cb + 1)],
                                            ident[:nt, :nt])
                        eng = evac_engines[ei % len(evac_engines)]
                        ei += 1
                        eng.tensor_copy(out=res[cb][:, off:off + nt], in_=ptb[:, :nt])
                    off += nt
                return off

            # ---------------- LN1 (two batches) + transposes --------------
            xhatT = make_tcat("xhatT", NT)
            b1 = ln_batch(x_tiles[:3], [128, 128, 128], "ln1a")
            b2 = ln_batch(x_tiles[3:], [128, 96], "ln1b")
            transpose_into(xhatT, b1 + b2, 0, [nc.vector])

            # ---------------- QKV ----------------
            # Q (mt 0,1), K (mt 2,3): psum accum over 2 k-blocks,
            # bias via Identity-ACT evacuation.
            qT, kT = [], []
            for mt in range(4):
                ncols = NQ if mt < 2 else NT
                dst = persist.tile([128, ncols], dt.bfloat16, tag=f"qk{mt}")
                for n0 in range(0, ncols, 512):
                    nn = min(512, ncols - n0)
                    pt = ps.tile([128, 512], dt.float32, tag="pss", name="pqk")
                    for kb in range(2):
                        nc.tensor.matmul(
                            pt[:, :nn], wqk_sb[:, kb, 128 * mt:128 * (mt + 1)],
                            xhatT[kb][:, n0:n0 + nn],
                            start=(kb == 0), stop=(kb == 1))
                    # Q evacs on Scalar, K evacs on Vector (both Identity+bias
                    # capable? DVE copy can't add bias -> K bias via Scalar too)
                    nc.scalar.activation(out=dst[:, n0:n0 + nn], in_=pt[:, :nn],
                                         func=Act.Identity,
                                         bias=qkb_sb[:, mt:mt + 1])
                (qT if mt < 2 else kT).append(dst)

            # V key-windows computed lazily: scheduled into the attention
            # pipeline (see VW_SCHED) so early units start sooner.
            Vw = [None] * len(WINDOW_STARTS)

            def make_vwindow(wi):
                ws = WINDOW_STARTS[wi]
                vt = persist.tile([128, 388], dt.bfloat16, tag=f"vw{wi}", name=f"vw{wi}")
                pt = ps.tile([128, 512], dt.float32, tag="pss", name=f"pv{wi}")
                pv = pt[:, :388]
                for kb in range(2):
                    nc.tensor.matmul(pv, xhatT[kb][:, ws:ws + 128], wv_sb[:, kb, :],
                                     start=(kb == 0), stop=False)
                nc.tensor.matmul(pv, ones_sb[:, :128], vb_sb, start=False, stop=True)
                if wi % 2 == 0:
                    nc.vector.tensor_copy(out=vt, in_=pv)
                else:
                    nc.scalar.activation(out=vt, in_=pv, func=Act.Copy)
                Vw[wi] = vt

            # ---------------- attention: 8 (group, quad) units ------------
            # attnP2[quad]: [97, pair-local(2), NQ] — normalized attention^T
            attnP2 = [persist.tile([97, 2, NQ], dt.bfloat16, tag=f"attnP{q}",
                                   name=f"attnP{q}") for q in range(2)]

            UNITS = [(g, q) for g in range(4) for q in range(2)]

            def unit_S(u):
                """S^T matmuls + exp for unit u. Returns aq tile."""
                g, quad = UNITS[u]
                kb_tok, css = GROUPS[g]
                nch = len(css)
                aq = aqpool.tile([128, 4, 384], dt.bfloat16, tag="aq",
                                 name=f"aq_{u}")
                for slot in range(4):
                    pS = ps.tile([128, 512], dt.float32, tag="pss",
                                 name=f"pS_{u}_{slot}")
                    for c, cs in enumerate(css):
                        nc.tensor.matmul(
                            pS[:, 128 * c:128 * (c + 1)],
                            kT[quad][32 * slot:32 * slot + 32,
                                     kb_tok + cs:kb_tok + cs + 128],
                            qT[quad][32 * slot:32 * slot + 32,
                                     128 * g:128 * (g + 1)],
                            start=True, stop=True,
                            tile_position=(32 * slot, 0))
                    nc.scalar.activation(
                        out=aq[:, slot, :128 * nch],
                        in_=pS[:, :128 * nch], func=Act.Exp)
                return aq

            def unit_emult(u, aq):
                """aq *= E for unit u (pair 0 on GpSimd, pair 1 on Vector)."""
                g, quad = UNITS[u]
                nch = len(GROUPS[g][1])
                E_sb = Ee_sb if g == 0 else Es_sb
                for pl, eng in ((0, nc.gpsimd), (1, nc.vector)):
                    eng.tensor_tensor(
                        out=aq[:, 2 * pl:2 * pl + 2, :128 * nch],
                        in0=aq[:, 2 * pl:2 * pl + 2, :128 * nch],
                        in1=E_sb[:, 4 * quad + 2 * pl:4 * quad + 2 * pl + 2,
                                 :128 * nch],
                        op=Alu.mult)

            def unit_AV(u, aq):
                """AV matmuls for unit u — whole unit in ONE psum bank:
                pair0 numerators cols 0:128, pair1 cols 128:256,
                broadcasts go to 256:384 / 384:512 later.
                Returns (pav, rcd)."""
                g, quad = UNITS[u]
                kb_tok, css = GROUPS[g]
                nch = len(css)
                pav = psav.tile([128, 512], dt.float32, tag="psav",
                                name=f"pav_{u}")
                for pl in range(2):
                    p = 2 * quad + pl
                    pnum = pav[:97, 128 * pl:128 * (pl + 1)]
                    for hh_loc, po, mm in ((2 * pl, 0, 64), (2 * pl + 1, 64, 33)):
                        voff = 97 * p + (0 if po == 0 else 64)
                        for c, cs in enumerate(css):
                            vt = Vw[WIN_IDX[kb_tok + cs]]
                            nc.tensor.matmul(
                                pnum[po:po + mm, :],
                                vt[:, voff:voff + mm],
                                aq[:, hh_loc, 128 * c:128 * (c + 1)],
                                start=(c == 0), stop=(c == nch - 1),
                                tile_position=(0, po))
                # Vector: den gather (both pairs at once), one reciprocal.
                # den layout: [a0 | a1 | b0 | b1] blocks of 128
                den = work.tile([1, 512], dt.float32, tag="den", bufs=3)
                nc.vector.tensor_copy(out=den[:, 0:256], in_=pav[32:33, 0:256])
                nc.vector.tensor_copy(out=den[:, 256:512], in_=pav[96:97, 0:256])
                rcd = work.tile([1, 512], dt.float32, tag="rcd", bufs=3)
                nc.vector.reciprocal_approx_fast(out=rcd, in_=den)
                rc16 = work.tile([1, 512], dt.bfloat16, tag="rc16", bufs=3)
                nc.vector.tensor_copy(out=rc16, in_=rcd)
                return (pav, rc16)

            def unit_pB(avout, u):
                """PE broadcast of reciprocals (f32 matmuls), one evacuation,
                one Vector normalize into attnP2."""
                g, quad = UNITS[u]
                pav, rc16 = avout
                for pl in range(2):
                    pB = pav[:97, 256 + 128 * pl:384 + 128 * pl]
                    nc.tensor.matmul(pB, psel_a,
                                     rc16[:, 128 * pl:128 * (pl + 1)],
                                     start=True, stop=False)
                    nc.tensor.matmul(pB, psel_b,
                                     rc16[:, 256 + 128 * pl:384 + 128 * pl],
                                     start=False, stop=True)
                rcb = work.tile([97, 256], dt.bfloat16, tag="rcb", bufs=3)
                nc.vector.tensor_copy(out=rcb, in_=pav[:97, 256:512])
                nc.vector.tensor_tensor(
                    out=attnP2[quad][:, :, 128 * g:128 * (g + 1)],
                    in0=pav[:97, 0:256].rearrange("p (two q) -> p two q", two=2),
                    in1=rcb.rearrange("p (two q) -> p two q", two=2),
                    op=Alu.mult)

            # software pipeline: PE order  Vw | S(u) | pB(u-2) | AV(u-1)
            # V windows land just before the units that need them, filling
            # the early-pipeline PE bubbles.
            VW_SCHED = {0: [0, 1], 1: [2, 3], 2: [4, 5], 3: [6, 7], 4: [8]}
            aqs = [None] * 8
            avouts = [None] * 8
            for u in range(8):
                for wi in VW_SCHED.get(u, []):
                    make_vwindow(wi)
                aqs[u] = unit_S(u)
                if u >= 2:
                    unit_pB(avouts[u - 2], u - 2)
                unit_emult(u, aqs[u])
                if u >= 1:
                    avouts[u - 1] = unit_AV(u - 1, aqs[u - 1])
            avouts[7] = unit_AV(7, aqs[7])
            unit_pB(avouts[6], 6)
            unit_pB(avouts[7], 7)

            # ---------------- proj + residual ----------------
            keepalive(8)
            y_tiles = []
            for mt in range(4):
                pt = ps.tile([128, 512], dt.float32, tag="pss", name=f"pproj{mt}")
                pp = pt[:, :DIM]
                for p in range(4):
                    nc.tensor.matmul(pp,
                                     attnP2[p // 2][:, p % 2, 128 * mt:128 * (mt + 1)],
                                     pwT_sb[:, p, :], start=(p == 0), stop=False)
                nc.tensor.matmul(pp, ones_sb[:, :128], pb_sb, start=False, stop=True)
                yt = persist.tile([128, DIM], dt.float32, tag=f"y{mt}")
                nc.vector.tensor_tensor(out=yt, in0=pp, in1=x_tiles[mt][:],
                                        op=Alu.add)
                y_tiles.append(yt)

            # gelu table preload (dummy) while LN2 runs
            gdummy = work.tile([1, 2], dt.float32, tag="gdummy")
            nc.scalar.activation(out=gdummy, in_=ones_sb[0:1, 0:2], func=Act.Gelu)

            # ---------------- LN2 + MLP ----------------
            x2T = make_tcat("x2T", NQ)
            xh2a = ln_batch(y_tiles[:2], [128, 128], "ln2a")
            xh2b = ln_batch(y_tiles[2:], [128, 128], "ln2b")
            keepalive(16)
            transpose_into(x2T, xh2a + xh2b, 0, [nc.vector])

            keepalive(6)
            m1 = []
            for mh in range(8):
                pt = ps.tile([128, 512], dt.float32, tag="pss", name=f"pfc1{mh}")
                for kb in range(2):
                    nc.tensor.matmul(pt, f1w_sb[:, kb, 128 * mh:128 * (mh + 1)],
                                     x2T[kb], start=(kb == 0), stop=(kb == 1))
                mg = persist.tile([128, NQ], dt.bfloat16, tag=f"m1_{mh}")
                nc.scalar.activation(out=mg, in_=pt, func=Act.Gelu,
                                     bias=f1b_sb[:, mh:mh + 1], scale=1.0)
                m1.append(mg)

            # fc2 mt-major: finish each output tile early, DMA out alternating
            for mt in range(4):
                pt = ps.tile([128, 512], dt.float32, tag="pss", name=f"pfc2{mt}")
                pp = pt[:, :DIM]
                for kb in range(8):
                    nc.tensor.matmul(pp, m1[kb][:, 128 * mt:128 * (mt + 1)],
                                     f2w_sb[:, kb, :], start=(kb == 0), stop=False)
                nc.tensor.matmul(pp, ones_sb[:, :128], f2b_sb, start=False, stop=True)
                ot = work.tile([128, DIM], dt.float32, tag="outt", bufs=2)
                nc.vector.tensor_tensor(out=ot, in0=pp, in1=y_tiles[mt][:],
                                        op=Alu.add)
                eng = nc.sync if mt % 2 == 0 else nc.gpsimd
                eng.dma_start(out=out_d[128 * mt:128 * (mt + 1), :], in_=ot)

    nc.finalize()
    return nc


# --------------------------------------------------------------------------
# Entry point
# --------------------------------------------------------------------------

def kernel(**inputs):
    from concourse.bass_utils import run_bass_kernel_spmd

    if 'nc' not in _CACHE:
        _CACHE['nc'] = build_graph()
    nc = _CACHE['nc']

    in_maps = _prepare_inputs(inputs)
    res = run_bass_kernel_spmd(nc, in_maps, core_ids=list(range(8)))
    x = np.asarray(inputs['x'])
    Bsz, Hh, Ww, C = x.shape
    out = np.zeros((Bsz, Hh, Ww, C), np.float32)
    for i in range(2 * Bsz):
        b, half = divmod(i, 2)
        o = np.asarray(res.results[i]['out']).reshape(16, Ww, C)
        if half == 0:
            out[b, 0:16] = o
        else:
            out[b, 16:32] = o[::-1]
    return out.astype(x.dtype)



# revision 19
# speedup vs baseline: 1.3921x; 1.0182x over previous
"""Trainium2 Bass kernel for AdaNSABlock (7x7 neighborhood attention block).

Sharding: 8 cores = batch(4) x row-halves(2). Each core computes 16 image
rows (512 tokens) of one sample, reading 19 rows (3-row halo) of input.
Bottom halves are row-flipped on host so all cores run one SPMD graph.

v1 rewrite vs baseline:
  - input DMAs spread across sync/scalar/gpsimd queues, QKV weights first
  - LN normalize via ACT Identity (scale=rstd, bias=-mean*rstd)
  - Q/K bias folded into ACT Identity evacuation (no bias matmuls)
  - V key-windows computed directly from xhatT column slices (no SBUF DMAs)
  - attention software-pipelined at (group, quad) granularity:
      PE stream:  S(u) | pB(u-2) | AV(u-1); PSUM 4 S-banks + 4 AV-banks
      Scalar: exp;  Vector: E-mult + den/recip;  GpSimd: pB evac + normalize
  - proj on 128-row packed attnT (K=128), gelu table preloaded off-path
  - keepalive matmuls bridge PE gaps to hold the p-state ramp
"""

import numpy as np
import ml_dtypes

KS = 7
HEADS = 8
DIM = 256
HID = 1024
HD = 32
H = 32
W = 32
NT = 19 * 32          # local tokens incl halo
NQ = 512              # query tokens per core
EPS = 1e-5
BF16 = ml_dtypes.bfloat16

# token tiles covering NT
TOK_TILES = [(0, 128), (128, 128), (256, 128), (384, 128), (512, 96)]
# groups: (key_base_token, chunk_starts)
GROUPS = [(0, (0, 96)), (32, (0, 128, 192)), (160, (0, 128, 192)), (288, (0, 128, 192))]
# distinct V key-windows (start tokens, each 128 wide), in first-use order
WINDOW_STARTS = [0, 96, 32, 160, 224, 288, 352, 416, 480]
WIN_IDX = {s: i for i, s in enumerate(WINDOW_STARTS)}

MAGIC = 0x5F3759DF

_CACHE = {}


def _bf(x):
    return np.ascontiguousarray(np.asarray(x, np.float32).astype(BF16))


def _f32(x):
    return np.ascontiguousarray(np.asarray(x, np.float32))


# --------------------------------------------------------------------------
# Host-side folding + mask construction
# --------------------------------------------------------------------------

def _fold_weights(inp):
    quality = inp['quality']
    s = int(quality) - 1
    l = float(quality % 1)
    g1 = np.abs(np.asarray(inp['gamma_1'], np.float64))
    g2 = np.abs(np.asarray(inp['gamma_2'], np.float64))
    if s == g1.shape[0] - 1:
        G1, G2 = g1[s], g2[s]
    else:
        G1 = g1[s] ** (1 - l) * g1[s + 1] ** l
        G2 = g2[s] ** (1 - l) * g2[s + 1] ** l

    qkv_w = np.asarray(inp['qkv_w'], np.float64)
    qkv_b = np.asarray(inp['qkv_b'], np.float64)
    n1w = np.asarray(inp['norm1_w'], np.float64)
    n1b = np.asarray(inp['norm1_b'], np.float64)
    Wq = qkv_w * n1w[None, :]
    bq = qkv_b + qkv_w @ n1b
    sc = HD ** -0.5
    Wq[:DIM] *= sc
    bq[:DIM] *= sc

    pw = G1[:, None] * np.asarray(inp['proj_w'], np.float64)
    pb = G1 * np.asarray(inp['proj_b'], np.float64)

    n2w = np.asarray(inp['norm2_w'], np.float64)
    n2b = np.asarray(inp['norm2_b'], np.float64)
    f1w = np.asarray(inp['fc1_w'], np.float64) * n2w[None, :]
    f1b = np.asarray(inp['fc1_b'], np.float64) + np.asarray(inp['fc1_w'], np.float64) @ n2b
    f2w = G2[:, None] * np.asarray(inp['fc2_w'], np.float64)
    f2b = G2 * np.asarray(inp['fc2_b'], np.float64)

    Wv = Wq[2 * DIM:]            # [256 vdims, 256 c]
    bv = bq[2 * DIM:]
    # ones-augmented V: pair p = heads (2p, 2p+1), V'' cols 97p..97p+96:
    #   col 0..31  = dims of head 2p      (psum rows 0..31)
    #   col 32     = one_a                (psum row 32 = denom_a)
    #   col 64..95 = dims of head 2p+1    (psum rows 64..95)
    #   col 96     = one_b                (psum row 96 = denom_b)
    # head-a AV matmul: M=64 cols [0:64); head-b: M=33 cols [64:97)
    # (partition offsets 0/32/64/96 keep every slice quarter-aligned)
    Wv_aug = np.zeros((DIM, 388))
    vb_aug = np.zeros(388)
    for p in range(4):
        ha, hb = 2 * p, 2 * p + 1
        base = 97 * p
        Wv_aug[:, base:base + 32] = Wv[32 * ha:32 * ha + 32].T
        vb_aug[base:base + 32] = bv[32 * ha:32 * ha + 32]
        vb_aug[base + 32] = 1.0
        Wv_aug[:, base + 64:base + 96] = Wv[32 * hb:32 * hb + 32].T
        vb_aug[base + 64:base + 96] = bv[32 * hb:32 * hb + 32]
        vb_aug[base + 96] = 1.0

    def kblocked(wT, kb):
        # [kb*128, N] -> [128, kb, N] (partition-major, contiguous for DMA)
        n = wT.shape[1]
        return np.ascontiguousarray(wT.reshape(kb, 128, n).transpose(1, 0, 2))

    # pairsel for recip broadcast: two K=1 matmuls per pair, lhsT [1, 97]
    # psel_a ones at rows 0..63 (head-a block), psel_b ones at rows 64..96
    # smallw bf16 [1, 1094]: pairsel(194) | vb | pb | f2b
    smallw = np.zeros((1, 1094))
    smallw[0, 0:64] = 1.0
    smallw[0, 97 + 64:97 + 97] = 1.0
    smallw[0, 194:582] = vb_aug
    smallw[0, 582:838] = pb
    smallw[0, 838:1094] = f2b

    # cbias f32 [128, 12]: qkb (q0,q1,k0,k1) | f1b (8 cols)
    cbias = np.zeros((128, 12))
    for mt in range(4):
        cbias[:, mt] = bq[128 * mt:128 * mt + 128]
    for mh in range(8):
        cbias[:, 4 + mh] = f1b[128 * mh:128 * mh + 128]

    # proj lhsT comes from attnT pair tiles [97, 128]: pair p rows 0-31 =
    # head 2p (+den rows 32/96 junk -> zero weight), rows 64-95 = head 2p+1.
    pwT_aug = np.zeros((4, 97, DIM))
    for p in range(4):
        ha, hb = 2 * p, 2 * p + 1
        pwT_aug[p, 0:32] = pw[:, 32 * ha:32 * ha + 32].T
        pwT_aug[p, 64:96] = pw[:, 32 * hb:32 * hb + 32].T

    return dict(
        wqk=_bf(kblocked(Wq[:512].T, 2)),       # [128, 2, 512]
        wv=_bf(kblocked(Wv_aug, 2)),            # [128, 2, 388]
        f1w=_bf(kblocked(f1w.T, 2)),            # [128, 2, 1024]
        f2w=_bf(kblocked(f2w.T, 8)),            # [128, 8, 256]
        pwT=_bf(np.ascontiguousarray(pwT_aug.transpose(1, 0, 2))),  # [97, 4, 256]
        smallw=_bf(smallw),                     # [1, 1094]
        cbias=_f32(cbias),                      # [128, 12]
        rpb=np.asarray(inp['rpb'], np.float64),
    )


def _build_E(rpb, flip):
    """Vectorized E (exp of bias, masked/dedup-zeroed).
    Returns E_edge [8,128,256], E_std [8,128,384] float32."""
    def img_row(r):
        return (31 - r) if flip else r

    def make(group):
        if group == 0:
            keybase, chunk_starts = 0, np.array([0, 96])
        else:
            keybase, chunk_starts = (4 * group - 3) * 32, np.array([0, 128, 192])
        nch = len(chunk_starts)
        a = np.arange(4)[:, None, None, None]         # q row in group
        qj = np.arange(32)[None, :, None, None]
        c = np.arange(nch)[None, None, :, None]
        kk = np.arange(128)[None, None, None, :]
        key = chunk_starts[c] + kk                    # rel key idx
        tloc = (keybase + key) // 32
        kj = (keybase + key) % 32
        rloc_q = 4 * group + a
        qi = img_row(rloc_q)
        ki = img_row(tloc)
        sh = np.clip(qi - 3, 0, H - KS)
        sw = np.clip(qj - 3, 0, H - KS)
        valid = (ki >= sh) & (ki < sh + KS) & (kj >= sw) & (kj < sw + KS)
        if nch > 1:
            dedup = ~((c > 0) & (key < chunk_starts[np.maximum(c - 1, 0)] + 128))
            valid = valid & dedup
        bh = np.clip(ki - qi + KS - 1, 0, 2 * KS - 2)
        bw = np.clip(kj - qj + KS - 1, 0, 2 * KS - 2)
        # [8, 4, 32, nch, 128]
        bias = rpb[:, bh, bw]
        E = np.where(valid[None], np.exp(bias), 0.0)
        return np.ascontiguousarray(
            E.reshape(HEADS, 4 * 32, nch * 128).astype(np.float32))
    return make(0), make(1)


def _prepare_inputs(inp):
    F = _fold_weights(inp)
    E_e_t, E_s_t = _build_E(F['rpb'], flip=False)
    E_e_b, E_s_b = _build_E(F['rpb'], flip=True)
    x = np.asarray(inp['x'], np.float32)
    Bsz = x.shape[0]
    shared = {k: v for k, v in F.items() if k != 'rpb'}
    in_maps = []
    for b in range(Bsz):
        for half in range(2):
            if half == 0:
                x_loc = x[b, 0:19].reshape(NT, DIM)
                Ee, Es = E_e_t, E_s_t
            else:
                x_loc = x[b, 31:12:-1].reshape(NT, DIM)
                Ee, Es = E_e_b, E_s_b
            m = dict(shared)
            m['x'] = _f32(x_loc)
            # device layout: [key-within-chunk(128), head, chunk*128 + q]
            def dev(E):
                nch = E.shape[2] // 128
                return np.ascontiguousarray(
                    E.reshape(HEADS, 128, nch, 128)
                    .transpose(3, 0, 2, 1)
                    .reshape(128, HEADS * nch * 128))
            m['Eall'] = _bf(np.concatenate([dev(Ee), dev(Es)], axis=1))
            in_maps.append(m)
    return in_maps


# --------------------------------------------------------------------------
# Bass kernel graph
# --------------------------------------------------------------------------

def build_graph():
    import concourse.bass as bass
    import concourse.tile as tile
    import concourse.mybir as mybir
    from concourse import bacc
    from concourse.masks import make_identity

    dt = mybir.dt
    Alu = mybir.AluOpType
    Act = mybir.ActivationFunctionType

    nc = bacc.Bacc()

    def param(name, shape, dtype, out=False):
        return nc.declare_dram_parameter(name, list(shape), dtype, isOutput=out)

    x_d = param("x", (NT, DIM), dt.float32)
    wqk_d = param("wqk", (128, 2, 512), dt.bfloat16)
    wv_d = param("wv", (128, 2, 388), dt.bfloat16)
    f1w_d = param("f1w", (128, 2, HID), dt.bfloat16)
    f2w_d = param("f2w", (128, 8, DIM), dt.bfloat16)
    pwT_d = param("pwT", (97, 4, DIM), dt.bfloat16)
    smallw_d = param("smallw", (1, 1094), dt.bfloat16)
    cbias_d = param("cbias", (128, 12), dt.float32)
    Eall_d = param("Eall", (128, HEADS * (256 + 384)), dt.bfloat16)
    out_d = param("out", (NQ, DIM), dt.float32, out=True)

    with tile.TileContext(nc) as tc:
        with (
            tc.tile_pool(name="consts", bufs=1) as consts,
            tc.tile_pool(name="persist", bufs=1) as persist,
            tc.tile_pool(name="work", bufs=3) as work,
            tc.tile_pool(name="aq", bufs=3) as aqpool,
            tc.tile_pool(name="ps", bufs=6, space="PSUM") as ps,
            tc.tile_pool(name="psav", bufs=2, space="PSUM") as psav,
        ):
            # ---------------- input DMAs ---------------------------------
            # All issued from sync in need-order: transfers appear to drain
            # a shared DMA device serially, so order = landing order.
            x_tiles = []
            for t, (off, nt) in enumerate(TOK_TILES):
                xt = persist.tile([128, DIM], dt.float32, tag=f"x{t}", name=f"x{t}")
                nc.sync.dma_start(out=xt[:nt], in_=x_d[off:off + nt, :])
                x_tiles.append(xt)
            cbias_sb = consts.tile([128, 12], dt.float32, tag="cbias")
            nc.sync.dma_start(out=cbias_sb, in_=cbias_d[:])
            wqk_sb = consts.tile([128, 2, 512], dt.bfloat16, tag="wqk")
            nc.sync.dma_start(out=wqk_sb, in_=wqk_d[:])
            smallw_sb = consts.tile([1, 1094], dt.bfloat16, tag="smallw")
            nc.sync.dma_start(out=smallw_sb, in_=smallw_d[:])
            wv_sb = consts.tile([128, 2, 388], dt.bfloat16, tag="wv")
            nc.sync.dma_start(out=wv_sb, in_=wv_d[:])
            Eall_sb = consts.tile([128, HEADS * 640], dt.bfloat16, tag="Eall")
            nc.sync.dma_start(out=Eall_sb, in_=Eall_d[:])
            pwT_sb = consts.tile([97, 4, DIM], dt.bfloat16, tag="pwT")
            nc.sync.dma_start(out=pwT_sb, in_=pwT_d[:])
            f1w_sb = consts.tile([128, 2, HID], dt.bfloat16, tag="f1w")
            nc.sync.dma_start(out=f1w_sb, in_=f1w_d[:])
            f2w_sb = consts.tile([128, 8, DIM], dt.bfloat16, tag="f2w")
            nc.sync.dma_start(out=f2w_sb, in_=f2w_d[:])

            psel_a = smallw_sb[0:1, 0:97]
            psel_b = smallw_sb[0:1, 97:194]
            vb_sb = smallw_sb[0:1, 194:582]
            pb_sb = smallw_sb[0:1, 582:838]
            f2b_sb = smallw_sb[0:1, 838:1094]
            qkb_sb = cbias_sb[:, 0:4]
            f1b_sb = cbias_sb[:, 4:12]

            ident = consts.tile([128, 128], dt.bfloat16, tag="ident")
            make_identity(nc, ident)
            Ee_sb = Eall_sb[:, 0:HEADS * 256].rearrange("p (h c) -> p h c", h=HEADS)
            Es_sb = Eall_sb[:, HEADS * 256:].rearrange("p (h c) -> p h c", h=HEADS)

            ones_sb = consts.tile([1, NT], dt.bfloat16, tag="ones")
            nc.vector.memset(ones_sb, 1.0)
            # preload the ACT table set early (Identity lives in every set,
            # but the very first ACTIVATE pays the table load — hide it here)
            idummy = work.tile([1, 2], dt.float32, tag="idummy")
            nc.scalar.activation(out=idummy, in_=ones_sb[0:1, 0:2], func=Act.Exp)

            # ---------------- PE keepalive (p-state ramp) -----------------
            def keepalive(n, nk=128):
                ka = ps.tile([128, 512], dt.float32, tag="pss", name="ka")
                for _ in range(n):
                    nc.tensor.matmul(ka[:, :nk], ident, ident[:, :nk],
                                     start=True, stop=True)

            keepalive(28)

            # ---------------- helpers ----------------
            def dve_rsqrt(dst, src, n):
                """dst[:,0:n] = 1/sqrt(src[:,0:n] + EPS); small-n f32 tiles."""
                ve = work.tile([128, n], dt.float32, tag="rsq_ve", bufs=2)
                nc.vector.tensor_scalar(out=ve, in0=src, scalar1=float(EPS),
                                        scalar2=None, op0=Alu.add)
                yi = work.tile([128, n], dt.int32, tag="rsq_yi", bufs=2)
                nc.vector.tensor_scalar(out=yi, in0=ve[:].bitcast(dt.int32),
                                        scalar1=1, scalar2=None,
                                        op0=Alu.logical_shift_right)
                nc.vector.tensor_scalar(out=yi, in0=yi, scalar1=-1,
                                        scalar2=MAGIC, op0=Alu.mult, op1=Alu.add)
                y = yi[:].bitcast(dt.float32)
                t = work.tile([128, n], dt.float32, tag="rsq_t", bufs=2)
                for _ in range(1):
                    nc.vector.tensor_tensor(out=t, in0=y, in1=y, op=Alu.mult)
                    nc.vector.tensor_tensor(out=t, in0=t, in1=ve, op=Alu.mult)
                    nc.vector.tensor_scalar(out=t, in0=t, scalar1=-0.5,
                                            scalar2=1.5, op0=Alu.mult, op1=Alu.add)
                    nc.vector.tensor_tensor(out=y, in0=y, in1=t, op=Alu.mult)
                nc.vector.tensor_copy(out=dst, in_=y)

            def ln_batch(x_list, sizes, tagp):
                """LayerNorm a batch of tiles. Vector: stats + 1/(v+eps);
                Scalar: sqrt -> rstd; Vector: -m*rstd;
                Scalar: per-tile Identity ACT (x*r - m*r) -> bf16 xhat.
                Returns list of (xhat_tile, nt)."""
                ntile = len(x_list)
                mv = work.tile([128, 2 * ntile], dt.float32, tag=tagp + "_mv", bufs=2)
                nc.vector.memset(mv, 1.0)
                for t, (xt, nt) in enumerate(zip(x_list, sizes)):
                    stats = work.tile([128, 6], dt.float32, tag=tagp + "_st", bufs=2)
                    nc.vector.bn_stats(out=stats[:nt], in_=xt[:nt])
                    nc.vector.bn_aggr(out=mv[:nt, 2 * t:2 * t + 2], in_=stats[:nt])
                rstd = work.tile([128, ntile], dt.float32, tag=tagp + "_rs", bufs=2)
                dve_rsqrt(rstd, mv[:, 1::2], ntile)
                negmr = work.tile([128, ntile], dt.float32, tag=tagp + "_nm", bufs=2)
                nc.vector.scalar_tensor_tensor(out=negmr, in0=mv[:, 0::2],
                                               scalar=-1.0, in1=rstd,
                                               op0=Alu.mult, op1=Alu.mult)
                outs = []
                for t, (xt, nt) in enumerate(zip(x_list, sizes)):
                    xh = persist.tile([128, DIM], dt.bfloat16,
                                      tag=f"{tagp}_xh{t}", name=f"{tagp}_xh{t}")
                    nc.scalar.activation(out=xh[:nt], in_=xt[:nt],
                                         func=Act.Identity,
                                         scale=rstd[:nt, t:t + 1],
                                         bias=negmr[:nt, t:t + 1])
                    outs.append((xh, nt))
                return outs

            def make_tcat(tagp, total):
                return [persist.tile([128, total], dt.bfloat16,
                                     tag=f"{tagp}_{cb}", name=f"{tagp}_{cb}")
                        for cb in range(2)]

            def transpose_into(res, xh_tiles, off, evac_engines, ei0=0):
                """Transpose [nt, 256] bf16 tiles into res c-block tiles at
                column offset off. Returns next offset."""
                ei = ei0
                for xh, nt in xh_tiles:
                    for cb in range(2):
                        ptb = ps.tile([128, 128], dt.bfloat16, tag="pss", name="ptb")
                        nc.tensor.transpose(ptb[:, :nt], xh[:nt, 128 * cb:128 * (cb + 1)],
                                            ident[:nt, :nt])
                        eng = evac_engines[ei % len(evac_engines)]
                        ei += 1
                        eng.tensor_copy(out=res[cb][:, off:off + nt], in_=ptb[:, :nt])
                    off += nt
                return off

            # ---------------- LN1 (two batches) + transposes --------------
            xhatT = make_tcat("xhatT", NT)
            b1 = ln_batch(x_tiles[:3], [128, 128, 128], "ln1a")
            b2 = ln_batch(x_tiles[3:], [128, 96], "ln1b")
            transpose_into(xhatT, b1 + b2, 0, [nc.vector])

            # ---------------- QKV ----------------
            # Q (mt 0,1), K (mt 2,3): psum accum over 2 k-blocks,
            # bias via Identity-ACT evacuation.
            qT, kT = [], []
            for mt in range(4):
                ncols = NQ if mt < 2 else NT
                dst = persist.tile([128, ncols], dt.bfloat16, tag=f"qk{mt}")
                for n0 in range(0, ncols, 512):
                    nn = min(512, ncols - n0)
                    pt = ps.tile([128, 512], dt.float32, tag="pss", name="pqk")
                    for kb in range(2):
                        nc.tensor.matmul(
                            pt[:, :nn], wqk_sb[:, kb, 128 * mt:128 * (mt + 1)],
                            xhatT[kb][:, n0:n0 + nn],
                            start=(kb == 0), stop=(kb == 1))
                    # Q evacs on Scalar, K evacs on Vector (both Identity+bias
                    # capable? DVE copy can't add bias -> K bias via Scalar too)
                    nc.scalar.activation(out=dst[:, n0:n0 + nn], in_=pt[:, :nn],
                                         func=Act.Identity,
                                         bias=qkb_sb[:, mt:mt + 1])
                (qT if mt < 2 else kT).append(dst)

            # V key-windows computed lazily: scheduled into the attention
            # pipeline (see VW_SCHED) so early units start sooner.
            Vw = [None] * len(WINDOW_STARTS)

            def make_vwindow(wi):
                ws = WINDOW_STARTS[wi]
                vt = persist.tile([128, 388], dt.bfloat16, tag=f"vw{wi}", name=f"vw{wi}")
                pt = ps.tile([128, 512], dt.float32, tag="pss", name=f"pv{wi}")
                pv = pt[:, :388]
                for kb in range(2):
                    nc.tensor.matmul(pv, xhatT[kb][:, ws:ws + 128], wv_sb[:, kb, :],
                                     start=(kb == 0), stop=False)
                nc.tensor.matmul(pv, ones_sb[:, :128], vb_sb, start=False, stop=True)
                if wi % 2 == 0:
                    nc.vector.tensor_copy(out=vt, in_=pv)
                else:
                    nc.scalar.activation(out=vt, in_=pv, func=Act.Copy)
                Vw[wi] = vt

            # ---------------- attention: 8 (group, quad) units ------------
            # attnP2[quad]: [97, pair-local(2), NQ] — normalized attention^T
            attnP2 = [persist.tile([97, 2, NQ], dt.bfloat16, tag=f"attnP{q}",
                                   name=f"attnP{q}") for q in range(2)]

            UNITS = [(g, q) for g in range(4) for q in range(2)]

            def unit_S(u):
                """S^T matmuls + exp for unit u. Returns aq tile."""
                g, quad = UNITS[u]
                kb_tok, css = GROUPS[g]
                nch = len(css)
                aq = aqpool.tile([128, 4, 384], dt.bfloat16, tag="aq",
                                 name=f"aq_{u}")
                for slot in range(4):
                    pS = ps.tile([128, 512], dt.float32, tag="pss",
                                 name=f"pS_{u}_{slot}")
                    for c, cs in enumerate(css):
                        nc.tensor.matmul(
                            pS[:, 128 * c:128 * (c + 1)],
                            kT[quad][32 * slot:32 * slot + 32,
                                     kb_tok + cs:kb_tok + cs + 128],
                            qT[quad][32 * slot:32 * slot + 32,
                                     128 * g:128 * (g + 1)],
                            start=True, stop=True,
                            tile_position=(32 * slot, 0))
                    nc.scalar.activation(
                        out=aq[:, slot, :128 * nch],
                        in_=pS[:, :128 * nch], func=Act.Exp)
                return aq

            def unit_emult(u, aq):
                """aq *= E for unit u (pair 0 on GpSimd, pair 1 on Vector)."""
                g, quad = UNITS[u]
                nch = len(GROUPS[g][1])
                E_sb = Ee_sb if g == 0 else Es_sb
                for pl, eng in ((0, nc.gpsimd), (1, nc.vector)):
                    eng.tensor_tensor(
                        out=aq[:, 2 * pl:2 * pl + 2, :128 * nch],
                        in0=aq[:, 2 * pl:2 * pl + 2, :128 * nch],
                        in1=E_sb[:, 4 * quad + 2 * pl:4 * quad + 2 * pl + 2,
                                 :128 * nch],
                        op=Alu.mult)

            def unit_AV(u, aq):
                """AV matmuls for unit u — whole unit in ONE psum bank:
                pair0 numerators cols 0:128, pair1 cols 128:256,
                broadcasts go to 256:384 / 384:512 later.
                Returns (pav, rcd)."""
                g, quad = UNITS[u]
                kb_tok, css = GROUPS[g]
                nch = len(css)
                pav = psav.tile([128, 512], dt.float32, tag="psav",
                                name=f"pav_{u}")
                for pl in range(2):
                    p = 2 * quad + pl
                    pnum = pav[:97, 128 * pl:128 * (pl + 1)]
                    for hh_loc, po, mm in ((2 * pl, 0, 64), (2 * pl + 1, 64, 33)):
                        voff = 97 * p + (0 if po == 0 else 64)
                        for c, cs in enumerate(css):
                            vt = Vw[WIN_IDX[kb_tok + cs]]
                            nc.tensor.matmul(
                                pnum[po:po + mm, :],
                                vt[:, voff:voff + mm],
                                aq[:, hh_loc, 128 * c:128 * (c + 1)],
                                start=(c == 0), stop=(c == nch - 1),
                                tile_position=(0, po))
                # Vector: den gather (both pairs at once), one reciprocal.
                # den layout: [a0 | a1 | b0 | b1] blocks of 128
                den = work.tile([1, 512], dt.float32, tag="den", bufs=3)
                nc.vector.tensor_copy(out=den[:, 0:256], in_=pav[32:33, 0:256])
                nc.vector.tensor_copy(out=den[:, 256:512], in_=pav[96:97, 0:256])
                rcd = work.tile([1, 512], dt.float32, tag="rcd", bufs=3)
                nc.vector.reciprocal_approx_fast(out=rcd, in_=den)
                rc16 = work.tile([1, 512], dt.bfloat16, tag="rc16", bufs=3)
                nc.vector.tensor_copy(out=rc16, in_=rcd)
                return (pav, rc16)

            def unit_pB(avout, u):
                """PE broadcast of reciprocals (f32 matmuls), one evacuation,
                one Vector normalize into attnP2."""
                g, quad = UNITS[u]
                pav, rc16 = avout
                for pl in range(2):
                    pB = pav[:97, 256 + 128 * pl:384 + 128 * pl]
                    nc.tensor.matmul(pB, psel_a,
                                     rc16[:, 128 * pl:128 * (pl + 1)],
                                     start=True, stop=False)
                    nc.tensor.matmul(pB, psel_b,
                                     rc16[:, 256 + 128 * pl:384 + 128 * pl],
                                     start=False, stop=True)
                rcb = work.tile([97, 256], dt.bfloat16, tag="rcb", bufs=3)
                nc.vector.tensor_copy(out=rcb, in_=pav[:97, 256:512])
                nc.vector.tensor_tensor(
                    out=attnP2[quad][:, :, 128 * g:128 * (g + 1)],
                    in0=pav[:97, 0:256].rearrange("p (two q) -> p two q", two=2),
                    in1=rcb.rearrange("p (two q) -> p two q", two=2),
                    op=Alu.mult)

            # software pipeline: PE order  Vw | S(u) | pB(u-2) | AV(u-1)
            # V windows land just before the units that need them, filling
            # the early-pipeline PE bubbles.
            VW_SCHED = {0: [0, 1], 1: [2, 3], 2: [4, 5], 3: [6, 7], 4: [8]}
            aqs = [None] * 8
            avouts = [None] * 8
            for u in range(8):
                for wi in VW_SCHED.get(u, []):
                    make_vwindow(wi)
                aqs[u] = unit_S(u)
                if u >= 2:
                    unit_pB(avouts[u - 2], u - 2)
                unit_emult(u, aqs[u])
                if u >= 1:
                    avouts[u - 1] = unit_AV(u - 1, aqs[u - 1])
            avouts[7] = unit_AV(7, aqs[7])
            unit_pB(avouts[6], 6)
            unit_pB(avouts[7], 7)

            # ---------------- proj + residual ----------------
            keepalive(5)
            y_tiles = []
            for mt in range(4):
                pt = ps.tile([128, 512], dt.float32, tag="pss", name=f"pproj{mt}")
                pp = pt[:, :DIM]
                for p in range(4):
                    nc.tensor.matmul(pp,
                                     attnP2[p // 2][:, p % 2, 128 * mt:128 * (mt + 1)],
                                     pwT_sb[:, p, :], start=(p == 0), stop=False)
                nc.tensor.matmul(pp, ones_sb[:, :128], pb_sb, start=False, stop=True)
                yt = persist.tile([128, DIM], dt.float32, tag=f"y{mt}")
                nc.vector.tensor_tensor(out=yt, in0=pp, in1=x_tiles[mt][:],
                                        op=Alu.add)
                y_tiles.append(yt)

            # gelu table preload (dummy) while LN2 runs
            gdummy = work.tile([1, 2], dt.float32, tag="gdummy")
            nc.scalar.activation(out=gdummy, in_=ones_sb[0:1, 0:2], func=Act.Gelu)

            # ---------------- LN2 + MLP ----------------
            x2T = make_tcat("x2T", NQ)
            xh2a = ln_batch(y_tiles[:2], [128, 128], "ln2a")
            xh2b = ln_batch(y_tiles[2:], [128, 128], "ln2b")
            keepalive(10)
            transpose_into(x2T, xh2a + xh2b, 0, [nc.vector])

            keepalive(4)
            m1 = []
            for mh in range(8):
                pt = ps.tile([128, 512], dt.float32, tag="pss", name=f"pfc1{mh}")
                for kb in range(2):
                    nc.tensor.matmul(pt, f1w_sb[:, kb, 128 * mh:128 * (mh + 1)],
                                     x2T[kb], start=(kb == 0), stop=(kb == 1))
                mg = persist.tile([128, NQ], dt.bfloat16, tag=f"m1_{mh}")
                nc.scalar.activation(out=mg, in_=pt, func=Act.Gelu,
                                     bias=f1b_sb[:, mh:mh + 1], scale=1.0)
                m1.append(mg)

            # fc2 mt-major: finish each output tile early, DMA out alternating
            for mt in range(4):
                pt = ps.tile([128, 512], dt.float32, tag="pss", name=f"pfc2{mt}")
                pp = pt[:, :DIM]
                for kb in range(8):
                    nc.tensor.matmul(pp, m1[kb][:, 128 * mt:128 * (mt + 1)],
                                     f2w_sb[:, kb, :], start=(kb == 0), stop=False)
                nc.tensor.matmul(pp, ones_sb[:, :128], f2b_sb, start=False, stop=True)
                ot = work.tile([128, DIM], dt.float32, tag="outt", bufs=2)
                nc.vector.tensor_tensor(out=ot, in0=pp, in1=y_tiles[mt][:],
                                        op=Alu.add)
                eng = nc.sync if mt % 2 == 0 else nc.gpsimd
                eng.dma_start(out=out_d[128 * mt:128 * (mt + 1), :], in_=ot)

    nc.finalize()
    return nc


# --------------------------------------------------------------------------
# Entry point
# --------------------------------------------------------------------------

def kernel(**inputs):
    from concourse.bass_utils import run_bass_kernel_spmd

    if 'nc' not in _CACHE:
        _CACHE['nc'] = build_graph()
    nc = _CACHE['nc']

    in_maps = _prepare_inputs(inputs)
    res = run_bass_kernel_spmd(nc, in_maps, core_ids=list(range(8)))
    x = np.asarray(inputs['x'])
    Bsz, Hh, Ww, C = x.shape
    out = np.zeros((Bsz, Hh, Ww, C), np.float32)
    for i in range(2 * Bsz):
        b, half = divmod(i, 2)
        o = np.asarray(res.results[i]['out']).reshape(16, Ww, C)
        if half == 0:
            out[b, 0:16] = o
        else:
            out[b, 16:32] = o[::-1]
    return out.astype(x.dtype)

